# revision 8
# baseline (speedup 1.0000x reference)
"""Multi-head attention (B=4, L=2048, C=1024, H=16, D=64) on 8 TRN2 NeuronCores.

Sharding: core c handles batch b = c//2 and head-group hg = c%2 (8 heads).
Megatron-style: w_qkv column-sharded, w_proj row-sharded; the proj all-reduce
(2 cores per batch) happens on the host during unshard.

All-bf16 dataflow (matmul cost model: 1.0 cyc/row at any free size; cost is
keyed on the moving operand's dtype; measured end-to-end error ~6e-3):

  A (per 512-l chunk): q/k proj for pair 0 first so phase B starts at ~5us,
     then V proj (V stored bf16 with a ones column for the softmax denom).
     RoPE via host-permuted w_qkv rows + quadrant stream_shuffle (as before),
     but in bf16.
  B (per pair, per 512-q tile):
     scores:  S^T[k128, q512] = kT.T @ qT per (kt, hd), one PSUM bank each
              (tile_position packs 2 heads on the 128 partitions).
     exp:     ScalarE Exp (scale 1/8 fused) PSUM -> persistent SBUF tile
              pt[128, 32, 512] bf16 (whole qtile, double-buffered).
     attn@V:  FLIPPED: O[q128, 65] += pt[:, s, qc*128:+128].T @ V'[k128, 65]
              accumulated over 16 kt (65 = 64 dims + ones column -> denom
              lands per-partition). Free size 65 halves PE cost vs the O^T
              form (cost model charges output free size only).
     norm:    reciprocal [128,1] + per-partition tensor_scalar mult -> bf16
              O_n[q,2,64]; then PE transpose (identity matmul) -> OT^T chunk;
              GPSIMD copies PSUM -> OT sbuf (OT kept fully in SBUF).
     A2 for later pairs interleaved at qtile boundaries.
  C (interleaved during last pair): out[l128, co512] partial over the 512
     local proj dims, accumulated over 4 pair-chunks, DMA'd per chunk.

PSUM budget (8 banks): ps_sc 2 tiles x 2 banks + misc pool 4 x 1 bank
(qkv-proj psum, AV accumulators, transpose dests, out-proj psum all share
misc as full-bank slots; ZERO_REGION = whole bank forbids co-tenancy with
any accumulating group).
"""

import sys

sys.path.insert(0, "/opt/trn_rl_repo")

import numpy as np

B, L, C, H, D = 4, 2048, 1024, 16, 64
NCORES = 8
QT = 512          # q-tile; one score mm per PSUM bank
GRP = 2           # score banks per exp group (exp ap = GRP*QT)
_built = {}


def _build(nc_mod):
    """Build the per-core Bass program (identical on all cores)."""
    import concourse.mybir as mybir
    import concourse.tile as tile
    from concourse import bacc
    from concourse.alu_op_type import AluOpType

    F32 = mybir.dt.float32
    BF16 = mybir.dt.bfloat16
    EXP = mybir.ActivationFunctionType.Exp
    MULT = AluOpType.mult
    ADD = AluOpType.add

    NKC = C // 128          # 8 contraction chunks for qkv proj
    NLT = L // 128          # 16 l-tiles (V rows, proj rows, k-chunks)
    NQT = L // QT           # 4 q-tiles per pair
    NPAIR = 4               # head pairs per core
    FV = 512                # v features per core
    VW = 65                 # V columns incl. ones
    NS = 2 * NLT            # 32 (kt, hd) score slices per qtile

    nc = bacc.Bacc(None, target_bir_lowering=False)

    xT_d = nc.dram_tensor("xT", [128, NKC, L], BF16, kind="ExternalInput")
    wqkT_d = nc.dram_tensor("wqkT", [8, 128, NKC, 128], BF16, kind="ExternalInput")
    wvT_d = nc.dram_tensor("wvT", [128, NKC, FV], BF16, kind="ExternalInput")
    wpT_d = nc.dram_tensor("wpT", [128, NPAIR, C], BF16, kind="ExternalInput")
    cos4_d = nc.dram_tensor("cos4", [128, L], BF16, kind="ExternalInput")
    sin4_d = nc.dram_tensor("sin4", [128, L], BF16, kind="ExternalInput")
    ident_d = nc.dram_tensor("ident", [128, 128], BF16, kind="ExternalInput")
    outp_d = nc.dram_tensor("outp", [NLT, 128, C], F32, kind="ExternalOutput")

    SWAP_MASK = list(range(16, 32)) + list(range(16))

    with tile.TileContext(nc) as tc:
        import contextlib

        with contextlib.ExitStack() as outer:
            persist = outer.enter_context(tc.tile_pool(name="persist", bufs=1))
            qk_pool = outer.enter_context(tc.tile_pool(name="qkt", bufs=4))
            pt_pool = outer.enter_context(tc.tile_pool(name="pt", bufs=2))
            on_pool = outer.enter_context(tc.tile_pool(name="on", bufs=3))
            rc_pool = outer.enter_context(tc.tile_pool(name="rc", bufs=4))
            tpool = outer.enter_context(tc.tile_pool(name="tmp", bufs=3))
            ob_pool = outer.enter_context(tc.tile_pool(name="ob", bufs=3))
            ps_sc = outer.enter_context(tc.tile_pool(name="ps_sc", bufs=2, space="PSUM"))
            ps_ms = outer.enter_context(tc.tile_pool(name="ps_ms", bufs=4, space="PSUM"))

            # ---- persistent tensors ----
            V_t = persist.tile([128, NLT, 8, VW], BF16, tag="V")
            OT_t = persist.tile([128, NPAIR, L], BF16, tag="OT")
            xT_t = persist.tile([128, NKC, L], BF16, tag="xT")
            wvT_t = persist.tile([128, NKC, FV], BF16, tag="wv")
            wpT_t = persist.tile([128, NPAIR, C], BF16, tag="wp")
            cos4_t = persist.tile([128, L], BF16, tag="cos")
            sin4_t = persist.tile([128, L], BF16, tag="sin")
            ident_t = persist.tile([128, 128], BF16, tag="id")
            ones_t = persist.tile([128, NLT, 8], BF16, tag="ones")

            # ---- input DMAs (front section; wpT deferred to phase C) ----
            wqk = {}

            def load_wqk(ft):
                wqk[ft] = qkw_pool.tile([128, NKC, 128], BF16, tag="wqk", name=f"wqk{ft}")
                nc.sync.dma_start(wqk[ft][:], wqkT_d[ft])

            qkw_pool = outer.enter_context(tc.tile_pool(name="qkw", bufs=4))
            load_wqk(0)
            load_wqk(4)
            for lq in range(NQT):
                sl = slice(lq * QT, (lq + 1) * QT)
                nc.sync.dma_start(xT_t[:, :, sl], xT_d[:, :, sl])
            nc.sync.dma_start(cos4_t[:], cos4_d[:])
            nc.sync.dma_start(sin4_t[:], sin4_d[:])
            nc.sync.dma_start(ident_t[:], ident_d[:])
            nc.sync.dma_start(wvT_t[:], wvT_d[:])

            nc.vector.memset(ones_t[:], 1.0)
            nc.vector.tensor_copy(V_t[:, :, :, 64:65], ones_t[:, :, :, None])

            # ---- A2 helper: qT/kT for one f-tile (one pair, q or k) ----
            qkT = {}

            def emit_qk_lq(ft, lq):
                """Project + rope one 512-l chunk of f-tile ft into qkT[ft]."""
                if ft not in qkT:
                    qkT[ft] = qk_pool.tile([128, L], BF16, tag="qkt", name=f"qkT{ft}")
                dst = qkT[ft]
                sl = slice(lq * QT, (lq + 1) * QT)
                qps = ps_ms.tile([128, QT], F32, tag="ms")
                for kc in range(NKC):
                    nc.tensor.matmul(
                        qps[:],
                        wqk[ft][:, kc, :],
                        xT_t[:, kc, sl],
                        start=(kc == 0),
                        stop=(kc == NKC - 1),
                    )
                # RoPE in bf16: dst = qb*cos4 + swap(qb)*sin4s
                qb = tpool.tile([128, QT], BF16, tag="qb")
                shufb = tpool.tile([128, QT], BF16, tag="shufb")
                nc.vector.tensor_copy(qb[:], qps[:])
                nc.vector.stream_shuffle(shufb[:], qb[:], SWAP_MASK)
                nc.vector.tensor_tensor(dst[:, sl], qb[:], cos4_t[:, sl], op=MULT)
                nc.vector.tensor_tensor(shufb[:], shufb[:], sin4_t[:, sl], op=MULT)
                nc.vector.tensor_tensor(dst[:, sl], dst[:, sl], shufb[:], op=ADD)

            def emit_qk(ft):
                for lq in range(NQT):
                    emit_qk_lq(ft, lq)

            # ---- phase A: pair-0 q/k first, then V proj ----
            for lq in range(NQT):
                emit_qk_lq(0, lq)
                emit_qk_lq(4, lq)
            for lt in range(NLT):
                vps = ps_ms.tile([128, FV], F32, tag="ms")
                for kc in range(NKC):
                    nc.tensor.matmul(
                        vps[:],
                        xT_t[:, kc, lt * 128:(lt + 1) * 128],
                        wvT_t[:, kc, :],
                        start=(kc == 0),
                        stop=(kc == NKC - 1),
                    )
                nc.vector.tensor_copy(V_t[:, lt, :, 0:64], vps[:])

            # wqk for later pairs (cheap DMAs, issued early; tiles persist in
            # qkw_pool rotation: 0,4 then 1,5 then 2,6 then 3,7 -> bufs=4
            # means ft0/4 slots are reused by ft2/6; emit order matches).
            nc.sync.dma_start(wpT_t[:], wpT_d[:])

            # ---- phase C helper: partial out-proj for one 128-l tile ----
            def emit_c(lt):
                lsl = slice(lt * 128, (lt + 1) * 128)
                for co in range(C // QT):
                    pps = ps_ms.tile([128, QT], F32, tag="ms")
                    for kd in range(NPAIR):
                        nc.tensor.matmul(
                            pps[:],
                            OT_t[:, kd, lsl],
                            wpT_t[:, kd, co * QT:(co + 1) * QT],
                            start=(kd == 0),
                            stop=(kd == NPAIR - 1),
                        )
                    ob = ob_pool.tile([128, QT], F32, tag="ob")
                    nc.vector.tensor_copy(ob[:], pps[:])
                    nc.sync.dma_start(outp_d[lt, :, co * QT:(co + 1) * QT], ob[:])

            def emit_transpose(pr, qt, qc, on):
                tp = ps_ms.tile([128, QT], F32, tag="ms")
                tpb = tp.bitcast(BF16)
                nc.tensor.transpose(
                    tpb[:, 0:128], on[:].rearrange("p a b -> p (a b)"), ident_t[:]
                )
                nc.vector.tensor_copy(
                    OT_t[:, pr, qt * QT + qc * 128: qt * QT + (qc + 1) * 128],
                    tpb[:, 0:128],
                )

            # ---- phase B: attention per (pair, q-tile) ----
            for pr in range(NPAIR):
                qT_t, kT_t = qkT[pr], qkT[4 + pr]
                for qt in range(NQT):
                    qsl = slice(qt * QT, (qt + 1) * QT)
                    pt = pt_pool.tile([128, NS, QT], BF16, tag="pt")
                    # scores + exp, GRP slices at a time
                    for g0 in range(0, NS, GRP):
                        sc = ps_sc.tile([128, GRP, QT], F32, tag="sc")
                        for j in range(GRP):
                            s = g0 + j
                            kt, hd = s // 2, s % 2
                            nc.tensor.matmul(
                                sc[:, j, :],
                                kT_t[hd * 64:(hd + 1) * 64, kt * 128:(kt + 1) * 128],
                                qT_t[hd * 64:(hd + 1) * 64, qsl],
                                start=True,
                                stop=True,
                                tile_position=(hd * 64, 0),
                            )
                        nc.scalar.activation(
                            pt[:, g0:g0 + GRP, :], sc[:], EXP, scale=float(D) ** -0.5
                        )
                    # flipped attn@V per 128-q chunk, then norm + transpose
                    tq = {}  # qc -> (On tile, transpose psum tile)
                    for qc in range(QT // 128):
                        av = {}
                        for hd in range(2):
                            av[hd] = ps_ms.tile([128, QT], F32, tag="ms", name=f"av{hd}")
                            for kt in range(NLT):
                                nc.tensor.matmul(
                                    av[hd][:, 0:VW],
                                    pt[:, 2 * kt + hd, qc * 128:(qc + 1) * 128],
                                    V_t[:, kt, pr * 2 + hd, :],
                                    start=(kt == 0),
                                    stop=(kt == NLT - 1),
                                )
                        # norm on DVE: per-partition scalar multiply
                        on = on_pool.tile([128, 2, 64], BF16, tag="on")
                        for hd in range(2):
                            rc = rc_pool.tile([128, 1], F32, tag="rc")
                            nc.vector.reciprocal(rc[:], av[hd][:, 64:65])
                            nc.vector.tensor_scalar(
                                on[:, hd, :], av[hd][:, 0:64], rc[:], None, op0=MULT
                            )
                        tq[qc] = on
                        # transpose previous chunk (deferred one chunk to let
                        # the DVE norm drain without stalling the PE queue)
                        if qc > 0:
                            emit_transpose(pr, qt, qc - 1, tq[qc - 1])
                    emit_transpose(pr, qt, QT // 128 - 1, tq[QT // 128 - 1])
                    # interleave A2 for later pairs / phase C for last pair
                    if pr < NPAIR - 1:
                        if qt == 1:
                            load_wqk(pr + 1)
                            emit_qk(pr + 1)
                        if qt == 3:
                            load_wqk(5 + pr)
                            emit_qk(5 + pr)
                    else:
                        for lt in range(qt * NQT, (qt + 1) * NQT):
                            emit_c(lt)

    nc.compile()
    return nc


def _get_nc():
    if "nc" not in _built:
        _built["nc"] = _build(None)
    return _built["nc"]


def _rope_perm():
    """Within-head row permutation: quadrant-local [evens(16) | odds(16)]."""
    perm = np.empty(64, np.int64)
    for j in range(2):
        for i in range(32):
            perm[j * 32 + i] = 2 * (j * 16 + i) if i < 16 else 2 * (j * 16 + i - 16) + 1
    return perm


def _shard_inputs(x, cos, sin, w_qkv, w_proj):
    import ml_dtypes

    Bb = ml_dtypes.bfloat16
    perm = _rope_perm()
    p = np.arange(128)
    quad, i = p // 32, p % 32
    pairidx = (quad % 2) * 16 + (i % 16)
    sign = np.where(i < 16, -1.0, 1.0).astype(np.float32)
    cos4 = np.ascontiguousarray(cos[:, pairidx].T).astype(Bb)              # [128, L]
    sin4 = np.ascontiguousarray((sin[:, pairidx] * sign[None, :]).T).astype(Bb)
    ident = np.eye(128, dtype=np.float32).astype(Bb)

    in_maps = []
    for c in range(NCORES):
        b, hg = c // 2, c % 2
        xT = np.ascontiguousarray(
            x[b].T.reshape(C // 128, 128, L).transpose(1, 0, 2)
        ).astype(Bb)  # [p, kc, l]

        rows = np.empty((8, 128), np.int64)
        for ft in range(8):
            t = 0 if ft < 4 else 1
            pr = ft % 4
            for fi in range(128):
                head = hg * 8 + 2 * pr + (0 if fi < 64 else 1)
                rows[ft, fi] = t * C + head * D + perm[fi % 64]
        wq = w_qkv[rows.reshape(-1)].reshape(8, 128, C // 128, 128)  # [ft, f, kc, p]
        wqkT = np.ascontiguousarray(wq.transpose(0, 3, 2, 1)).astype(Bb)  # [ft,p,kc,f]

        wv = w_qkv[2 * C + hg * 512: 2 * C + hg * 512 + 512]         # [fv, c]
        wvT = np.ascontiguousarray(
            wv.T.reshape(C // 128, 128, 512).transpose(1, 0, 2)
        ).astype(Bb)  # [p, kc, fv]

        wp = w_proj[:, hg * 512: hg * 512 + 512]                     # [co, d']
        wpT = np.ascontiguousarray(
            wp.T.reshape(4, 128, C).transpose(1, 0, 2)
        ).astype(Bb)  # [p, kd, co]

        in_maps.append(
            {
                "xT": xT, "wqkT": wqkT, "wvT": wvT, "wpT": wpT,
                "cos4": cos4, "sin4": sin4, "ident": ident,
            }
        )
    return in_maps


def kernel(x, cos, sin, w_qkv, w_proj, b_proj, _trace=False):
    from concourse.bass_utils import run_bass_kernel_spmd

    x = np.asarray(x, dtype=np.float32)
    cos = np.asarray(cos, dtype=np.float32)
    sin = np.asarray(sin, dtype=np.float32)
    w_qkv = np.asarray(w_qkv, dtype=np.float32)
    w_proj = np.asarray(w_proj, dtype=np.float32)
    b_proj = np.asarray(b_proj, dtype=np.float32)

    nc = _get_nc()
    in_maps = _shard_inputs(x, cos, sin, w_qkv, w_proj)
    res = run_bass_kernel_spmd(
        nc, in_maps, core_ids=list(range(NCORES)), trace=_trace
    )
    if _trace:
        print("exec_time_ns:", res.exec_time_ns)

    out = np.empty((B, L, C), dtype=np.float32)
    for b in range(B):
        p0 = res.results[2 * b]["outp"].reshape(L, C)
        p1 = res.results[2 * b + 1]["outp"].reshape(L, C)
        out[b] = p0 + p1
    out += b_proj[None, None, :]
    return out


# revision 13
# speedup vs baseline: 1.2984x; 1.2984x over previous
"""Multi-head attention (B=4, L=2048, C=1024, H=16, D=64) on 8 TRN2 NeuronCores.

Sharding: core c handles batch b = c//2 and head-group hg = c%2 (8 heads).
Megatron-style: w_qkv column-sharded, w_proj row-sharded; the proj all-reduce
(2 cores per batch) happens on the host during unshard.

All-bf16 dataflow (matmul cost model: 1.0 cyc/row at any free size; cost is
keyed on the moving operand's dtype; measured end-to-end error ~6e-3):

  A (per 512-l chunk): q/k proj for pair 0 first so phase B starts at ~5us,
     then V proj (V stored bf16 with a ones column for the softmax denom).
     RoPE via host-permuted w_qkv rows + quadrant stream_shuffle (as before),
     but in bf16.
  B (per pair, per 512-q tile):
     scores:  S^T[k128, q512] = kT.T @ qT per (kt, hd), one PSUM bank each
              (tile_position packs 2 heads on the 128 partitions).
     exp:     ScalarE Exp (scale 1/8 fused) PSUM -> persistent SBUF tile
              pt[128, 32, 512] bf16 (whole qtile, double-buffered).
     attn@V:  FLIPPED: O[q128, 65] += pt[:, s, qc*128:+128].T @ V'[k128, 65]
              accumulated over 16 kt (65 = 64 dims + ones column -> denom
              lands per-partition). Free size 65 halves PE cost vs the O^T
              form (cost model charges output free size only).
     norm:    reciprocal [128,1] + per-partition tensor_scalar mult -> bf16
              O_n[q,2,64]; then PE transpose (identity matmul) -> OT^T chunk;
              GPSIMD copies PSUM -> OT sbuf (OT kept fully in SBUF).
     A2 for later pairs interleaved at qtile boundaries.
  C (interleaved during last pair): out[l128, co512] partial over the 512
     local proj dims, accumulated over 4 pair-chunks, DMA'd per chunk.

PSUM budget (8 banks): ps_sc 2 tiles x 2 banks + misc pool 4 x 1 bank
(qkv-proj psum, AV accumulators, transpose dests, out-proj psum all share
misc as full-bank slots; ZERO_REGION = whole bank forbids co-tenancy with
any accumulating group).
"""

import sys

sys.path.insert(0, "/opt/trn_rl_repo")

import numpy as np

B, L, C, H, D = 4, 2048, 1024, 16, 64
NCORES = 8
QT = 512          # q-tile; one score mm per PSUM bank
GRP = 2           # score banks per exp group (exp ap = GRP*QT)
_built = {}


def _build(nc_mod):
    """Build the per-core Bass program (identical on all cores)."""
    import concourse.mybir as mybir
    import concourse.tile as tile
    from concourse import bacc
    from concourse.alu_op_type import AluOpType

    F32 = mybir.dt.float32
    BF16 = mybir.dt.bfloat16
    EXP = mybir.ActivationFunctionType.Exp
    MULT = AluOpType.mult
    ADD = AluOpType.add

    NKC = C // 128          # 8 contraction chunks for qkv proj
    NLT = L // 128          # 16 l-tiles (V rows, proj rows, k-chunks)
    NQT = L // QT           # 4 q-tiles per pair
    NPAIR = 4               # head pairs per core
    FV = 512                # v features per core
    VW = 65                 # V columns incl. ones
    NS = 2 * NLT            # 32 (kt, hd) score slices per qtile

    nc = bacc.Bacc(None, target_bir_lowering=False)

    xT_d = nc.dram_tensor("xT", [128, NKC, L], BF16, kind="ExternalInput")
    wqkT_d = nc.dram_tensor("wqkT", [8, 128, NKC, 128], BF16, kind="ExternalInput")
    wvT_d = nc.dram_tensor("wvT", [128, NKC, FV], BF16, kind="ExternalInput")
    wpT_d = nc.dram_tensor("wpT", [128, NPAIR, C], BF16, kind="ExternalInput")
    cos4_d = nc.dram_tensor("cos4", [128, L], BF16, kind="ExternalInput")
    sin4_d = nc.dram_tensor("sin4", [128, L], BF16, kind="ExternalInput")
    ident_d = nc.dram_tensor("ident", [128, 128], BF16, kind="ExternalInput")
    outp_d = nc.dram_tensor("outp", [NLT, 128, C], F32, kind="ExternalOutput")

    SWAP_MASK = list(range(16, 32)) + list(range(16))

    with tile.TileContext(nc) as tc:
        import contextlib

        with contextlib.ExitStack() as outer:
            persist = outer.enter_context(tc.tile_pool(name="persist", bufs=1))
            qk_pool = outer.enter_context(tc.tile_pool(name="qkt", bufs=4))
            pt_pool = outer.enter_context(tc.tile_pool(name="pt", bufs=2))
            on_pool = outer.enter_context(tc.tile_pool(name="on", bufs=3))
            rc_pool = outer.enter_context(tc.tile_pool(name="rc", bufs=4))
            tpool = outer.enter_context(tc.tile_pool(name="tmp", bufs=3))
            ob_pool = outer.enter_context(tc.tile_pool(name="ob", bufs=3))
            ps_sc = outer.enter_context(tc.tile_pool(name="ps_sc", bufs=2, space="PSUM"))
            ps_ms = outer.enter_context(tc.tile_pool(name="ps_ms", bufs=4, space="PSUM"))

            # ---- persistent tensors ----
            V_t = persist.tile([128, NLT, 8, VW], BF16, tag="V")
            OT_t = persist.tile([128, NPAIR, L], BF16, tag="OT")
            xT_t = persist.tile([128, NKC, L], BF16, tag="xT")
            wvT_t = persist.tile([128, NKC, FV], BF16, tag="wv")
            wpT_t = persist.tile([128, NPAIR, C], BF16, tag="wp")
            cos4_t = persist.tile([128, L], BF16, tag="cos")
            sin4_t = persist.tile([128, L], BF16, tag="sin")
            ident_t = persist.tile([128, 128], BF16, tag="id")
            ones_t = persist.tile([128, NLT, 8], BF16, tag="ones")

            # ---- input DMAs (front section; wpT deferred to phase C) ----
            wqk = {}

            def load_wqk(ft):
                wqk[ft] = qkw_pool.tile([128, NKC, 128], BF16, tag="wqk", name=f"wqk{ft}")
                nc.sync.dma_start(wqk[ft][:], wqkT_d[ft])

            qkw_pool = outer.enter_context(tc.tile_pool(name="qkw", bufs=4))
            load_wqk(0)
            load_wqk(4)
            nc.sync.dma_start(xT_t[:, 0:4, 0:QT], xT_d[:, 0:4, 0:QT])
            nc.sync.dma_start(xT_t[:, 4:8, 0:QT], xT_d[:, 4:8, 0:QT])
            nc.sync.dma_start(cos4_t[:], cos4_d[:])
            nc.sync.dma_start(sin4_t[:], sin4_d[:])
            for lq in range(1, NQT):
                sl = slice(lq * QT, (lq + 1) * QT)
                nc.sync.dma_start(xT_t[:, :, sl], xT_d[:, :, sl])
            nc.sync.dma_start(wvT_t[:], wvT_d[:])
            nc.sync.dma_start(ident_t[:], ident_d[:])

            nc.vector.memset(ones_t[:], 1.0)
            nc.vector.tensor_copy(V_t[:, :, :, 64:65], ones_t[:, :, :, None])

            # ---- A2 helper: qT/kT for one f-tile (one pair, q or k) ----
            qkT = {}

            def emit_qk_lq(ft, lq):
                """Project + rope one 512-l chunk of f-tile ft into qkT[ft]."""
                if ft not in qkT:
                    qkT[ft] = qk_pool.tile([128, L], BF16, tag="qkt", name=f"qkT{ft}")
                dst = qkT[ft]
                sl = slice(lq * QT, (lq + 1) * QT)
                qps = ps_ms.tile([128, QT], F32, tag="ms", name="qps")
                for kc in range(NKC):
                    nc.tensor.matmul(
                        qps[:],
                        wqk[ft][:, kc, :],
                        xT_t[:, kc, sl],
                        start=(kc == 0),
                        stop=(kc == NKC - 1),
                    )
                # RoPE in bf16: dst = qb*cos4 + swap(qb)*sin4s
                qb = tpool.tile([128, QT], BF16, tag="qb")
                shufb = tpool.tile([128, QT], BF16, tag="shufb")
                nc.vector.tensor_copy(qb[:], qps[:])
                nc.vector.stream_shuffle(shufb[:], qb[:], SWAP_MASK)
                nc.vector.tensor_tensor(dst[:, sl], qb[:], cos4_t[:, sl], op=MULT)
                nc.vector.tensor_tensor(shufb[:], shufb[:], sin4_t[:, sl], op=MULT)
                nc.vector.tensor_tensor(dst[:, sl], dst[:, sl], shufb[:], op=ADD)

            def emit_a1_lt(lt):
                vps = ps_ms.tile([128, FV], F32, tag="ms", name="vps")
                for kc in range(NKC):
                    nc.tensor.matmul(
                        vps[:],
                        xT_t[:, kc, lt * 128:(lt + 1) * 128],
                        wvT_t[:, kc, :],
                        start=(kc == 0),
                        stop=(kc == NKC - 1),
                    )
                nc.vector.tensor_copy(V_t[:, lt, :, 0:64], vps[:])

            nc.sync.dma_start(wpT_t[:], wpT_d[:])

            # ---- phase C helper: half out-proj (one co) for one 128-l tile ----
            def emit_c(lt, co):
                lsl = slice(lt * 128, (lt + 1) * 128)
                pps = ps_ms.tile([128, QT], F32, tag="ms", name="pps")
                for kd in range(NPAIR):
                    nc.tensor.matmul(
                        pps[:],
                        OT_t[:, kd, lsl],
                        wpT_t[:, kd, co * QT:(co + 1) * QT],
                        start=(kd == 0),
                        stop=(kd == NPAIR - 1),
                    )
                ob = ob_pool.tile([128, QT], F32, tag="ob")
                nc.vector.tensor_copy(ob[:], pps[:])
                nc.sync.dma_start(outp_d[lt, :, co * QT:(co + 1) * QT], ob[:])

            def emit_av(pr, qt, qc, pt, tq):
                """Flipped attn@V for one 128-q chunk (both heads) + norm."""
                av = {}
                for hd in range(2):
                    av[hd] = ps_ms.tile([128, QT], F32, tag="ms", name=f"av{hd}")
                    for kt in range(NLT):
                        nc.tensor.matmul(
                            av[hd][:, 0:VW],
                            pt[:, 2 * kt + hd, qc * 128:(qc + 1) * 128],
                            V_t[:, kt, pr * 2 + hd, :],
                            start=(kt == 0),
                            stop=(kt == NLT - 1),
                        )
                on = on_pool.tile([128, 2, 64], BF16, tag="on")
                for hd in range(2):
                    rc = rc_pool.tile([128, 1], F32, tag="rc")
                    nc.vector.reciprocal(rc[:], av[hd][:, 64:65])
                    nc.vector.tensor_scalar(
                        on[:, hd, :], av[hd][:, 0:64], rc[:], None, op0=MULT
                    )
                tq[qc] = on

            def emit_transpose(pr, qt, qc, on):
                tp = ps_ms.tile([128, QT], F32, tag="ms", name="tp")
                tpb = tp.bitcast(BF16)
                nc.tensor.transpose(
                    tpb[:, 0:128], on[:].rearrange("p a b -> p (a b)"), ident_t[:]
                )
                nc.vector.tensor_copy(
                    OT_t[:, pr, qt * QT + qc * 128: qt * QT + (qc + 1) * 128],
                    tpb[:, 0:128],
                )

            # ---- interleaved emission: weave filler PE chunks between score
            # groups so the Activation engine (the per-qtile bottleneck) never
            # starves behind the in-order PE stream.
            import collections

            fillers = collections.deque()  # (cost_cycles, fn, epoch)
            debt = [0.0]
            PUMP = 1400.0  # PE filler cycles per score group (Act group ~1.04us)

            def pump():
                debt[0] += PUMP
                while fillers and debt[0] >= fillers[0][0]:
                    c, fn, _ = fillers.popleft()
                    fn()
                    debt[0] -= c

            def drain_upto(ep):
                while fillers and fillers[0][2] <= ep:
                    c, fn, _ = fillers.popleft()
                    fn()
                debt[0] = 0.0

            def drain_all():
                drain_upto(1 << 30)

            # A1 V-proj chunks: FIFO-ahead of qt0's AV, deadline epoch 0
            for lt in range(NLT):
                fillers.append((4096, (lambda lt=lt: emit_a1_lt(lt)), 0))

            # ---- phase B driver ----
            for pr in range(NPAIR):
                for qt in range(NQT):
                    ep = pr * NQT + qt
                    drain_upto(ep - 2)
                    if pr == 0 and qt == 0:
                        emit_qk_lq(0, 0)
                        emit_qk_lq(4, 0)
                    qT_t, kT_t = qkT[pr], qkT[4 + pr]
                    qsl = slice(qt * QT, (qt + 1) * QT)
                    pt = pt_pool.tile([128, NS, QT], BF16, tag="pt", name="pt")
                    for g0 in range(0, NS, GRP):
                        if pr == 0 and qt == 0 and g0 // 2 in (1, 3, 5):
                            # stream pair-0 q/k projection just ahead of the
                            # score chunks that consume it
                            lq = {1: 1, 3: 2, 5: 3}[g0 // 2]
                            emit_qk_lq(0, lq)
                            emit_qk_lq(4, lq)
                        sc = ps_sc.tile([128, GRP, QT], F32, tag="sc")
                        for j in range(GRP):
                            s = g0 + j
                            kt, hd = s // 2, s % 2
                            nc.tensor.matmul(
                                sc[:, j, :],
                                kT_t[hd * 64:(hd + 1) * 64, kt * 128:(kt + 1) * 128],
                                qT_t[hd * 64:(hd + 1) * 64, qsl],
                                start=True,
                                stop=True,
                                tile_position=(hd * 64, 0),
                            )
                        nc.scalar.activation(
                            pt[:, g0:g0 + GRP, :], sc[:], EXP, scale=float(D) ** -0.5
                        )
                        if not (pr == 0 and qt == 0):
                            pump()
                    # queue this qtile's AV + norm + transpose (+ C for pr3):
                    # they weave through the next qtile's score stream and must
                    # be emitted before epoch ep+2 reuses the pt buffer.
                    tq = {}
                    for qc in range(QT // 128):
                        fillers.append(
                            (2080, (lambda pr=pr, qt=qt, qc=qc, pt=pt, tq=tq:
                                    emit_av(pr, qt, qc, pt, tq)), ep)
                        )
                        if qc > 0:
                            fillers.append(
                                (150, (lambda pr=pr, qt=qt, qc=qc, tq=tq:
                                       emit_transpose(pr, qt, qc - 1, tq[qc - 1])), ep)
                            )
                            if pr == NPAIR - 1:
                                lt = qt * NQT + qc - 1
                                for co in range(C // QT):
                                    fillers.append(
                                        (2048, (lambda lt=lt, co=co: emit_c(lt, co)), ep)
                                    )
                    fillers.append(
                        (150, (lambda pr=pr, qt=qt, tq=tq:
                               emit_transpose(pr, qt, NQT - 1, tq[NQT - 1])), ep)
                    )
                    if pr == NPAIR - 1:
                        lt = qt * NQT + NQT - 1
                        for co in range(C // QT):
                            fillers.append(
                                (2048, (lambda lt=lt, co=co: emit_c(lt, co)), ep)
                            )
                    # queue A2 q/k chunks for the next pair (deadline: both
                    # tiles fully emitted before that pair's first scores)
                    if pr < NPAIR - 1:
                        if qt == 0:
                            load_wqk(pr + 1)
                            for lq in range(NQT):
                                fillers.append(
                                    (4300, (lambda ft=pr + 1, lq=lq: emit_qk_lq(ft, lq)), ep)
                                )
                        if qt == 1:
                            load_wqk(5 + pr)
                            for lq in range(NQT):
                                fillers.append(
                                    (4300, (lambda ft=5 + pr, lq=lq: emit_qk_lq(ft, lq)), ep)
                                )
            drain_all()

    nc.compile()
    return nc


def _get_nc():
    if "nc" not in _built:
        _built["nc"] = _build(None)
    return _built["nc"]


def _rope_perm():
    """Within-head row permutation: quadrant-local [evens(16) | odds(16)]."""
    perm = np.empty(64, np.int64)
    for j in range(2):
        for i in range(32):
            perm[j * 32 + i] = 2 * (j * 16 + i) if i < 16 else 2 * (j * 16 + i - 16) + 1
    return perm


def _shard_inputs(x, cos, sin, w_qkv, w_proj):
    import ml_dtypes

    Bb = ml_dtypes.bfloat16
    perm = _rope_perm()
    p = np.arange(128)
    quad, i = p // 32, p % 32
    pairidx = (quad % 2) * 16 + (i % 16)
    sign = np.where(i < 16, -1.0, 1.0).astype(np.float32)
    cos4 = np.ascontiguousarray(cos[:, pairidx].T).astype(Bb)              # [128, L]
    sin4 = np.ascontiguousarray((sin[:, pairidx] * sign[None, :]).T).astype(Bb)
    ident = np.eye(128, dtype=np.float32).astype(Bb)

    in_maps = []
    for c in range(NCORES):
        b, hg = c // 2, c % 2
        xT = np.ascontiguousarray(
            x[b].T.reshape(C // 128, 128, L).transpose(1, 0, 2)
        ).astype(Bb)  # [p, kc, l]

        rows = np.empty((8, 128), np.int64)
        for ft in range(8):
            t = 0 if ft < 4 else 1
            pr = ft % 4
            for fi in range(128):
                head = hg * 8 + 2 * pr + (0 if fi < 64 else 1)
                rows[ft, fi] = t * C + head * D + perm[fi % 64]
        wq = w_qkv[rows.reshape(-1)].reshape(8, 128, C // 128, 128)  # [ft, f, kc, p]
        wqkT = np.ascontiguousarray(wq.transpose(0, 3, 2, 1)).astype(Bb)  # [ft,p,kc,f]

        wv = w_qkv[2 * C + hg * 512: 2 * C + hg * 512 + 512]         # [fv, c]
        wvT = np.ascontiguousarray(
            wv.T.reshape(C // 128, 128, 512).transpose(1, 0, 2)
        ).astype(Bb)  # [p, kc, fv]

        wp = w_proj[:, hg * 512: hg * 512 + 512]                     # [co, d']
        wpT = np.ascontiguousarray(
            wp.T.reshape(4, 128, C).transpose(1, 0, 2)
        ).astype(Bb)  # [p, kd, co]

        in_maps.append(
            {
                "xT": xT, "wqkT": wqkT, "wvT": wvT, "wpT": wpT,
                "cos4": cos4, "sin4": sin4, "ident": ident,
            }
        )
    return in_maps


def kernel(x, cos, sin, w_qkv, w_proj, b_proj, _trace=False):
    from concourse.bass_utils import run_bass_kernel_spmd

    x = np.asarray(x, dtype=np.float32)
    cos = np.asarray(cos, dtype=np.float32)
    sin = np.asarray(sin, dtype=np.float32)
    w_qkv = np.asarray(w_qkv, dtype=np.float32)
    w_proj = np.asarray(w_proj, dtype=np.float32)
    b_proj = np.asarray(b_proj, dtype=np.float32)

    nc = _get_nc()
    in_maps = _shard_inputs(x, cos, sin, w_qkv, w_proj)
    res = run_bass_kernel_spmd(
        nc, in_maps, core_ids=list(range(NCORES)), trace=_trace
    )
    if _trace:
        print("exec_time_ns:", res.exec_time_ns)

    out = np.empty((B, L, C), dtype=np.float32)
    for b in range(B):
        p0 = res.results[2 * b]["outp"].reshape(L, C)
        p1 = res.results[2 * b + 1]["outp"].reshape(L, C)
        out[b] = p0 + p1
    out += b_proj[None, None, :]
    return out


# revision 18
# speedup vs baseline: 1.3299x; 1.0243x over previous
"""Multi-head attention (B=4, L=2048, C=1024, H=16, D=64) on 8 TRN2 NeuronCores.

Sharding: core c handles batch b = c//2 and head-group hg = c%2 (8 heads).
Megatron-style: w_qkv column-sharded, w_proj row-sharded; the proj all-reduce
(2 cores per batch) happens on the host during unshard.

All-bf16 dataflow (matmul cost model: 1.0 cyc/row at any free size; cost is
keyed on the moving operand's dtype; measured end-to-end error ~6e-3):

  A (per 512-l chunk): q/k proj for pair 0 first so phase B starts at ~5us,
     then V proj (V stored bf16 with a ones column for the softmax denom).
     RoPE via host-permuted w_qkv rows + quadrant stream_shuffle (as before),
     but in bf16.
  B (per pair, per 512-q tile):
     scores:  S^T[k128, q512] = kT.T @ qT per (kt, hd), one PSUM bank each
              (tile_position packs 2 heads on the 128 partitions).
     exp:     ScalarE Exp (scale 1/8 fused) PSUM -> persistent SBUF tile
              pt[128, 32, 512] bf16 (whole qtile, double-buffered).
     attn@V:  FLIPPED: O[q128, 65] += pt[:, s, qc*128:+128].T @ V'[k128, 65]
              accumulated over 16 kt (65 = 64 dims + ones column -> denom
              lands per-partition). Free size 65 halves PE cost vs the O^T
              form (cost model charges output free size only).
     norm:    reciprocal [128,1] + per-partition tensor_scalar mult -> bf16
              O_n[q,2,64]; then PE transpose (identity matmul) -> OT^T chunk;
              GPSIMD copies PSUM -> OT sbuf (OT kept fully in SBUF).
     A2 for later pairs interleaved at qtile boundaries.
  C (interleaved during last pair): out[l128, co512] partial over the 512
     local proj dims, accumulated over 4 pair-chunks, DMA'd per chunk.

PSUM budget (8 banks): ps_sc 2 tiles x 2 banks + misc pool 4 x 1 bank
(qkv-proj psum, AV accumulators, transpose dests, out-proj psum all share
misc as full-bank slots; ZERO_REGION = whole bank forbids co-tenancy with
any accumulating group).
"""

import sys

sys.path.insert(0, "/opt/trn_rl_repo")

import numpy as np

B, L, C, H, D = 4, 2048, 1024, 16, 64
NCORES = 8
QT = 512          # q-tile; one score mm per PSUM bank
GRP = 2           # score banks per exp group (exp ap = GRP*QT)
_built = {}


def _build(nc_mod):
    """Build the per-core Bass program (identical on all cores)."""
    import concourse.mybir as mybir
    import concourse.tile as tile
    from concourse import bacc
    from concourse.alu_op_type import AluOpType

    F32 = mybir.dt.float32
    BF16 = mybir.dt.bfloat16
    EXP = mybir.ActivationFunctionType.Exp
    MULT = AluOpType.mult
    ADD = AluOpType.add

    NKC = C // 128          # 8 contraction chunks for qkv proj
    NLT = L // 128          # 16 l-tiles (V rows, proj rows, k-chunks)
    NQT = L // QT           # 4 q-tiles per pair
    NPAIR = 4               # head pairs per core
    FV = 512                # v features per core
    VW = 65                 # V columns incl. ones
    NS = 2 * NLT            # 32 (kt, hd) score slices per qtile

    nc = bacc.Bacc(None, target_bir_lowering=False)

    xT_d = nc.dram_tensor("xT", [128, NKC, L], BF16, kind="ExternalInput")
    wqkT_d = nc.dram_tensor("wqkT", [8, 128, NKC, 128], BF16, kind="ExternalInput")
    wvT_d = nc.dram_tensor("wvT", [128, NKC, FV], BF16, kind="ExternalInput")
    wpT_d = nc.dram_tensor("wpT", [128, NPAIR, C], BF16, kind="ExternalInput")
    cos4_d = nc.dram_tensor("cos4", [128, L], BF16, kind="ExternalInput")
    sin4_d = nc.dram_tensor("sin4", [128, L], BF16, kind="ExternalInput")
    ident_d = nc.dram_tensor("ident", [128, 128], BF16, kind="ExternalInput")
    outp_d = nc.dram_tensor("outp", [NLT, 128, C], F32, kind="ExternalOutput")

    SWAP_MASK = list(range(16, 32)) + list(range(16))

    with tile.TileContext(nc) as tc:
        import contextlib

        with contextlib.ExitStack() as outer:
            persist = outer.enter_context(tc.tile_pool(name="persist", bufs=1))
            qk_pool = outer.enter_context(tc.tile_pool(name="qkt", bufs=4))
            pt_pool = outer.enter_context(tc.tile_pool(name="pt", bufs=2))
            on_pool = outer.enter_context(tc.tile_pool(name="on", bufs=3))
            rc_pool = outer.enter_context(tc.tile_pool(name="rc", bufs=4))
            tpool = outer.enter_context(tc.tile_pool(name="tmp", bufs=3))
            ob_pool = outer.enter_context(tc.tile_pool(name="ob", bufs=3))
            ps_sc = outer.enter_context(tc.tile_pool(name="ps_sc", bufs=2, space="PSUM"))
            ps_ms = outer.enter_context(tc.tile_pool(name="ps_ms", bufs=4, space="PSUM"))

            # ---- persistent tensors ----
            V_t = persist.tile([128, NLT, 8, VW], BF16, tag="V")
            OT_t = persist.tile([128, NPAIR, L], BF16, tag="OT")
            xT_t = persist.tile([128, NKC, L], BF16, tag="xT")
            wvT_t = persist.tile([128, NKC, FV], BF16, tag="wv")
            wpT_t = persist.tile([128, NPAIR, C], BF16, tag="wp")
            cos4_t = persist.tile([128, L], BF16, tag="cos")
            sin4_t = persist.tile([128, L], BF16, tag="sin")
            ident_t = persist.tile([128, 128], BF16, tag="id")
            ones_t = persist.tile([128, NLT, 8], BF16, tag="ones")

            # ---- input DMAs (front section; wpT deferred to phase C) ----
            wqk = {}

            def load_wqk(ft):
                wqk[ft] = qkw_pool.tile([128, NKC, 128], BF16, tag="wqk", name=f"wqk{ft}")
                nc.sync.dma_start(wqk[ft][:], wqkT_d[ft])

            qkw_pool = outer.enter_context(tc.tile_pool(name="qkw", bufs=4))
            load_wqk(0)
            load_wqk(4)
            nc.sync.dma_start(xT_t[:, 0:4, 0:QT], xT_d[:, 0:4, 0:QT])
            nc.sync.dma_start(xT_t[:, 4:8, 0:QT], xT_d[:, 4:8, 0:QT])
            nc.sync.dma_start(cos4_t[:], cos4_d[:])
            nc.sync.dma_start(sin4_t[:], sin4_d[:])
            for lq in range(1, NQT):
                sl = slice(lq * QT, (lq + 1) * QT)
                nc.sync.dma_start(xT_t[:, :, sl], xT_d[:, :, sl])
            nc.sync.dma_start(wvT_t[:], wvT_d[:])
            nc.sync.dma_start(ident_t[:], ident_d[:])

            nc.vector.memset(ones_t[:], 1.0)
            nc.vector.tensor_copy(V_t[:, :, :, 64:65], ones_t[:, :, :, None])

            # ---- A2 helper: qT/kT for one f-tile (one pair, q or k) ----
            qkT = {}

            def emit_qk_lq(ft, lq):
                """Project + rope one 512-l chunk of f-tile ft into qkT[ft]."""
                if ft not in qkT:
                    qkT[ft] = qk_pool.tile([128, L], BF16, tag="qkt", name=f"qkT{ft}")
                dst = qkT[ft]
                sl = slice(lq * QT, (lq + 1) * QT)
                qps = ps_ms.tile([128, QT], F32, tag="ms", name="qps")
                for kc in range(NKC):
                    nc.tensor.matmul(
                        qps[:],
                        wqk[ft][:, kc, :],
                        xT_t[:, kc, sl],
                        start=(kc == 0),
                        stop=(kc == NKC - 1),
                    )
                # RoPE in bf16: dst = qb*cos4 + swap(qb)*sin4s
                qb = tpool.tile([128, QT], BF16, tag="qb")
                shufb = tpool.tile([128, QT], BF16, tag="shufb")
                nc.vector.tensor_copy(qb[:], qps[:])
                nc.vector.stream_shuffle(shufb[:], qb[:], SWAP_MASK)
                nc.vector.tensor_tensor(dst[:, sl], qb[:], cos4_t[:, sl], op=MULT)
                nc.vector.tensor_tensor(shufb[:], shufb[:], sin4_t[:, sl], op=MULT)
                nc.vector.tensor_tensor(dst[:, sl], dst[:, sl], shufb[:], op=ADD)

            def emit_a1_lt(lt):
                vps = ps_ms.tile([128, FV], F32, tag="ms", name="vps")
                for kc in range(NKC):
                    nc.tensor.matmul(
                        vps[:],
                        xT_t[:, kc, lt * 128:(lt + 1) * 128],
                        wvT_t[:, kc, :],
                        start=(kc == 0),
                        stop=(kc == NKC - 1),
                    )
                nc.vector.tensor_copy(V_t[:, lt, :, 0:64], vps[:])

            nc.sync.dma_start(wpT_t[:], wpT_d[:])

            # ---- phase C helper: half out-proj (one co) for one 128-l tile ----
            def emit_c(lt, co):
                lsl = slice(lt * 128, (lt + 1) * 128)
                pps = ps_ms.tile([128, QT], F32, tag="ms", name="pps")
                for kd in range(NPAIR):
                    nc.tensor.matmul(
                        pps[:],
                        OT_t[:, kd, lsl],
                        wpT_t[:, kd, co * QT:(co + 1) * QT],
                        start=(kd == 0),
                        stop=(kd == NPAIR - 1),
                    )
                ob = ob_pool.tile([128, QT], F32, tag="ob")
                nc.vector.tensor_copy(ob[:], pps[:])
                nc.sync.dma_start(outp_d[lt, :, co * QT:(co + 1) * QT], ob[:])

            def emit_av(pr, qt, qc, pt, tq):
                """Flipped attn@V for one 128-q chunk (both heads) + norm."""
                av = {}
                for hd in range(2):
                    av[hd] = ps_ms.tile([128, QT], F32, tag="ms", name=f"av{hd}")
                    for kt in range(NLT):
                        nc.tensor.matmul(
                            av[hd][:, 0:VW],
                            pt[:, 2 * kt + hd, qc * 128:(qc + 1) * 128],
                            V_t[:, kt, pr * 2 + hd, :],
                            start=(kt == 0),
                            stop=(kt == NLT - 1),
                        )
                on = on_pool.tile([128, 2, 64], BF16, tag="on")
                for hd in range(2):
                    rc = rc_pool.tile([128, 1], F32, tag="rc")
                    nc.vector.reciprocal(rc[:], av[hd][:, 64:65])
                    nc.vector.tensor_scalar(
                        on[:, hd, :], av[hd][:, 0:64], rc[:], None, op0=MULT
                    )
                tq[qc] = on

            def emit_transpose(pr, qt, qc, on):
                tp = ps_ms.tile([128, QT], F32, tag="ms", name="tp")
                tpb = tp.bitcast(BF16)
                nc.tensor.transpose(
                    tpb[:, 0:128], on[:].rearrange("p a b -> p (a b)"), ident_t[:]
                )
                nc.vector.tensor_copy(
                    OT_t[:, pr, qt * QT + qc * 128: qt * QT + (qc + 1) * 128],
                    tpb[:, 0:128],
                )

            # ---- interleaved emission: weave filler PE chunks between score
            # groups so the Activation engine (the per-qtile bottleneck) never
            # starves behind the in-order PE stream.
            import collections

            fillers = collections.deque()  # (cost_cycles, fn, epoch)
            debt = [0.0]
            need = [0.0]   # deadline-critical cycles to force-spread this epoch
            forced = [0.0]
            PUMP = 1400.0  # PE filler cycles per score group (Act group ~1.04us)
            NGROUPS = 2 * (L // 128) // GRP

            def epoch_start(ep):
                # hard drain: entries tagged <= ep-2 must precede this epoch's
                # pt allocation (emission-order WAR on the pt pool slot);
                # normally empty because spreading finished them in ep-1
                while fillers and fillers[0][2] <= ep - 2:
                    _, fn, _ = fillers.popleft()
                    fn()
                # spread target: entries tagged <= ep-1 finish within this
                # epoch, woven across its score groups
                n = 0.0
                for c, _, e in fillers:
                    if e > ep - 1:
                        break
                    n += c
                need[0] = n
                forced[0] = 0.0
                debt[0] = 0.0

            def pump(g):
                # spread deadline-critical work across the epoch's groups,
                # plus opportunistic pumping at the steady rate
                target = need[0] * (g + 1) / NGROUPS
                debt[0] += PUMP
                while fillers and (
                    forced[0] < target or debt[0] >= fillers[0][0]
                ):
                    c, fn, _ = fillers.popleft()
                    fn()
                    forced[0] += c
                    debt[0] -= c

            def drain_all():
                while fillers:
                    _, fn, _ = fillers.popleft()
                    fn()

            # A1 V-proj chunks: FIFO-ahead of qt0's AV, deadline epoch 0
            for lt in range(NLT):
                fillers.append((4096, (lambda lt=lt: emit_a1_lt(lt)), 0))

            # ---- phase B driver ----
            for pr in range(NPAIR):
                for qt in range(NQT):
                    ep = pr * NQT + qt
                    epoch_start(ep)
                    if pr == 0 and qt == 0:
                        emit_qk_lq(0, 0)
                        emit_qk_lq(4, 0)
                    qT_t, kT_t = qkT[pr], qkT[4 + pr]
                    qsl = slice(qt * QT, (qt + 1) * QT)
                    pt = pt_pool.tile([128, NS, QT], BF16, tag="pt", name="pt")
                    for g0 in range(0, NS, GRP):
                        if pr == 0 and qt == 0 and g0 // 2 in (1, 3, 5):
                            # stream pair-0 q/k projection just ahead of the
                            # score chunks that consume it
                            lq = {1: 1, 3: 2, 5: 3}[g0 // 2]
                            emit_qk_lq(0, lq)
                            emit_qk_lq(4, lq)
                        sc = ps_sc.tile([128, GRP, QT], F32, tag="sc")
                        for j in range(GRP):
                            s = g0 + j
                            kt, hd = s // 2, s % 2
                            nc.tensor.matmul(
                                sc[:, j, :],
                                kT_t[hd * 64:(hd + 1) * 64, kt * 128:(kt + 1) * 128],
                                qT_t[hd * 64:(hd + 1) * 64, qsl],
                                start=True,
                                stop=True,
                                tile_position=(hd * 64, 0),
                            )
                        nc.scalar.activation(
                            pt[:, g0:g0 + GRP, :], sc[:], EXP, scale=float(D) ** -0.5
                        )
                        if not (pr == 0 and qt == 0):
                            pump(g0 // GRP)
                    # queue this qtile's AV + norm + transpose (+ C for pr3):
                    # they weave through the next qtile's score stream and must
                    # be emitted before epoch ep+2 reuses the pt buffer.
                    tq = {}
                    for qc in range(QT // 128):
                        fillers.append(
                            (2080, (lambda pr=pr, qt=qt, qc=qc, pt=pt, tq=tq:
                                    emit_av(pr, qt, qc, pt, tq)), ep)
                        )
                        if qc > 0:
                            fillers.append(
                                (150, (lambda pr=pr, qt=qt, qc=qc, tq=tq:
                                       emit_transpose(pr, qt, qc - 1, tq[qc - 1])), ep)
                            )
                            if pr == NPAIR - 1:
                                lt = qt * NQT + qc - 1
                                for co in range(C // QT):
                                    fillers.append(
                                        (2048, (lambda lt=lt, co=co: emit_c(lt, co)), ep)
                                    )
                    fillers.append(
                        (150, (lambda pr=pr, qt=qt, tq=tq:
                               emit_transpose(pr, qt, NQT - 1, tq[NQT - 1])), ep)
                    )
                    if pr == NPAIR - 1:
                        lt = qt * NQT + NQT - 1
                        for co in range(C // QT):
                            fillers.append(
                                (2048, (lambda lt=lt, co=co: emit_c(lt, co)), ep)
                            )
                    # queue A2 q/k chunks for the next pair (deadline: both
                    # tiles fully emitted before that pair's first scores)
                    if pr < NPAIR - 1:
                        if qt == 0:
                            load_wqk(pr + 1)
                            for lq in range(NQT):
                                fillers.append(
                                    (4300, (lambda ft=pr + 1, lq=lq: emit_qk_lq(ft, lq)), ep + 1)
                                )
                        if qt == 1:
                            load_wqk(5 + pr)
                            for lq in range(NQT):
                                fillers.append(
                                    (4300, (lambda ft=5 + pr, lq=lq: emit_qk_lq(ft, lq)), ep + 1)
                                )
            drain_all()

    nc.compile()
    return nc


def _get_nc():
    if "nc" not in _built:
        _built["nc"] = _build(None)
    return _built["nc"]


def _rope_perm():
    """Within-head row permutation: quadrant-local [evens(16) | odds(16)]."""
    perm = np.empty(64, np.int64)
    for j in range(2):
        for i in range(32):
            perm[j * 32 + i] = 2 * (j * 16 + i) if i < 16 else 2 * (j * 16 + i - 16) + 1
    return perm


def _shard_inputs(x, cos, sin, w_qkv, w_proj):
    import ml_dtypes

    Bb = ml_dtypes.bfloat16
    perm = _rope_perm()
    p = np.arange(128)
    quad, i = p // 32, p % 32
    pairidx = (quad % 2) * 16 + (i % 16)
    sign = np.where(i < 16, -1.0, 1.0).astype(np.float32)
    cos4 = np.ascontiguousarray(cos[:, pairidx].T).astype(Bb)              # [128, L]
    sin4 = np.ascontiguousarray((sin[:, pairidx] * sign[None, :]).T).astype(Bb)
    ident = np.eye(128, dtype=np.float32).astype(Bb)

    in_maps = []
    for c in range(NCORES):
        b, hg = c // 2, c % 2
        xT = np.ascontiguousarray(
            x[b].T.reshape(C // 128, 128, L).transpose(1, 0, 2)
        ).astype(Bb)  # [p, kc, l]

        rows = np.empty((8, 128), np.int64)
        for ft in range(8):
            t = 0 if ft < 4 else 1
            pr = ft % 4
            for fi in range(128):
                head = hg * 8 + 2 * pr + (0 if fi < 64 else 1)
                rows[ft, fi] = t * C + head * D + perm[fi % 64]
        wq = w_qkv[rows.reshape(-1)].reshape(8, 128, C // 128, 128)  # [ft, f, kc, p]
        wqkT = np.ascontiguousarray(wq.transpose(0, 3, 2, 1)).astype(Bb)  # [ft,p,kc,f]

        wv = w_qkv[2 * C + hg * 512: 2 * C + hg * 512 + 512]         # [fv, c]
        wvT = np.ascontiguousarray(
            wv.T.reshape(C // 128, 128, 512).transpose(1, 0, 2)
        ).astype(Bb)  # [p, kc, fv]

        wp = w_proj[:, hg * 512: hg * 512 + 512]                     # [co, d']
        wpT = np.ascontiguousarray(
            wp.T.reshape(4, 128, C).transpose(1, 0, 2)
        ).astype(Bb)  # [p, kd, co]

        in_maps.append(
            {
                "xT": xT, "wqkT": wqkT, "wvT": wvT, "wpT": wpT,
                "cos4": cos4, "sin4": sin4, "ident": ident,
            }
        )
    return in_maps


def kernel(x, cos, sin, w_qkv, w_proj, b_proj, _trace=False):
    from concourse.bass_utils import run_bass_kernel_spmd

    x = np.asarray(x, dtype=np.float32)
    cos = np.asarray(cos, dtype=np.float32)
    sin = np.asarray(sin, dtype=np.float32)
    w_qkv = np.asarray(w_qkv, dtype=np.float32)
    w_proj = np.asarray(w_proj, dtype=np.float32)
    b_proj = np.asarray(b_proj, dtype=np.float32)

    nc = _get_nc()
    in_maps = _shard_inputs(x, cos, sin, w_qkv, w_proj)
    res = run_bass_kernel_spmd(
        nc, in_maps, core_ids=list(range(NCORES)), trace=_trace
    )
    if _trace:
        print("exec_time_ns:", res.exec_time_ns)

    out = np.empty((B, L, C), dtype=np.float32)
    for b in range(B):
        p0 = res.results[2 * b]["outp"].reshape(L, C)
        p1 = res.results[2 * b + 1]["outp"].reshape(L, C)
        out[b] = p0 + p1
    out += b_proj[None, None, :]
    return out


# revision 45
# speedup vs baseline: 1.3346x; 1.0035x over previous
"""Multi-head attention (B=4, L=2048, C=1024, H=16, D=64) on 8 TRN2 NeuronCores.

Sharding: core c handles batch b = c//2 and head-group hg = c%2 (8 heads).
Megatron-style: w_qkv column-sharded, w_proj row-sharded; the proj all-reduce
(2 cores per batch) happens on the host during unshard.

All-bf16 dataflow (matmul cost model: 1.0 cyc/row at any free size; cost is
keyed on the moving operand's dtype; measured end-to-end error ~6e-3):

  A (per 512-l chunk): q/k proj for pair 0 first so phase B starts at ~5us,
     then V proj (V stored bf16 with a ones column for the softmax denom).
     RoPE via host-permuted w_qkv rows + quadrant stream_shuffle (as before),
     but in bf16.
  B (per pair, per 512-q tile):
     scores:  S^T[k128, q512] = kT.T @ qT per (kt, hd), one PSUM bank each
              (tile_position packs 2 heads on the 128 partitions).
     exp:     ScalarE Exp (scale 1/8 fused) PSUM -> persistent SBUF tile
              pt[128, 32, 512] bf16 (whole qtile, double-buffered).
     attn@V:  FLIPPED: O[q128, 65] += pt[:, s, qc*128:+128].T @ V'[k128, 65]
              accumulated over 16 kt (65 = 64 dims + ones column -> denom
              lands per-partition). Free size 65 halves PE cost vs the O^T
              form (cost model charges output free size only).
     norm:    reciprocal [128,1] + per-partition tensor_scalar mult -> bf16
              O_n[q,2,64]; then PE transpose (identity matmul) -> OT^T chunk;
              GPSIMD copies PSUM -> OT sbuf (OT kept fully in SBUF).
     A2 for later pairs interleaved at qtile boundaries.
  C (interleaved during last pair): out[l128, co512] partial over the 512
     local proj dims, accumulated over 4 pair-chunks, DMA'd per chunk.

PSUM budget (8 banks): ps_sc 2 tiles x 2 banks + misc pool 4 x 1 bank
(qkv-proj psum, AV accumulators, transpose dests, out-proj psum all share
misc as full-bank slots; ZERO_REGION = whole bank forbids co-tenancy with
any accumulating group).
"""

import sys

sys.path.insert(0, "/opt/trn_rl_repo")

import numpy as np

B, L, C, H, D = 4, 2048, 1024, 16, 64
NCORES = 8
QT = 512          # q-tile; one score mm per PSUM bank
GRP = 2           # score banks per exp group (exp ap = GRP*QT)
_built = {}


def _build(nc_mod):
    """Build the per-core Bass program (identical on all cores)."""
    import concourse.mybir as mybir
    import concourse.tile as tile
    from concourse import bacc
    from concourse.alu_op_type import AluOpType

    F32 = mybir.dt.float32
    BF16 = mybir.dt.bfloat16
    EXP = mybir.ActivationFunctionType.Exp
    MULT = AluOpType.mult
    ADD = AluOpType.add

    NKC = C // 128          # 8 contraction chunks for qkv proj
    NLT = L // 128          # 16 l-tiles (V rows, proj rows, k-chunks)
    NQT = L // QT           # 4 q-tiles per pair
    NPAIR = 4               # head pairs per core
    FV = 512                # v features per core
    VW = 65                 # V columns incl. ones
    NS = 2 * NLT            # 32 (kt, hd) score slices per qtile

    nc = bacc.Bacc(None, target_bir_lowering=False)

    xT_d = nc.dram_tensor("xT", [128, NKC, L], BF16, kind="ExternalInput")
    wqkT_d = nc.dram_tensor("wqkT", [8, 128, NKC, 128], BF16, kind="ExternalInput")
    wvT_d = nc.dram_tensor("wvT", [128, NKC, FV], BF16, kind="ExternalInput")
    wpT_d = nc.dram_tensor("wpT", [128, NPAIR, C], BF16, kind="ExternalInput")
    cos4_d = nc.dram_tensor("cos4", [128, L], BF16, kind="ExternalInput")
    sin4_d = nc.dram_tensor("sin4", [128, L], BF16, kind="ExternalInput")
    ident_d = nc.dram_tensor("ident", [128, 128], BF16, kind="ExternalInput")
    outp_d = nc.dram_tensor("outp", [NLT, 128, C], F32, kind="ExternalOutput")
    # last-window outputs go out in bf16 so the end-of-kernel DMA drain is
    # half as long (quantization adds ~0.2% of max, within budget)
    outpb_d = nc.dram_tensor("outpb", [4, 128, C], BF16, kind="ExternalOutput")

    SWAP_MASK = list(range(16, 32)) + list(range(16))

    with tile.TileContext(nc) as tc:
        import contextlib

        with contextlib.ExitStack() as outer:
            persist = outer.enter_context(tc.tile_pool(name="persist", bufs=1))
            qk_pool = outer.enter_context(tc.tile_pool(name="qkt", bufs=5))
            pt_pool = outer.enter_context(tc.tile_pool(name="pt", bufs=2))
            on_pool = outer.enter_context(tc.tile_pool(name="on", bufs=3))
            rc_pool = outer.enter_context(tc.tile_pool(name="rc", bufs=4))
            tpool = outer.enter_context(tc.tile_pool(name="tmp", bufs=3))
            ob_pool = outer.enter_context(tc.tile_pool(name="ob", bufs=3))
            oba_pool = outer.enter_context(tc.tile_pool(name="oba", bufs=9))
            ps_sc = outer.enter_context(tc.tile_pool(name="ps_sc", bufs=2, space="PSUM"))
            ps_ms = outer.enter_context(tc.tile_pool(name="ps_ms", bufs=4, space="PSUM"))

            # ---- persistent tensors ----
            V_t = persist.tile([128, NLT, 8, VW], BF16, tag="V")
            OT_t = persist.tile([128, NPAIR, L], BF16, tag="OT")
            xT_t = persist.tile([128, NKC, L], BF16, tag="xT")
            wvT_t = persist.tile([128, NKC, FV], BF16, tag="wv")
            wpT_t = persist.tile([128, NPAIR, C], BF16, tag="wp")
            cos4_t = persist.tile([128, L], BF16, tag="cos")
            sin4_t = persist.tile([128, L], BF16, tag="sin")
            ident_t = persist.tile([128, 128], BF16, tag="id")
            ones_t = persist.tile([128, NLT, 8], BF16, tag="ones")

            # ---- input DMAs (front section; wpT deferred to phase C) ----
            wqk = {}

            def load_wqk(ft, split=False):
                wqk[ft] = qkw_pool.tile([128, NKC, 128], BF16, tag="wqk", name=f"wqk{ft}")
                if split:
                    nc.sync.dma_start(wqk[ft][:, 0:2], wqkT_d[ft, :, 0:2])
                    nc.sync.dma_start(wqk[ft][:, 2:NKC], wqkT_d[ft, :, 2:NKC])
                else:
                    nc.sync.dma_start(wqk[ft][:], wqkT_d[ft])

            qkw_pool = outer.enter_context(tc.tile_pool(name="qkw", bufs=4))
            # DMA order matches first-consumption: wqk0's first chunks, the
            # lq0 x slices, the lq0 cos/sin slices (for the first ropes), the
            # rest of wqk0 and wqk4, then everything else.
            load_wqk(0)
            nc.sync.dma_start(xT_t[:, :, 0:QT], xT_d[:, :, 0:QT])
            nc.sync.dma_start(cos4_t[:, 0:QT], cos4_d[:, 0:QT])
            nc.sync.dma_start(sin4_t[:, 0:QT], sin4_d[:, 0:QT])
            load_wqk(4)
            nc.sync.dma_start(cos4_t[:, QT:], cos4_d[:, QT:])
            nc.sync.dma_start(sin4_t[:, QT:], sin4_d[:, QT:])
            for lq in range(1, NQT):
                sl = slice(lq * QT, (lq + 1) * QT)
                nc.sync.dma_start(xT_t[:, :, sl], xT_d[:, :, sl])
            nc.sync.dma_start(wvT_t[:], wvT_d[:])
            nc.sync.dma_start(ident_t[:], ident_d[:])

            # warm the Exp activation table while the input DMAs stream in so
            # the first real exp doesn't pay the 1283ns table load
            warm = tpool.tile([1, 1], F32, tag="warm", name="warm")
            nc.vector.memset(warm[:], 0.0)
            nc.scalar.activation(warm[:], warm[:], EXP)

            nc.vector.memset(ones_t[:], 1.0)
            nc.vector.tensor_copy(V_t[:, :, :, 64:65], ones_t[:, :, :, None])

            # ---- A2 helper: qT/kT for one f-tile (one pair, q or k) ----
            qkT = {}

            qk_acc = {}

            def emit_qk_lq(ft, lq, half=None):
                """Project + rope one 512-l chunk of f-tile ft into qkT[ft].
                half=0 emits the first 4 contraction chunks (PSUM group stays
                open), half=1 finishes and ropes; None does both."""
                if ft not in qkT:
                    qkT[ft] = qk_pool.tile([128, L], BF16, tag="qkt", name=f"qkT{ft}")
                dst = qkT[ft]
                sl = slice(lq * QT, (lq + 1) * QT)
                if half in (0, None):
                    qps = ps_ms.tile([128, QT], F32, tag="ms", name="qps")
                    qk_acc[ft, lq] = qps
                    kcs = range(0, 4 if half == 0 else NKC)
                else:
                    qps = qk_acc.pop((ft, lq))
                    kcs = range(4, NKC)
                for kc in kcs:
                    nc.tensor.matmul(
                        qps[:],
                        wqk[ft][:, kc, :],
                        xT_t[:, kc, sl],
                        start=(kc == 0),
                        stop=(kc == NKC - 1),
                    )
                if half == 0:
                    return
                # RoPE in bf16: dst = qb*cos4 + swap(qb)*sin4s
                qb = tpool.tile([128, QT], BF16, tag="qb")
                shufb = tpool.tile([128, QT], BF16, tag="shufb")
                nc.vector.tensor_copy(qb[:], qps[:])
                nc.vector.stream_shuffle(shufb[:], qb[:], SWAP_MASK)
                nc.vector.tensor_tensor(dst[:, sl], qb[:], cos4_t[:, sl], op=MULT)
                nc.vector.tensor_tensor(shufb[:], shufb[:], sin4_t[:, sl], op=MULT)
                nc.vector.tensor_tensor(dst[:, sl], dst[:, sl], shufb[:], op=ADD)

            a1_acc = {}

            def emit_a1_lt(lt, half=None):
                if half in (0, None):
                    vps = ps_ms.tile([128, FV], F32, tag="ms", name="vps")
                    a1_acc[lt] = vps
                    kcs = range(0, 4 if half == 0 else NKC)
                else:
                    vps = a1_acc.pop(lt)
                    kcs = range(4, NKC)
                for kc in kcs:
                    nc.tensor.matmul(
                        vps[:],
                        xT_t[:, kc, lt * 128:(lt + 1) * 128],
                        wvT_t[:, kc, :],
                        start=(kc == 0),
                        stop=(kc == NKC - 1),
                    )
                if half == 0:
                    return
                nc.vector.tensor_copy(V_t[:, lt, :, 0:64], vps[:])

            nc.sync.dma_start(wpT_t[:], wpT_d[:])

            # ---- phase C helpers: out-proj split into a pairs-0..2 partial
            # (weavable as soon as pair 2 finishes) and a pair-3 finisher so
            # only one matmul per output chunk trails the last attention tile.
            oba = {}

            def emit_c(lt, co):
                """Full 4-pair out-proj chunk + DVE copy + DMA (windows 0-2)."""
                pps = ps_ms.tile([128, QT], F32, tag="ms", name="pps")
                for kd in range(NPAIR):
                    nc.tensor.matmul(
                        pps[:],
                        OT_t[:, kd, lt * 128:(lt + 1) * 128],
                        wpT_t[:, kd, co * QT:(co + 1) * QT],
                        start=(kd == 0),
                        stop=(kd == NPAIR - 1),
                    )
                ob = ob_pool.tile([128, QT], F32, tag="ob")
                nc.vector.tensor_copy(ob[:], pps[:])
                nc.sync.dma_start(outp_d[lt, :, co * QT:(co + 1) * QT], ob[:])

            def emit_ca(lt, co):
                """Pairs 0-2 partial for the last window, parked in bf16."""
                pps = ps_ms.tile([128, QT], F32, tag="ms", name="ppsa")
                for kd in range(NPAIR - 1):
                    nc.tensor.matmul(
                        pps[:],
                        OT_t[:, kd, lt * 128:(lt + 1) * 128],
                        wpT_t[:, kd, co * QT:(co + 1) * QT],
                        start=(kd == 0),
                        stop=(kd == NPAIR - 2),
                    )
                t = oba_pool.tile([128, QT], BF16, tag="oba", name="oba")
                nc.vector.tensor_copy(t[:], pps[:])
                oba[lt, co] = t

            def emit_cb(lt, co):
                """Tail finisher: pair-3 matmul + identity-matmul to fold the
                parked partial into the same PSUM group (no DVE in the tail);
                copy-out on the idle Act engine."""
                pps = ps_ms.tile([128, QT], F32, tag="ms", name="ppsb")
                nc.tensor.matmul(
                    pps[:],
                    OT_t[:, NPAIR - 1, lt * 128:(lt + 1) * 128],
                    wpT_t[:, NPAIR - 1, co * QT:(co + 1) * QT],
                    start=True,
                    stop=False,
                )
                nc.tensor.matmul(
                    pps[:],
                    ident_t[:],
                    oba[lt, co][:],
                    start=False,
                    stop=True,
                )
                ob = ob_pool.tile([128, QT], BF16, tag="ob", name="obb")
                nc.vector.tensor_copy(ob[:], pps[:])
                nc.sync.dma_start(outpb_d[lt - 12, :, co * QT:(co + 1) * QT], ob[:])

            def emit_av(pr, qt, qc, pt, tq, on_act=False):
                """Flipped attn@V for one 128-q chunk (both heads) + norm."""
                av = {}
                for hd in range(2):
                    av[hd] = ps_ms.tile([128, QT], F32, tag="ms", name=f"av{hd}")
                    for kt in range(NLT):
                        nc.tensor.matmul(
                            av[hd][:, 0:VW],
                            pt[:, 2 * kt + hd, qc * 128:(qc + 1) * 128],
                            V_t[:, kt, pr * 2 + hd, :],
                            start=(kt == 0),
                            stop=(kt == NLT - 1),
                        )
                on = on_pool.tile([128, 2, 64], BF16, tag="on")
                for hd in range(2):
                    rc = rc_pool.tile([128, 1], F32, tag="rc")
                    nc.vector.reciprocal(rc[:], av[hd][:, 64:65])
                    nc.vector.tensor_scalar(
                        on[:, hd, :], av[hd][:, 0:64], rc[:], None, op0=MULT
                    )
                tq[qc] = on

            def emit_transpose(pr, qt, qc, on, on_act=False):
                tp = ps_ms.tile([128, QT], F32, tag="ms", name="tp")
                tpb = tp.bitcast(BF16)
                nc.tensor.transpose(
                    tpb[:, 0:128], on[:].rearrange("p a b -> p (a b)"), ident_t[:]
                )
                dst = OT_t[:, pr, qt * QT + qc * 128: qt * QT + (qc + 1) * 128]
                nc.vector.tensor_copy(dst, tpb[:, 0:128])

            # ---- interleaved emission: weave filler PE chunks between score
            # groups so the Activation engine (the per-qtile bottleneck) never
            # starves behind the in-order PE stream.
            import collections

            fillers = collections.deque()  # (cost_cycles, fn, epoch)
            debt = [0.0]
            need = [0.0]   # deadline-critical cycles to force-spread this epoch
            forced = [0.0]
            PUMP = 1400.0  # PE filler cycles per score group (Act group ~1.04us)
            NGROUPS = 2 * (L // 128) // GRP

            def epoch_start(ep):
                # hard drain: entries tagged <= ep-2 must precede this epoch's
                # pt allocation (emission-order WAR on the pt pool slot);
                # normally empty because spreading finished them in ep-1
                while fillers and fillers[0][2] <= ep - 2:
                    _, fn, _ = fillers.popleft()
                    fn()
                # spread target: entries tagged <= ep-1 finish within this
                # epoch, woven across its score groups
                n = 0.0
                for c, _, e in fillers:
                    if e > ep - 1:
                        break
                    n += c
                need[0] = n
                forced[0] = 0.0
                debt[0] = 0.0

            def pump(g):
                # spread deadline-critical work across the epoch's groups,
                # plus opportunistic pumping at the steady rate; cap per-group
                # emission so a filler burst never blocks the score stream
                # long enough to starve the Activation engine
                target = need[0] * (g + 1) / NGROUPS
                cap = max(3000.0, need[0] / NGROUPS + 1200.0)
                debt[0] += PUMP
                emitted = 0.0
                while fillers and emitted < cap and (
                    forced[0] < target or debt[0] >= fillers[0][0]
                ):
                    c, fn, _ = fillers.popleft()
                    fn()
                    forced[0] += c
                    debt[0] -= c
                    emitted += c

            def drain_all():
                while fillers:
                    _, fn, _ = fillers.popleft()
                    fn()

            # A1 V-proj chunks: FIFO-ahead of qt0's AV, deadline epoch 0
            for lt in range(NLT):
                fillers.append((2048, (lambda lt=lt: emit_a1_lt(lt, 0)), 0))
                fillers.append((2200, (lambda lt=lt: emit_a1_lt(lt, 1)), 0))

            # Explicit target schedule for A2 f-tile chunks and phase C:
            # extra[ep] = chunks queued at the END of epoch ep (tag ep, woven
            # during ep+1), chosen to fill otherwise-idle epochs while
            # respecting qkT/qkw pool-slot reuse (a pair's tiles are only
            # reused two pairs later) and OT availability for C.
            extra = collections.defaultdict(list)

            def qk_sched(ft, targets):
                for lq, tgt in enumerate(targets):
                    extra[tgt - 1].append((2048, lambda ft=ft, lq=lq: emit_qk_lq(ft, lq, 0)))
                    extra[tgt - 1].append((2300, lambda ft=ft, lq=lq: emit_qk_lq(ft, lq, 1)))

            qk_sched(1, [2, 2, 2, 2])
            qk_sched(5, [3, 3, 3, 3])
            qk_sched(2, [4, 4, 5, 5])
            qk_sched(6, [5, 6, 6, 6])
            qk_sched(3, [7, 7, 8, 9])
            qk_sched(7, [8, 9, 10, 11])
            WQK_LOAD_EP = {1: 1, 5: 2, 2: 3, 6: 4, 3: 5, 7: 6}
            # C: last window's pairs-0..2 partial woven in ep 12 (pr0-2 OT is
            # complete after ep 11); full windows 0..2 one epoch after their
            # transposes; window-3 finishers drain in the tail.
            for lt in range(3 * NQT, 4 * NQT):
                for co in range(C // QT):
                    extra[11].append((1536, (lambda lt=lt, co=co: emit_ca(lt, co))))
            for w in range(3):
                for lt in range(w * NQT, (w + 1) * NQT):
                    for co in range(C // QT):
                        extra[12 + w].append((2048, (lambda lt=lt, co=co: emit_c(lt, co))))

            # ---- phase B driver ----
            for pr in range(NPAIR):
                for qt in range(NQT):
                    ep = pr * NQT + qt
                    epoch_start(ep)
                    if pr == 0 and qt == 0:
                        emit_qk_lq(0, 0)
                        emit_qk_lq(4, 0)
                    qT_t, kT_t = qkT[pr], qkT[4 + pr]
                    qsl = slice(qt * QT, (qt + 1) * QT)
                    pt = pt_pool.tile([128, NS, QT], BF16, tag="pt", name="pt")
                    for g0 in range(0, NS, GRP):
                        if pr == 0 and qt == 0 and g0 // 2 in (1, 3, 5):
                            # stream pair-0 q/k projection just ahead of the
                            # score chunks that consume it
                            lq = {1: 1, 3: 2, 5: 3}[g0 // 2]
                            emit_qk_lq(0, lq)
                            emit_qk_lq(4, lq)
                        sc = ps_sc.tile([128, GRP, QT], F32, tag="sc")
                        for j in range(GRP):
                            s = g0 + j
                            kt, hd = s // 2, s % 2
                            nc.tensor.matmul(
                                sc[:, j, :],
                                kT_t[hd * 64:(hd + 1) * 64, kt * 128:(kt + 1) * 128],
                                qT_t[hd * 64:(hd + 1) * 64, qsl],
                                start=True,
                                stop=True,
                                tile_position=(hd * 64, 0),
                            )
                        nc.scalar.activation(
                            pt[:, g0:g0 + GRP, :], sc[:], EXP, scale=float(D) ** -0.5
                        )
                        if not (pr == 0 and qt == 0):
                            pump(g0 // GRP)
                    # queue this qtile's AV + norm + transpose (+ C for pr3):
                    # they weave through the next qtile's score stream and must
                    # be emitted before epoch ep+2 reuses the pt buffer.
                    last_ep = ep == NPAIR * NQT - 1
                    tq = {}
                    for qc in range(QT // 128):
                        fillers.append(
                            (2080, (lambda pr=pr, qt=qt, qc=qc, pt=pt, tq=tq, a=last_ep:
                                    emit_av(pr, qt, qc, pt, tq, on_act=a)), ep)
                        )
                        if qc > 0:
                            fillers.append(
                                (150, (lambda pr=pr, qt=qt, qc=qc, tq=tq, a=last_ep:
                                       emit_transpose(pr, qt, qc - 1, tq[qc - 1], on_act=a)), ep)
                            )
                            if last_ep:
                                lt = qt * NQT + qc - 1
                                for co in range(C // QT):
                                    fillers.append(
                                        (1024, (lambda lt=lt, co=co: emit_cb(lt, co)), ep)
                                    )
                    fillers.append(
                        (150, (lambda pr=pr, qt=qt, tq=tq, a=last_ep:
                               emit_transpose(pr, qt, NQT - 1, tq[NQT - 1], on_act=a)), ep)
                    )
                    if last_ep:
                        lt = qt * NQT + NQT - 1
                        for co in range(C // QT):
                            fillers.append(
                                (1024, (lambda lt=lt, co=co: emit_cb(lt, co)), ep)
                            )
                    for ent in extra.get(ep, ()):
                        fillers.append((ent[0], ent[1], ep))
                    for ft, lep in WQK_LOAD_EP.items():
                        if lep == ep:
                            load_wqk(ft)
            drain_all()

    nc.compile()
    return nc


def _get_nc():
    if "nc" not in _built:
        _built["nc"] = _build(None)
    return _built["nc"]


def _rope_perm():
    """Within-head row permutation: quadrant-local [evens(16) | odds(16)]."""
    perm = np.empty(64, np.int64)
    for j in range(2):
        for i in range(32):
            perm[j * 32 + i] = 2 * (j * 16 + i) if i < 16 else 2 * (j * 16 + i - 16) + 1
    return perm


def _shard_inputs(x, cos, sin, w_qkv, w_proj):
    import ml_dtypes

    Bb = ml_dtypes.bfloat16
    perm = _rope_perm()
    p = np.arange(128)
    quad, i = p // 32, p % 32
    pairidx = (quad % 2) * 16 + (i % 16)
    sign = np.where(i < 16, -1.0, 1.0).astype(np.float32)
    cos4 = np.ascontiguousarray(cos[:, pairidx].T).astype(Bb)              # [128, L]
    sin4 = np.ascontiguousarray((sin[:, pairidx] * sign[None, :]).T).astype(Bb)
    ident = np.eye(128, dtype=np.float32).astype(Bb)

    in_maps = []
    for c in range(NCORES):
        b, hg = c // 2, c % 2
        xT = np.ascontiguousarray(
            x[b].T.reshape(C // 128, 128, L).transpose(1, 0, 2)
        ).astype(Bb)  # [p, kc, l]

        rows = np.empty((8, 128), np.int64)
        for ft in range(8):
            t = 0 if ft < 4 else 1
            pr = ft % 4
            for fi in range(128):
                head = hg * 8 + 2 * pr + (0 if fi < 64 else 1)
                rows[ft, fi] = t * C + head * D + perm[fi % 64]
        wq = w_qkv[rows.reshape(-1)].reshape(8, 128, C // 128, 128)  # [ft, f, kc, p]
        wqkT = np.ascontiguousarray(wq.transpose(0, 3, 2, 1)).astype(Bb)  # [ft,p,kc,f]

        wv = w_qkv[2 * C + hg * 512: 2 * C + hg * 512 + 512]         # [fv, c]
        wvT = np.ascontiguousarray(
            wv.T.reshape(C // 128, 128, 512).transpose(1, 0, 2)
        ).astype(Bb)  # [p, kc, fv]

        wp = w_proj[:, hg * 512: hg * 512 + 512]                     # [co, d']
        wpT = np.ascontiguousarray(
            wp.T.reshape(4, 128, C).transpose(1, 0, 2)
        ).astype(Bb)  # [p, kd, co]

        in_maps.append(
            {
                "xT": xT, "wqkT": wqkT, "wvT": wvT, "wpT": wpT,
                "cos4": cos4, "sin4": sin4, "ident": ident,
            }
        )
    return in_maps


def kernel(x, cos, sin, w_qkv, w_proj, b_proj, _trace=False):
    from concourse.bass_utils import run_bass_kernel_spmd

    x = np.asarray(x, dtype=np.float32)
    cos = np.asarray(cos, dtype=np.float32)
    sin = np.asarray(sin, dtype=np.float32)
    w_qkv = np.asarray(w_qkv, dtype=np.float32)
    w_proj = np.asarray(w_proj, dtype=np.float32)
    b_proj = np.asarray(b_proj, dtype=np.float32)

    nc = _get_nc()
    in_maps = _shard_inputs(x, cos, sin, w_qkv, w_proj)
    res = run_bass_kernel_spmd(
        nc, in_maps, core_ids=list(range(NCORES)), trace=_trace
    )
    if _trace:
        print("exec_time_ns:", res.exec_time_ns)

    out = np.empty((B, L, C), dtype=np.float32)
    for b in range(B):
        p0 = res.results[2 * b]["outp"].reshape(L, C)
        p1 = res.results[2 * b + 1]["outp"].reshape(L, C)
        out[b] = p0 + p1
        p0b = res.results[2 * b]["outpb"].astype(np.float32).reshape(512, C)
        p1b = res.results[2 * b + 1]["outpb"].astype(np.float32).reshape(512, C)
        out[b, 1536:2048] = p0b + p1b
    out += b_proj[None, None, :]
    return out


# revision 55
# speedup vs baseline: 1.3353x; 1.0006x over previous
"""Multi-head attention (B=4, L=2048, C=1024, H=16, D=64) on 8 TRN2 NeuronCores.

Sharding: core c handles batch b = c//2 and head-group hg = c%2 (8 heads).
Megatron-style: w_qkv column-sharded, w_proj row-sharded; the proj all-reduce
(2 cores per batch) happens on the host during unshard.

All-bf16 dataflow (matmul cost: 1.0 cyc/row at any free size, keyed on the
moving operand's dtype; measured end-to-end max rel err ~1e-2 vs 2e-2 gate):

  scores:  S^T[k128, q512] = kT.T @ qT per (kt, hd); one PSUM bank per mm
           (tile_position packs the 2 heads on the 128 partitions).
  exp:     ScalarE Exp (scale 1/8 fused, no max-subtraction; |scores|<~6.5)
           PSUM -> persistent SBUF tile pt[128, 32, 512] bf16 per qtile,
           double-buffered.  The Activation engine is the second wall
           (~267us busy: 218us roofline + 185ns/inst access overhead).
  attn@V:  FLIPPED: O[q128, 65] += pt[:, 2kt+hd, qc*128:+128].T @ V'[k,65]
           over 16 kt; 65 = 64 dims + ones column so the softmax denom
           lands per-partition.  Free size 65 (vs 512 in the O^T form)
           halves attn@V PE cost since cost = output free size.
  norm:    DVE reciprocal [128,1] + per-partition tensor_scalar -> bf16
           O_n[q, 2, 64]; PE transpose via identity matmul -> OT (SBUF,
           no DRAM roundtrip); out-proj consumes OT chunks.
  proj:    out[l128, co512] over 4 pair-chunks; the last l-window is split
           into a pairs-0..2 partial (computed early, parked bf16) plus a
           tail finisher (pair-3 mm + identity-mm folding the partial into
           the same PSUM group) so only ~2 mms/chunk trail the last exp;
           its outputs ship bf16 to halve the final DMA drain.

Scheduling: the PE executes in program order, so all non-score PE work
(V-proj, q/k projections+RoPE for later pairs, attn@V of the previous
qtile, transposes, out-proj) is chopped into ~0.9us chunks and woven
between score groups by a debt/deadline pump ("fillers"), keeping the
Activation engine fed continuously.  Correctness constraint: a qtile's
attn@V must be emitted before the exp two qtiles later reuses its pt pool
slot (emission-order WAR), enforced by epoch deadlines + hard drain.

PSUM budget (8 banks): ps_sc 2 tiles x 2 banks (scores/exp double buffer;
ZERO_REGION = whole bank forbids co-tenant accumulation groups, capping
exp at 1024 elem/inst) + ps_ms 4 x 1 bank shared by qkv-proj psum, the
two attn@V accumulators, transpose dests, and out-proj psum.
"""

import sys

sys.path.insert(0, "/opt/trn_rl_repo")

import numpy as np

B, L, C, H, D = 4, 2048, 1024, 16, 64
NCORES = 8
QT = 512          # q-tile; one score mm per PSUM bank
GRP = 2           # score banks per exp group (exp ap = GRP*QT)
_built = {}


def _build(nc_mod):
    """Build the per-core Bass program (identical on all cores)."""
    import concourse.mybir as mybir
    import concourse.tile as tile
    from concourse import bacc
    from concourse.alu_op_type import AluOpType

    F32 = mybir.dt.float32
    BF16 = mybir.dt.bfloat16
    EXP = mybir.ActivationFunctionType.Exp
    MULT = AluOpType.mult
    ADD = AluOpType.add

    NKC = C // 128          # 8 contraction chunks for qkv proj
    NLT = L // 128          # 16 l-tiles (V rows, proj rows, k-chunks)
    NQT = L // QT           # 4 q-tiles per pair
    NPAIR = 4               # head pairs per core
    FV = 512                # v features per core
    VW = 65                 # V columns incl. ones
    NS = 2 * NLT            # 32 (kt, hd) score slices per qtile

    nc = bacc.Bacc(None, target_bir_lowering=False)

    xT_d = nc.dram_tensor("xT", [128, NKC, L], BF16, kind="ExternalInput")
    wqkT_d = nc.dram_tensor("wqkT", [8, 128, NKC, 128], BF16, kind="ExternalInput")
    wvT_d = nc.dram_tensor("wvT", [128, NKC, FV], BF16, kind="ExternalInput")
    wpT_d = nc.dram_tensor("wpT", [128, NPAIR, C], BF16, kind="ExternalInput")
    cos4_d = nc.dram_tensor("cos4", [128, L], BF16, kind="ExternalInput")
    sin4_d = nc.dram_tensor("sin4", [128, L], BF16, kind="ExternalInput")
    ident_d = nc.dram_tensor("ident", [128, 128], BF16, kind="ExternalInput")
    outp_d = nc.dram_tensor("outp", [NLT, 128, C], F32, kind="ExternalOutput")
    # last-window outputs go out in bf16 so the end-of-kernel DMA drain is
    # half as long (quantization adds ~0.2% of max, within budget)
    outpb_d = nc.dram_tensor("outpb", [4, 128, C], BF16, kind="ExternalOutput")

    SWAP_MASK = list(range(16, 32)) + list(range(16))

    with tile.TileContext(nc) as tc:
        import contextlib

        with contextlib.ExitStack() as outer:
            persist = outer.enter_context(tc.tile_pool(name="persist", bufs=1))
            qk_pool = outer.enter_context(tc.tile_pool(name="qkt", bufs=5))
            pt_pool = outer.enter_context(tc.tile_pool(name="pt", bufs=2))
            on_pool = outer.enter_context(tc.tile_pool(name="on", bufs=3))
            rc_pool = outer.enter_context(tc.tile_pool(name="rc", bufs=4))
            tpool = outer.enter_context(tc.tile_pool(name="tmp", bufs=3))
            ob_pool = outer.enter_context(tc.tile_pool(name="ob", bufs=3))
            oba_pool = outer.enter_context(tc.tile_pool(name="oba", bufs=9))
            ps_sc = outer.enter_context(tc.tile_pool(name="ps_sc", bufs=2, space="PSUM"))
            ps_ms = outer.enter_context(tc.tile_pool(name="ps_ms", bufs=4, space="PSUM"))

            # ---- persistent tensors ----
            V_t = persist.tile([128, NLT, 8, VW], BF16, tag="V")
            OT_t = persist.tile([128, NPAIR, L], BF16, tag="OT")
            xT_t = persist.tile([128, NKC, L], BF16, tag="xT")
            wvT_t = persist.tile([128, NKC, FV], BF16, tag="wv")
            wpT_t = persist.tile([128, NPAIR, C], BF16, tag="wp")
            cos4_t = persist.tile([128, L], BF16, tag="cos")
            sin4_t = persist.tile([128, L], BF16, tag="sin")
            ident_t = persist.tile([128, 128], BF16, tag="id")
            ones_t = persist.tile([128, NLT, 8], BF16, tag="ones")

            # ---- input DMAs (front section; wpT deferred to phase C) ----
            wqk = {}

            def load_wqk(ft, split=False):
                wqk[ft] = qkw_pool.tile([128, NKC, 128], BF16, tag="wqk", name=f"wqk{ft}")
                if split:
                    nc.sync.dma_start(wqk[ft][:, 0:2], wqkT_d[ft, :, 0:2])
                    nc.sync.dma_start(wqk[ft][:, 2:NKC], wqkT_d[ft, :, 2:NKC])
                else:
                    nc.sync.dma_start(wqk[ft][:], wqkT_d[ft])

            qkw_pool = outer.enter_context(tc.tile_pool(name="qkw", bufs=4))
            # DMA order matches first-consumption: wqk0's first chunks, the
            # lq0 x slices, the lq0 cos/sin slices (for the first ropes), the
            # rest of wqk0 and wqk4, then everything else.
            load_wqk(0)
            nc.sync.dma_start(xT_t[:, 0:4, 0:QT], xT_d[:, 0:4, 0:QT])
            nc.sync.dma_start(xT_t[:, 4:NKC, 0:QT], xT_d[:, 4:NKC, 0:QT])
            nc.sync.dma_start(cos4_t[:, 0:QT], cos4_d[:, 0:QT])
            nc.sync.dma_start(sin4_t[:, 0:QT], sin4_d[:, 0:QT])
            load_wqk(4)
            nc.sync.dma_start(cos4_t[:, QT:], cos4_d[:, QT:])
            nc.sync.dma_start(sin4_t[:, QT:], sin4_d[:, QT:])
            for lq in range(1, NQT):
                sl = slice(lq * QT, (lq + 1) * QT)
                nc.sync.dma_start(xT_t[:, :, sl], xT_d[:, :, sl])
            nc.sync.dma_start(wvT_t[:], wvT_d[:])
            nc.sync.dma_start(ident_t[:], ident_d[:])

            # warm the Exp activation table while the input DMAs stream in so
            # the first real exp doesn't pay the 1283ns table load
            warm = tpool.tile([1, 1], F32, tag="warm", name="warm")
            nc.vector.memset(warm[:], 0.0)
            nc.scalar.activation(warm[:], warm[:], EXP)

            nc.vector.memset(ones_t[:], 1.0)
            nc.vector.tensor_copy(V_t[:, :, :, 64:65], ones_t[:, :, :, None])

            # ---- A2 helper: qT/kT for one f-tile (one pair, q or k) ----
            qkT = {}

            qk_acc = {}

            def emit_qk_lq(ft, lq, half=None):
                """Project + rope one 512-l chunk of f-tile ft into qkT[ft].
                half=0 emits the first 4 contraction chunks (PSUM group stays
                open), half=1 finishes and ropes; None does both."""
                if ft not in qkT:
                    qkT[ft] = qk_pool.tile([128, L], BF16, tag="qkt", name=f"qkT{ft}")
                dst = qkT[ft]
                sl = slice(lq * QT, (lq + 1) * QT)
                if half in (0, None):
                    qps = ps_ms.tile([128, QT], F32, tag="ms", name="qps")
                    qk_acc[ft, lq] = qps
                    kcs = range(0, 4 if half == 0 else NKC)
                else:
                    qps = qk_acc.pop((ft, lq))
                    kcs = range(4, NKC)
                for kc in kcs:
                    nc.tensor.matmul(
                        qps[:],
                        wqk[ft][:, kc, :],
                        xT_t[:, kc, sl],
                        start=(kc == 0),
                        stop=(kc == NKC - 1),
                    )
                if half == 0:
                    return
                # RoPE in bf16: dst = qb*cos4 + swap(qb)*sin4s
                qb = tpool.tile([128, QT], BF16, tag="qb")
                shufb = tpool.tile([128, QT], BF16, tag="shufb")
                nc.vector.tensor_copy(qb[:], qps[:])
                nc.vector.stream_shuffle(shufb[:], qb[:], SWAP_MASK)
                nc.vector.tensor_tensor(dst[:, sl], qb[:], cos4_t[:, sl], op=MULT)
                nc.vector.tensor_tensor(shufb[:], shufb[:], sin4_t[:, sl], op=MULT)
                nc.vector.tensor_tensor(dst[:, sl], dst[:, sl], shufb[:], op=ADD)

            a1_acc = {}

            def emit_a1_lt(lt, half=None):
                if half in (0, None):
                    vps = ps_ms.tile([128, FV], F32, tag="ms", name="vps")
                    a1_acc[lt] = vps
                    kcs = range(0, 4 if half == 0 else NKC)
                else:
                    vps = a1_acc.pop(lt)
                    kcs = range(4, NKC)
                for kc in kcs:
                    nc.tensor.matmul(
                        vps[:],
                        xT_t[:, kc, lt * 128:(lt + 1) * 128],
                        wvT_t[:, kc, :],
                        start=(kc == 0),
                        stop=(kc == NKC - 1),
                    )
                if half == 0:
                    return
                nc.vector.tensor_copy(V_t[:, lt, :, 0:64], vps[:])

            nc.sync.dma_start(wpT_t[:], wpT_d[:])

            # ---- phase C helpers: out-proj split into a pairs-0..2 partial
            # (weavable as soon as pair 2 finishes) and a pair-3 finisher so
            # only one matmul per output chunk trails the last attention tile.
            oba = {}

            def emit_c(lt, co):
                """Full 4-pair out-proj chunk + DVE copy + DMA (windows 0-2)."""
                pps = ps_ms.tile([128, QT], F32, tag="ms", name="pps")
                for kd in range(NPAIR):
                    nc.tensor.matmul(
                        pps[:],
                        OT_t[:, kd, lt * 128:(lt + 1) * 128],
                        wpT_t[:, kd, co * QT:(co + 1) * QT],
                        start=(kd == 0),
                        stop=(kd == NPAIR - 1),
                    )
                ob = ob_pool.tile([128, QT], F32, tag="ob")
                nc.vector.tensor_copy(ob[:], pps[:])
                nc.sync.dma_start(outp_d[lt, :, co * QT:(co + 1) * QT], ob[:])

            def emit_ca(lt, co):
                """Pairs 0-2 partial for the last window, parked in bf16."""
                pps = ps_ms.tile([128, QT], F32, tag="ms", name="ppsa")
                for kd in range(NPAIR - 1):
                    nc.tensor.matmul(
                        pps[:],
                        OT_t[:, kd, lt * 128:(lt + 1) * 128],
                        wpT_t[:, kd, co * QT:(co + 1) * QT],
                        start=(kd == 0),
                        stop=(kd == NPAIR - 2),
                    )
                t = oba_pool.tile([128, QT], BF16, tag="oba", name="oba")
                nc.vector.tensor_copy(t[:], pps[:])
                oba[lt, co] = t

            def emit_cb(lt, co):
                """Tail finisher: pair-3 matmul + identity-matmul to fold the
                parked partial into the same PSUM group (no DVE in the tail);
                copy-out on the idle Act engine."""
                pps = ps_ms.tile([128, QT], F32, tag="ms", name="ppsb")
                nc.tensor.matmul(
                    pps[:],
                    OT_t[:, NPAIR - 1, lt * 128:(lt + 1) * 128],
                    wpT_t[:, NPAIR - 1, co * QT:(co + 1) * QT],
                    start=True,
                    stop=False,
                )
                nc.tensor.matmul(
                    pps[:],
                    ident_t[:],
                    oba[lt, co][:],
                    start=False,
                    stop=True,
                )
                ob = ob_pool.tile([128, QT], BF16, tag="ob", name="obb")
                nc.vector.tensor_copy(ob[:], pps[:])
                nc.sync.dma_start(outpb_d[lt - 12, :, co * QT:(co + 1) * QT], ob[:])

            def emit_av(pr, qt, qc, pt, tq):
                """Flipped attn@V for one 128-q chunk (both heads) + norm."""
                av = {}
                for hd in range(2):
                    av[hd] = ps_ms.tile([128, QT], F32, tag="ms", name=f"av{hd}")
                    for kt in range(NLT):
                        nc.tensor.matmul(
                            av[hd][:, 0:VW],
                            pt[:, 2 * kt + hd, qc * 128:(qc + 1) * 128],
                            V_t[:, kt, pr * 2 + hd, :],
                            start=(kt == 0),
                            stop=(kt == NLT - 1),
                        )
                on = on_pool.tile([128, 2, 64], BF16, tag="on")
                for hd in range(2):
                    rc = rc_pool.tile([128, 1], F32, tag="rc")
                    nc.vector.reciprocal(rc[:], av[hd][:, 64:65])
                    nc.vector.tensor_scalar(
                        on[:, hd, :], av[hd][:, 0:64], rc[:], None, op0=MULT
                    )
                tq[qc] = on

            def emit_transpose(pr, qt, qc, on):
                tp = ps_ms.tile([128, QT], F32, tag="ms", name="tp")
                tpb = tp.bitcast(BF16)
                nc.tensor.transpose(
                    tpb[:, 0:128], on[:].rearrange("p a b -> p (a b)"), ident_t[:]
                )
                dst = OT_t[:, pr, qt * QT + qc * 128: qt * QT + (qc + 1) * 128]
                nc.vector.tensor_copy(dst, tpb[:, 0:128])

            # ---- interleaved emission: weave filler PE chunks between score
            # groups so the Activation engine (the per-qtile bottleneck) never
            # starves behind the in-order PE stream.
            import collections

            fillers = collections.deque()  # (cost_cycles, fn, epoch)
            debt = [0.0]
            need = [0.0]   # deadline-critical cycles to force-spread this epoch
            forced = [0.0]
            PUMP = 1400.0  # PE filler cycles per score group (Act group ~1.04us)
            NGROUPS = 2 * (L // 128) // GRP

            def epoch_start(ep):
                # hard drain: entries tagged <= ep-2 must precede this epoch's
                # pt allocation (emission-order WAR on the pt pool slot);
                # normally empty because spreading finished them in ep-1
                while fillers and fillers[0][2] <= ep - 2:
                    _, fn, _ = fillers.popleft()
                    fn()
                # spread target: entries tagged <= ep-1 finish within this
                # epoch, woven across its score groups
                n = 0.0
                for c, _, e in fillers:
                    if e > ep - 1:
                        break
                    n += c
                need[0] = n
                forced[0] = 0.0
                debt[0] = 0.0

            def pump(g):
                # spread deadline-critical work across the epoch's groups,
                # plus opportunistic pumping at the steady rate; cap per-group
                # emission so a filler burst never blocks the score stream
                # long enough to starve the Activation engine
                target = need[0] * (g + 1) / NGROUPS
                cap = max(3000.0, need[0] / NGROUPS + 1200.0)
                debt[0] += PUMP
                emitted = 0.0
                while fillers and emitted < cap and (
                    forced[0] < target or debt[0] >= fillers[0][0]
                ):
                    c, fn, _ = fillers.popleft()
                    fn()
                    forced[0] += c
                    debt[0] -= c
                    emitted += c

            def drain_all():
                while fillers:
                    _, fn, _ = fillers.popleft()
                    fn()

            # A1 V-proj chunks: FIFO-ahead of qt0's AV, deadline epoch 0
            for lt in range(NLT):
                fillers.append((2048, (lambda lt=lt: emit_a1_lt(lt, 0)), 0))
                fillers.append((2200, (lambda lt=lt: emit_a1_lt(lt, 1)), 0))

            # Explicit target schedule for A2 f-tile chunks and phase C:
            # extra[ep] = chunks queued at the END of epoch ep (tag ep, woven
            # during ep+1), chosen to fill otherwise-idle epochs while
            # respecting qkT/qkw pool-slot reuse (a pair's tiles are only
            # reused two pairs later) and OT availability for C.
            extra = collections.defaultdict(list)

            def qk_sched(ft, targets):
                for lq, tgt in enumerate(targets):
                    extra[tgt - 1].append((2048, lambda ft=ft, lq=lq: emit_qk_lq(ft, lq, 0)))
                    extra[tgt - 1].append((2300, lambda ft=ft, lq=lq: emit_qk_lq(ft, lq, 1)))

            qk_sched(1, [2, 2, 2, 2])
            qk_sched(5, [3, 3, 3, 3])
            qk_sched(2, [4, 4, 5, 5])
            qk_sched(6, [5, 6, 6, 6])
            qk_sched(3, [7, 7, 8, 9])
            qk_sched(7, [8, 9, 10, 11])
            WQK_LOAD_EP = {1: 1, 5: 2, 2: 3, 6: 4, 3: 5, 7: 6}
            # C: last window's pairs-0..2 partial woven in ep 12 (pr0-2 OT is
            # complete after ep 11); full windows 0..2 one epoch after their
            # transposes; window-3 finishers drain in the tail.
            for lt in range(3 * NQT, 4 * NQT):
                for co in range(C // QT):
                    extra[11].append((1536, (lambda lt=lt, co=co: emit_ca(lt, co))))
            for w in range(3):
                for lt in range(w * NQT, (w + 1) * NQT):
                    for co in range(C // QT):
                        extra[12 + w].append((2048, (lambda lt=lt, co=co: emit_c(lt, co))))

            # ---- phase B driver ----
            for pr in range(NPAIR):
                for qt in range(NQT):
                    ep = pr * NQT + qt
                    epoch_start(ep)
                    if pr == 0 and qt == 0:
                        emit_qk_lq(0, 0)
                        emit_qk_lq(4, 0)
                    qT_t, kT_t = qkT[pr], qkT[4 + pr]
                    qsl = slice(qt * QT, (qt + 1) * QT)
                    pt = pt_pool.tile([128, NS, QT], BF16, tag="pt", name="pt")
                    for g0 in range(0, NS, GRP):
                        if pr == 0 and qt == 0 and g0 // 2 in (1, 3, 5):
                            # stream pair-0 q/k projection just ahead of the
                            # score chunks that consume it
                            lq = {1: 1, 3: 2, 5: 3}[g0 // 2]
                            emit_qk_lq(0, lq)
                            emit_qk_lq(4, lq)
                        sc = ps_sc.tile([128, GRP, QT], F32, tag="sc")
                        for j in range(GRP):
                            s = g0 + j
                            kt, hd = s // 2, s % 2
                            nc.tensor.matmul(
                                sc[:, j, :],
                                kT_t[hd * 64:(hd + 1) * 64, kt * 128:(kt + 1) * 128],
                                qT_t[hd * 64:(hd + 1) * 64, qsl],
                                start=True,
                                stop=True,
                                tile_position=(hd * 64, 0),
                            )
                        nc.scalar.activation(
                            pt[:, g0:g0 + GRP, :], sc[:], EXP, scale=float(D) ** -0.5
                        )
                        if not (pr == 0 and qt == 0):
                            pump(g0 // GRP)
                    # queue this qtile's AV + norm + transpose (+ C for pr3):
                    # they weave through the next qtile's score stream and must
                    # be emitted before epoch ep+2 reuses the pt buffer.
                    last_ep = ep == NPAIR * NQT - 1
                    tq = {}
                    for qc in range(QT // 128):
                        fillers.append(
                            (2080, (lambda pr=pr, qt=qt, qc=qc, pt=pt, tq=tq:
                                    emit_av(pr, qt, qc, pt, tq)), ep)
                        )
                        if qc > 0:
                            fillers.append(
                                (150, (lambda pr=pr, qt=qt, qc=qc, tq=tq:
                                       emit_transpose(pr, qt, qc - 1, tq[qc - 1])), ep)
                            )
                            if last_ep:
                                lt = qt * NQT + qc - 1
                                for co in range(C // QT):
                                    fillers.append(
                                        (1024, (lambda lt=lt, co=co: emit_cb(lt, co)), ep)
                                    )
                    fillers.append(
                        (150, (lambda pr=pr, qt=qt, tq=tq:
                               emit_transpose(pr, qt, NQT - 1, tq[NQT - 1])), ep)
                    )
                    if last_ep:
                        lt = qt * NQT + NQT - 1
                        for co in range(C // QT):
                            fillers.append(
                                (1024, (lambda lt=lt, co=co: emit_cb(lt, co)), ep)
                            )
                    for ent in extra.get(ep, ()):
                        fillers.append((ent[0], ent[1], ep))
                    for ft, lep in WQK_LOAD_EP.items():
                        if lep == ep:
                            load_wqk(ft)
            drain_all()

    nc.compile()
    return nc


def _get_nc():
    if "nc" not in _built:
        _built["nc"] = _build(None)
    return _built["nc"]


def _rope_perm():
    """Within-head row permutation: quadrant-local [evens(16) | odds(16)]."""
    perm = np.empty(64, np.int64)
    for j in range(2):
        for i in range(32):
            perm[j * 32 + i] = 2 * (j * 16 + i) if i < 16 else 2 * (j * 16 + i - 16) + 1
    return perm


def _shard_inputs(x, cos, sin, w_qkv, w_proj):
    import ml_dtypes

    Bb = ml_dtypes.bfloat16
    perm = _rope_perm()
    p = np.arange(128)
    quad, i = p // 32, p % 32
    pairidx = (quad % 2) * 16 + (i % 16)
    sign = np.where(i < 16, -1.0, 1.0).astype(np.float32)
    cos4 = np.ascontiguousarray(cos[:, pairidx].T).astype(Bb)              # [128, L]
    sin4 = np.ascontiguousarray((sin[:, pairidx] * sign[None, :]).T).astype(Bb)
    ident = np.eye(128, dtype=np.float32).astype(Bb)

    in_maps = []
    for c in range(NCORES):
        b, hg = c // 2, c % 2
        xT = np.ascontiguousarray(
            x[b].T.reshape(C // 128, 128, L).transpose(1, 0, 2)
        ).astype(Bb)  # [p, kc, l]

        rows = np.empty((8, 128), np.int64)
        for ft in range(8):
            t = 0 if ft < 4 else 1
            pr = ft % 4
            for fi in range(128):
                head = hg * 8 + 2 * pr + (0 if fi < 64 else 1)
                rows[ft, fi] = t * C + head * D + perm[fi % 64]
        wq = w_qkv[rows.reshape(-1)].reshape(8, 128, C // 128, 128)  # [ft, f, kc, p]
        wqkT = np.ascontiguousarray(wq.transpose(0, 3, 2, 1)).astype(Bb)  # [ft,p,kc,f]

        wv = w_qkv[2 * C + hg * 512: 2 * C + hg * 512 + 512]         # [fv, c]
        wvT = np.ascontiguousarray(
            wv.T.reshape(C // 128, 128, 512).transpose(1, 0, 2)
        ).astype(Bb)  # [p, kc, fv]

        wp = w_proj[:, hg * 512: hg * 512 + 512]                     # [co, d']
        wpT = np.ascontiguousarray(
            wp.T.reshape(4, 128, C).transpose(1, 0, 2)
        ).astype(Bb)  # [p, kd, co]

        in_maps.append(
            {
                "xT": xT, "wqkT": wqkT, "wvT": wvT, "wpT": wpT,
                "cos4": cos4, "sin4": sin4, "ident": ident,
            }
        )
    return in_maps


def kernel(x, cos, sin, w_qkv, w_proj, b_proj, _trace=False):
    from concourse.bass_utils import run_bass_kernel_spmd

    x = np.asarray(x, dtype=np.float32)
    cos = np.asarray(cos, dtype=np.float32)
    sin = np.asarray(sin, dtype=np.float32)
    w_qkv = np.asarray(w_qkv, dtype=np.float32)
    w_proj = np.asarray(w_proj, dtype=np.float32)
    b_proj = np.asarray(b_proj, dtype=np.float32)

    nc = _get_nc()
    in_maps = _shard_inputs(x, cos, sin, w_qkv, w_proj)
    res = run_bass_kernel_spmd(
        nc, in_maps, core_ids=list(range(NCORES)), trace=_trace
    )
    if _trace:
        print("exec_time_ns:", res.exec_time_ns)

    out = np.empty((B, L, C), dtype=np.float32)
    for b in range(B):
        p0 = res.results[2 * b]["outp"].reshape(L, C)
        p1 = res.results[2 * b + 1]["outp"].reshape(L, C)
        out[b] = p0 + p1
        p0b = res.results[2 * b]["outpb"].astype(np.float32).reshape(512, C)
        p1b = res.results[2 * b + 1]["outpb"].astype(np.float32).reshape(512, C)
        out[b, 1536:2048] = p0b + p1b
    out += b_proj[None, None, :]
    return out


# revision 60
# speedup vs baseline: 1.3393x; 1.0030x over previous
"""Multi-head attention (B=4, L=2048, C=1024, H=16, D=64) on 8 TRN2 NeuronCores.

Sharding: core c handles batch b = c//2 and head-group hg = c%2 (8 heads).
Megatron-style: w_qkv column-sharded, w_proj row-sharded; the proj all-reduce
(2 cores per batch) happens on the host during unshard.

All-bf16 dataflow (matmul cost: 1.0 cyc/row at any free size, keyed on the
moving operand's dtype; measured end-to-end max rel err ~1e-2 vs 2e-2 gate):

  scores:  S^T[k128, q512] = kT.T @ qT per (kt, hd); one PSUM bank per mm
           (tile_position packs the 2 heads on the 128 partitions).
  exp:     ScalarE Exp (scale 1/8 fused, no max-subtraction; |scores|<~6.5)
           PSUM -> persistent SBUF tile pt[128, 32, 512] bf16 per qtile,
           double-buffered.  The Activation engine is the second wall
           (~267us busy: 218us roofline + 185ns/inst access overhead).
  attn@V:  FLIPPED: O[q128, 65] += pt[:, 2kt+hd, qc*128:+128].T @ V'[k,65]
           over 16 kt; 65 = 64 dims + ones column so the softmax denom
           lands per-partition.  Free size 65 (vs 512 in the O^T form)
           halves attn@V PE cost since cost = output free size.
  norm:    DVE reciprocal [128,1] + per-partition tensor_scalar -> bf16
           O_n[q, 2, 64]; PE transpose via identity matmul -> OT (SBUF,
           no DRAM roundtrip); out-proj consumes OT chunks.
  proj:    out[l128, co512] over 4 pair-chunks; the last l-window is split
           into a pairs-0..2 partial (computed early, parked bf16) plus a
           tail finisher (pair-3 mm + identity-mm folding the partial into
           the same PSUM group) so only ~2 mms/chunk trail the last exp;
           its outputs ship bf16 to halve the final DMA drain.

Scheduling: the PE executes in program order, so all non-score PE work
(V-proj, q/k projections+RoPE for later pairs, attn@V of the previous
qtile, transposes, out-proj) is chopped into ~0.9us chunks and woven
between score groups by a debt/deadline pump ("fillers"), keeping the
Activation engine fed continuously.  Correctness constraint: a qtile's
attn@V must be emitted before the exp two qtiles later reuses its pt pool
slot (emission-order WAR), enforced by epoch deadlines + hard drain.

PSUM budget (8 banks): ps_sc 2 tiles x 2 banks (scores/exp double buffer;
ZERO_REGION = whole bank forbids co-tenant accumulation groups, capping
exp at 1024 elem/inst) + ps_ms 4 x 1 bank shared by qkv-proj psum, the
two attn@V accumulators, transpose dests, and out-proj psum.
"""

import sys

sys.path.insert(0, "/opt/trn_rl_repo")

import numpy as np

B, L, C, H, D = 4, 2048, 1024, 16, 64
NCORES = 8
QT = 512          # q-tile; one score mm per PSUM bank
GRP = 2           # score banks per exp group (exp ap = GRP*QT)
_built = {}


def _build(nc_mod):
    """Build the per-core Bass program (identical on all cores)."""
    import concourse.mybir as mybir
    import concourse.tile as tile
    from concourse import bacc
    from concourse.alu_op_type import AluOpType

    F32 = mybir.dt.float32
    BF16 = mybir.dt.bfloat16
    EXP = mybir.ActivationFunctionType.Exp
    MULT = AluOpType.mult
    ADD = AluOpType.add

    NKC = C // 128          # 8 contraction chunks for qkv proj
    NLT = L // 128          # 16 l-tiles (V rows, proj rows, k-chunks)
    NQT = L // QT           # 4 q-tiles per pair
    NPAIR = 4               # head pairs per core
    FV = 512                # v features per core
    VW = 65                 # V columns incl. ones
    NS = 2 * NLT            # 32 (kt, hd) score slices per qtile

    nc = bacc.Bacc(None, target_bir_lowering=False)

    xT_d = nc.dram_tensor("xT", [128, NKC, L], BF16, kind="ExternalInput")
    wqkT_d = nc.dram_tensor("wqkT", [8, 128, NKC, 128], BF16, kind="ExternalInput")
    wvT_d = nc.dram_tensor("wvT", [128, NKC, FV], BF16, kind="ExternalInput")
    wpT_d = nc.dram_tensor("wpT", [128, NPAIR, C], BF16, kind="ExternalInput")
    cos4_d = nc.dram_tensor("cos4", [128, L], BF16, kind="ExternalInput")
    sin4_d = nc.dram_tensor("sin4", [128, L], BF16, kind="ExternalInput")
    ident_d = nc.dram_tensor("ident", [128, 128], BF16, kind="ExternalInput")
    outp_d = nc.dram_tensor("outp", [NLT, 128, C], F32, kind="ExternalOutput")
    # last-window outputs go out in bf16 so the end-of-kernel DMA drain is
    # half as long (quantization adds ~0.2% of max, within budget)
    outpb_d = nc.dram_tensor("outpb", [4, 128, C], BF16, kind="ExternalOutput")

    SWAP_MASK = list(range(16, 32)) + list(range(16))

    with tile.TileContext(nc) as tc:
        import contextlib

        with contextlib.ExitStack() as outer:
            persist = outer.enter_context(tc.tile_pool(name="persist", bufs=1))
            qk_pool = outer.enter_context(tc.tile_pool(name="qkt", bufs=5))
            pt_pool = outer.enter_context(tc.tile_pool(name="pt", bufs=2))
            on_pool = outer.enter_context(tc.tile_pool(name="on", bufs=4))
            rc_pool = outer.enter_context(tc.tile_pool(name="rc", bufs=6))
            tpool = outer.enter_context(tc.tile_pool(name="tmp", bufs=4))
            ob_pool = outer.enter_context(tc.tile_pool(name="ob", bufs=4))
            oba_pool = outer.enter_context(tc.tile_pool(name="oba", bufs=9))
            ps_sc = outer.enter_context(tc.tile_pool(name="ps_sc", bufs=2, space="PSUM"))
            ps_ms = outer.enter_context(tc.tile_pool(name="ps_ms", bufs=4, space="PSUM"))

            # ---- persistent tensors ----
            V_t = persist.tile([128, NLT, 8, VW], BF16, tag="V")
            OT_t = persist.tile([128, NPAIR, L], BF16, tag="OT")
            xT_t = persist.tile([128, NKC, L], BF16, tag="xT")
            wvT_t = persist.tile([128, NKC, FV], BF16, tag="wv")
            wpT_t = persist.tile([128, NPAIR, C], BF16, tag="wp")
            cos4_t = persist.tile([128, L], BF16, tag="cos")
            sin4_t = persist.tile([128, L], BF16, tag="sin")
            ident_t = persist.tile([128, 128], BF16, tag="id")
            ones_t = persist.tile([128, NLT, 8], BF16, tag="ones")

            # ---- input DMAs (front section; wpT deferred to phase C) ----
            wqk = {}

            def load_wqk(ft, split=False):
                wqk[ft] = qkw_pool.tile([128, NKC, 128], BF16, tag="wqk", name=f"wqk{ft}")
                if split:
                    nc.sync.dma_start(wqk[ft][:, 0:2], wqkT_d[ft, :, 0:2])
                    nc.sync.dma_start(wqk[ft][:, 2:NKC], wqkT_d[ft, :, 2:NKC])
                else:
                    nc.sync.dma_start(wqk[ft][:], wqkT_d[ft])

            qkw_pool = outer.enter_context(tc.tile_pool(name="qkw", bufs=4))
            # DMA order matches first-consumption: wqk0's first chunks, the
            # lq0 x slices, the lq0 cos/sin slices (for the first ropes), the
            # rest of wqk0 and wqk4, then everything else.
            load_wqk(0)
            nc.sync.dma_start(xT_t[:, 0:4, 0:QT], xT_d[:, 0:4, 0:QT])
            nc.sync.dma_start(xT_t[:, 4:NKC, 0:QT], xT_d[:, 4:NKC, 0:QT])
            nc.sync.dma_start(cos4_t[:, 0:QT], cos4_d[:, 0:QT])
            nc.sync.dma_start(sin4_t[:, 0:QT], sin4_d[:, 0:QT])
            load_wqk(4)
            nc.sync.dma_start(cos4_t[:, QT:], cos4_d[:, QT:])
            nc.sync.dma_start(sin4_t[:, QT:], sin4_d[:, QT:])
            for lq in range(1, NQT):
                sl = slice(lq * QT, (lq + 1) * QT)
                nc.sync.dma_start(xT_t[:, :, sl], xT_d[:, :, sl])
            nc.sync.dma_start(wvT_t[:], wvT_d[:])
            nc.sync.dma_start(ident_t[:], ident_d[:])

            # warm the Exp activation table while the input DMAs stream in so
            # the first real exp doesn't pay the 1283ns table load
            warm = tpool.tile([1, 1], F32, tag="warm", name="warm")
            nc.vector.memset(warm[:], 0.0)
            nc.scalar.activation(warm[:], warm[:], EXP)

            nc.vector.memset(ones_t[:], 1.0)
            nc.vector.tensor_copy(V_t[:, :, :, 64:65], ones_t[:, :, :, None])

            # ---- A2 helper: qT/kT for one f-tile (one pair, q or k) ----
            qkT = {}

            qk_acc = {}

            def emit_qk_lq(ft, lq, half=None):
                """Project + rope one 512-l chunk of f-tile ft into qkT[ft].
                half=0 emits the first 4 contraction chunks (PSUM group stays
                open), half=1 finishes and ropes; None does both."""
                if ft not in qkT:
                    qkT[ft] = qk_pool.tile([128, L], BF16, tag="qkt", name=f"qkT{ft}")
                dst = qkT[ft]
                sl = slice(lq * QT, (lq + 1) * QT)
                if half in (0, None):
                    qps = ps_ms.tile([128, QT], F32, tag="ms", name="qps")
                    qk_acc[ft, lq] = qps
                    kcs = range(0, 4 if half == 0 else NKC)
                else:
                    qps = qk_acc.pop((ft, lq))
                    kcs = range(4, NKC)
                for kc in kcs:
                    nc.tensor.matmul(
                        qps[:],
                        wqk[ft][:, kc, :],
                        xT_t[:, kc, sl],
                        start=(kc == 0),
                        stop=(kc == NKC - 1),
                    )
                if half == 0:
                    return
                # RoPE in bf16: dst = qb*cos4 + swap(qb)*sin4s
                qb = tpool.tile([128, QT], BF16, tag="qb")
                shufb = tpool.tile([128, QT], BF16, tag="shufb")
                nc.vector.tensor_copy(qb[:], qps[:])
                nc.vector.stream_shuffle(shufb[:], qb[:], SWAP_MASK)
                nc.vector.tensor_tensor(dst[:, sl], qb[:], cos4_t[:, sl], op=MULT)
                nc.vector.tensor_tensor(shufb[:], shufb[:], sin4_t[:, sl], op=MULT)
                nc.vector.tensor_tensor(dst[:, sl], dst[:, sl], shufb[:], op=ADD)

            a1_acc = {}

            def emit_a1_lt(lt, half=None):
                if half in (0, None):
                    vps = ps_ms.tile([128, FV], F32, tag="ms", name="vps")
                    a1_acc[lt] = vps
                    kcs = range(0, 4 if half == 0 else NKC)
                else:
                    vps = a1_acc.pop(lt)
                    kcs = range(4, NKC)
                for kc in kcs:
                    nc.tensor.matmul(
                        vps[:],
                        xT_t[:, kc, lt * 128:(lt + 1) * 128],
                        wvT_t[:, kc, :],
                        start=(kc == 0),
                        stop=(kc == NKC - 1),
                    )
                if half == 0:
                    return
                nc.vector.tensor_copy(V_t[:, lt, :, 0:64], vps[:])

            nc.sync.dma_start(wpT_t[:], wpT_d[:])

            # ---- phase C helpers: out-proj split into a pairs-0..2 partial
            # (weavable as soon as pair 2 finishes) and a pair-3 finisher so
            # only one matmul per output chunk trails the last attention tile.
            oba = {}

            def emit_c(lt, co):
                """Full 4-pair out-proj chunk + DVE copy + DMA (windows 0-2)."""
                pps = ps_ms.tile([128, QT], F32, tag="ms", name="pps")
                for kd in range(NPAIR):
                    nc.tensor.matmul(
                        pps[:],
                        OT_t[:, kd, lt * 128:(lt + 1) * 128],
                        wpT_t[:, kd, co * QT:(co + 1) * QT],
                        start=(kd == 0),
                        stop=(kd == NPAIR - 1),
                    )
                ob = ob_pool.tile([128, QT], F32, tag="ob")
                nc.vector.tensor_copy(ob[:], pps[:])
                nc.sync.dma_start(outp_d[lt, :, co * QT:(co + 1) * QT], ob[:])

            def emit_ca(lt, co):
                """Pairs 0-2 partial for the last window, parked in bf16."""
                pps = ps_ms.tile([128, QT], F32, tag="ms", name="ppsa")
                for kd in range(NPAIR - 1):
                    nc.tensor.matmul(
                        pps[:],
                        OT_t[:, kd, lt * 128:(lt + 1) * 128],
                        wpT_t[:, kd, co * QT:(co + 1) * QT],
                        start=(kd == 0),
                        stop=(kd == NPAIR - 2),
                    )
                t = oba_pool.tile([128, QT], BF16, tag="oba", name="oba")
                nc.vector.tensor_copy(t[:], pps[:])
                oba[lt, co] = t

            def emit_cb(lt, co):
                """Tail finisher: pair-3 matmul + identity-matmul to fold the
                parked partial into the same PSUM group (no DVE in the tail);
                copy-out on the idle Act engine."""
                pps = ps_ms.tile([128, QT], F32, tag="ms", name="ppsb")
                nc.tensor.matmul(
                    pps[:],
                    OT_t[:, NPAIR - 1, lt * 128:(lt + 1) * 128],
                    wpT_t[:, NPAIR - 1, co * QT:(co + 1) * QT],
                    start=True,
                    stop=False,
                )
                nc.tensor.matmul(
                    pps[:],
                    ident_t[:],
                    oba[lt, co][:],
                    start=False,
                    stop=True,
                )
                ob = ob_pool.tile([128, QT], BF16, tag="ob", name="obb")
                nc.vector.tensor_copy(ob[:], pps[:])
                nc.sync.dma_start(outpb_d[lt - 12, :, co * QT:(co + 1) * QT], ob[:])

            def emit_av(pr, qt, qc, pt, tq):
                """Flipped attn@V for one 128-q chunk (both heads) + norm."""
                av = {}
                for hd in range(2):
                    av[hd] = ps_ms.tile([128, QT], F32, tag="ms", name=f"av{hd}")
                    for kt in range(NLT):
                        nc.tensor.matmul(
                            av[hd][:, 0:VW],
                            pt[:, 2 * kt + hd, qc * 128:(qc + 1) * 128],
                            V_t[:, kt, pr * 2 + hd, :],
                            start=(kt == 0),
                            stop=(kt == NLT - 1),
                        )
                on = on_pool.tile([128, 2, 64], BF16, tag="on")
                for hd in range(2):
                    rc = rc_pool.tile([128, 1], F32, tag="rc")
                    nc.vector.reciprocal(rc[:], av[hd][:, 64:65])
                    nc.vector.tensor_scalar(
                        on[:, hd, :], av[hd][:, 0:64], rc[:], None, op0=MULT
                    )
                tq[qc] = on

            def emit_transpose(pr, qt, qc, on):
                tp = ps_ms.tile([128, QT], F32, tag="ms", name="tp")
                tpb = tp.bitcast(BF16)
                nc.tensor.transpose(
                    tpb[:, 0:128], on[:].rearrange("p a b -> p (a b)"), ident_t[:]
                )
                dst = OT_t[:, pr, qt * QT + qc * 128: qt * QT + (qc + 1) * 128]
                nc.vector.tensor_copy(dst, tpb[:, 0:128])

            # ---- interleaved emission: weave filler PE chunks between score
            # groups so the Activation engine (the per-qtile bottleneck) never
            # starves behind the in-order PE stream.
            import collections

            fillers = collections.deque()  # (cost_cycles, fn, epoch)
            debt = [0.0]
            need = [0.0]   # deadline-critical cycles to force-spread this epoch
            forced = [0.0]
            PUMP = 1400.0  # PE filler cycles per score group (Act group ~1.04us)
            NGROUPS = 2 * (L // 128) // GRP

            def epoch_start(ep):
                # hard drain: entries tagged <= ep-2 must precede this epoch's
                # pt allocation (emission-order WAR on the pt pool slot);
                # normally empty because spreading finished them in ep-1
                while fillers and fillers[0][2] <= ep - 2:
                    _, fn, _ = fillers.popleft()
                    fn()
                # spread target: entries tagged <= ep-1 finish within this
                # epoch, woven across its score groups
                n = 0.0
                for c, _, e in fillers:
                    if e > ep - 1:
                        break
                    n += c
                need[0] = n
                forced[0] = 0.0
                debt[0] = 0.0

            def pump(g):
                # spread deadline-critical work across the epoch's groups,
                # plus opportunistic pumping at the steady rate; cap per-group
                # emission so a filler burst never blocks the score stream
                # long enough to starve the Activation engine
                target = need[0] * (g + 1) / NGROUPS
                cap = max(3000.0, need[0] / NGROUPS + 1200.0)
                debt[0] += PUMP
                emitted = 0.0
                while fillers and emitted < cap and (
                    forced[0] < target or debt[0] >= fillers[0][0]
                ):
                    c, fn, _ = fillers.popleft()
                    fn()
                    forced[0] += c
                    debt[0] -= c
                    emitted += c

            def drain_all():
                while fillers:
                    _, fn, _ = fillers.popleft()
                    fn()

            # A1 V-proj chunks: FIFO-ahead of qt0's AV, deadline epoch 0
            for lt in range(NLT):
                fillers.append((2048, (lambda lt=lt: emit_a1_lt(lt, 0)), 0))
                fillers.append((2200, (lambda lt=lt: emit_a1_lt(lt, 1)), 0))

            # Explicit target schedule for A2 f-tile chunks and phase C:
            # extra[ep] = chunks queued at the END of epoch ep (tag ep, woven
            # during ep+1), chosen to fill otherwise-idle epochs while
            # respecting qkT/qkw pool-slot reuse (a pair's tiles are only
            # reused two pairs later) and OT availability for C.
            extra = collections.defaultdict(list)

            def qk_sched(ft, targets):
                for lq, tgt in enumerate(targets):
                    extra[tgt - 1].append((2048, lambda ft=ft, lq=lq: emit_qk_lq(ft, lq, 0)))
                    extra[tgt - 1].append((2300, lambda ft=ft, lq=lq: emit_qk_lq(ft, lq, 1)))

            qk_sched(1, [2, 2, 2, 2])
            qk_sched(5, [3, 3, 3, 3])
            qk_sched(2, [4, 4, 5, 5])
            qk_sched(6, [5, 6, 6, 6])
            qk_sched(3, [7, 7, 8, 9])
            qk_sched(7, [8, 9, 10, 11])
            WQK_LOAD_EP = {1: 1, 5: 2, 2: 3, 6: 4, 3: 5, 7: 6}
            # C: last window's pairs-0..2 partial woven in ep 12 (pr0-2 OT is
            # complete after ep 11); full windows 0..2 one epoch after their
            # transposes; window-3 finishers drain in the tail.
            for lt in range(3 * NQT, 4 * NQT):
                for co in range(C // QT):
                    extra[11].append((1536, (lambda lt=lt, co=co: emit_ca(lt, co))))
            for w in range(3):
                for lt in range(w * NQT, (w + 1) * NQT):
                    for co in range(C // QT):
                        extra[12 + w].append((2048, (lambda lt=lt, co=co: emit_c(lt, co))))

            # ---- phase B driver ----
            for pr in range(NPAIR):
                for qt in range(NQT):
                    ep = pr * NQT + qt
                    epoch_start(ep)
                    if pr == 0 and qt == 0:
                        emit_qk_lq(0, 0)
                        emit_qk_lq(4, 0)
                    qT_t, kT_t = qkT[pr], qkT[4 + pr]
                    qsl = slice(qt * QT, (qt + 1) * QT)
                    pt = pt_pool.tile([128, NS, QT], BF16, tag="pt", name="pt")
                    for g0 in range(0, NS, GRP):
                        if pr == 0 and qt == 0 and g0 // 2 in (1, 3, 5):
                            # stream pair-0 q/k projection just ahead of the
                            # score chunks that consume it
                            lq = {1: 1, 3: 2, 5: 3}[g0 // 2]
                            emit_qk_lq(0, lq)
                            emit_qk_lq(4, lq)
                        sc = ps_sc.tile([128, GRP, QT], F32, tag="sc")
                        for j in range(GRP):
                            s = g0 + j
                            kt, hd = s // 2, s % 2
                            nc.tensor.matmul(
                                sc[:, j, :],
                                kT_t[hd * 64:(hd + 1) * 64, kt * 128:(kt + 1) * 128],
                                qT_t[hd * 64:(hd + 1) * 64, qsl],
                                start=True,
                                stop=True,
                                tile_position=(hd * 64, 0),
                            )
                        nc.scalar.activation(
                            pt[:, g0:g0 + GRP, :], sc[:], EXP, scale=float(D) ** -0.5
                        )
                        if not (pr == 0 and qt == 0):
                            pump(g0 // GRP)
                    # queue this qtile's AV + norm + transpose (+ C for pr3):
                    # they weave through the next qtile's score stream and must
                    # be emitted before epoch ep+2 reuses the pt buffer.
                    last_ep = ep == NPAIR * NQT - 1
                    tq = {}
                    for qc in range(QT // 128):
                        fillers.append(
                            (2080, (lambda pr=pr, qt=qt, qc=qc, pt=pt, tq=tq:
                                    emit_av(pr, qt, qc, pt, tq)), ep)
                        )
                        if qc > 0:
                            fillers.append(
                                (150, (lambda pr=pr, qt=qt, qc=qc, tq=tq:
                                       emit_transpose(pr, qt, qc - 1, tq[qc - 1])), ep)
                            )
                            if last_ep:
                                lt = qt * NQT + qc - 1
                                for co in range(C // QT):
                                    fillers.append(
                                        (1024, (lambda lt=lt, co=co: emit_cb(lt, co)), ep)
                                    )
                    fillers.append(
                        (150, (lambda pr=pr, qt=qt, tq=tq:
                               emit_transpose(pr, qt, NQT - 1, tq[NQT - 1])), ep)
                    )
                    if last_ep:
                        lt = qt * NQT + NQT - 1
                        for co in range(C // QT):
                            fillers.append(
                                (1024, (lambda lt=lt, co=co: emit_cb(lt, co)), ep)
                            )
                    for ent in extra.get(ep, ()):
                        fillers.append((ent[0], ent[1], ep))
                    for ft, lep in WQK_LOAD_EP.items():
                        if lep == ep:
                            load_wqk(ft)
            drain_all()

    nc.compile()
    return nc


def _get_nc():
    if "nc" not in _built:
        _built["nc"] = _build(None)
    return _built["nc"]


def _rope_perm():
    """Within-head row permutation: quadrant-local [evens(16) | odds(16)]."""
    perm = np.empty(64, np.int64)
    for j in range(2):
        for i in range(32):
            perm[j * 32 + i] = 2 * (j * 16 + i) if i < 16 else 2 * (j * 16 + i - 16) + 1
    return perm


def _shard_inputs(x, cos, sin, w_qkv, w_proj):
    import ml_dtypes

    Bb = ml_dtypes.bfloat16
    perm = _rope_perm()
    p = np.arange(128)
    quad, i = p // 32, p % 32
    pairidx = (quad % 2) * 16 + (i % 16)
    sign = np.where(i < 16, -1.0, 1.0).astype(np.float32)
    cos4 = np.ascontiguousarray(cos[:, pairidx].T).astype(Bb)              # [128, L]
    sin4 = np.ascontiguousarray((sin[:, pairidx] * sign[None, :]).T).astype(Bb)
    ident = np.eye(128, dtype=np.float32).astype(Bb)

    in_maps = []
    for c in range(NCORES):
        b, hg = c // 2, c % 2
        xT = np.ascontiguousarray(
            x[b].T.reshape(C // 128, 128, L).transpose(1, 0, 2)
        ).astype(Bb)  # [p, kc, l]

        rows = np.empty((8, 128), np.int64)
        for ft in range(8):
            t = 0 if ft < 4 else 1
            pr = ft % 4
            for fi in range(128):
                head = hg * 8 + 2 * pr + (0 if fi < 64 else 1)
                rows[ft, fi] = t * C + head * D + perm[fi % 64]
        wq = w_qkv[rows.reshape(-1)].reshape(8, 128, C // 128, 128)  # [ft, f, kc, p]
        wqkT = np.ascontiguousarray(wq.transpose(0, 3, 2, 1)).astype(Bb)  # [ft,p,kc,f]

        wv = w_qkv[2 * C + hg * 512: 2 * C + hg * 512 + 512]         # [fv, c]
        wvT = np.ascontiguousarray(
            wv.T.reshape(C // 128, 128, 512).transpose(1, 0, 2)
        ).astype(Bb)  # [p, kc, fv]

        wp = w_proj[:, hg * 512: hg * 512 + 512]                     # [co, d']
        wpT = np.ascontiguousarray(
            wp.T.reshape(4, 128, C).transpose(1, 0, 2)
        ).astype(Bb)  # [p, kd, co]

        in_maps.append(
            {
                "xT": xT, "wqkT": wqkT, "wvT": wvT, "wpT": wpT,
                "cos4": cos4, "sin4": sin4, "ident": ident,
            }
        )
    return in_maps


def kernel(x, cos, sin, w_qkv, w_proj, b_proj, _trace=False):
    from concourse.bass_utils import run_bass_kernel_spmd

    x = np.asarray(x, dtype=np.float32)
    cos = np.asarray(cos, dtype=np.float32)
    sin = np.asarray(sin, dtype=np.float32)
    w_qkv = np.asarray(w_qkv, dtype=np.float32)
    w_proj = np.asarray(w_proj, dtype=np.float32)
    b_proj = np.asarray(b_proj, dtype=np.float32)

    nc = _get_nc()
    in_maps = _shard_inputs(x, cos, sin, w_qkv, w_proj)
    res = run_bass_kernel_spmd(
        nc, in_maps, core_ids=list(range(NCORES)), trace=_trace
    )
    if _trace:
        print("exec_time_ns:", res.exec_time_ns)

    out = np.empty((B, L, C), dtype=np.float32)
    for b in range(B):
        p0 = res.results[2 * b]["outp"].reshape(L, C)
        p1 = res.results[2 * b + 1]["outp"].reshape(L, C)
        out[b] = p0 + p1
        p0b = res.results[2 * b]["outpb"].astype(np.float32).reshape(512, C)
        p1b = res.results[2 * b + 1]["outpb"].astype(np.float32).reshape(512, C)
        out[b, 1536:2048] = p0b + p1b
    out += b_proj[None, None, :]
    return out


# revision 72
# speedup vs baseline: 1.3472x; 1.0059x over previous
"""Multi-head attention (B=4, L=2048, C=1024, H=16, D=64) on 8 TRN2 NeuronCores.

Sharding: core c handles batch b = c//2 and head-group hg = c%2 (8 heads).
Megatron-style: w_qkv column-sharded, w_proj row-sharded; the proj all-reduce
(2 cores per batch) happens on the host during unshard.

All-bf16 dataflow (matmul cost: 1.0 cyc/row at any free size, keyed on the
moving operand's dtype; measured end-to-end max rel err ~1e-2 vs 2e-2 gate):

  scores:  S^T[k128, q512] = kT.T @ qT per (kt, hd); one PSUM bank per mm
           (tile_position packs the 2 heads on the 128 partitions).
  exp:     ScalarE Exp (scale 1/8 fused, no max-subtraction; |scores|<~6.5)
           PSUM -> persistent SBUF tile pt[128, 32, 512] bf16 per qtile,
           double-buffered.  The Activation engine is the second wall
           (~267us busy: 218us roofline + 185ns/inst access overhead).
  attn@V:  FLIPPED: O[q128, 65] += pt[:, 2kt+hd, qc*128:+128].T @ V'[k,65]
           over 16 kt; 65 = 64 dims + ones column so the softmax denom
           lands per-partition.  Free size 65 (vs 512 in the O^T form)
           halves attn@V PE cost since cost = output free size.
  norm:    DVE reciprocal [128,1] + per-partition tensor_scalar -> bf16
           O_n[q, 2, 64]; PE transpose via identity matmul -> OT (SBUF,
           no DRAM roundtrip); out-proj consumes OT chunks.
  proj:    out[l128, co512] over 4 pair-chunks; the last l-window is split
           into a pairs-0..2 partial (computed early, parked bf16) plus a
           tail finisher (pair-3 mm + identity-mm folding the partial into
           the same PSUM group) so only ~2 mms/chunk trail the last exp;
           its outputs ship bf16 to halve the final DMA drain.

Scheduling: the PE executes in program order, so all non-score PE work
(V-proj, q/k projections+RoPE for later pairs, attn@V of the previous
qtile, transposes, out-proj) is chopped into ~0.9us chunks and woven
between score groups by a debt/deadline pump ("fillers"), keeping the
Activation engine fed continuously.  Correctness constraint: a qtile's
attn@V must be emitted before the exp two qtiles later reuses its pt pool
slot (emission-order WAR), enforced by epoch deadlines + hard drain.

PSUM budget (8 banks): ps_sc 2 tiles x 2 banks (scores/exp double buffer;
ZERO_REGION = whole bank forbids co-tenant accumulation groups, capping
exp at 1024 elem/inst) + ps_ms 4 x 1 bank shared by qkv-proj psum, the
two attn@V accumulators, transpose dests, and out-proj psum.
"""

import sys

sys.path.insert(0, "/opt/trn_rl_repo")

import numpy as np

B, L, C, H, D = 4, 2048, 1024, 16, 64
NCORES = 8
QT = 512          # q-tile; one score mm per PSUM bank
GRP = 2           # score banks per exp group (exp ap = GRP*QT)
_built = {}


def _build(nc_mod):
    """Build the per-core Bass program (identical on all cores)."""
    import concourse.mybir as mybir
    import concourse.tile as tile
    from concourse import bacc
    from concourse.alu_op_type import AluOpType

    F32 = mybir.dt.float32
    BF16 = mybir.dt.bfloat16
    EXP = mybir.ActivationFunctionType.Exp
    MULT = AluOpType.mult
    ADD = AluOpType.add

    NKC = C // 128          # 8 contraction chunks for qkv proj
    NLT = L // 128          # 16 l-tiles (V rows, proj rows, k-chunks)
    NQT = L // QT           # 4 q-tiles per pair
    NPAIR = 4               # head pairs per core
    FV = 512                # v features per core
    VW = 65                 # V columns incl. ones
    NS = 2 * NLT            # 32 (kt, hd) score slices per qtile

    nc = bacc.Bacc(None, target_bir_lowering=False)

    xT_d = nc.dram_tensor("xT", [128, NKC, L], BF16, kind="ExternalInput")
    wqkT_d = nc.dram_tensor("wqkT", [8, 128, NKC, 128], BF16, kind="ExternalInput")
    wvT_d = nc.dram_tensor("wvT", [128, NKC, FV], BF16, kind="ExternalInput")
    wpT_d = nc.dram_tensor("wpT", [128, NPAIR, C], BF16, kind="ExternalInput")
    cos4_d = nc.dram_tensor("cos4", [128, L], BF16, kind="ExternalInput")
    sin4_d = nc.dram_tensor("sin4", [128, L], BF16, kind="ExternalInput")
    ident_d = nc.dram_tensor("ident", [128, 128], BF16, kind="ExternalInput")
    outp_d = nc.dram_tensor("outp", [NLT, 128, C], F32, kind="ExternalOutput")
    # last-window outputs go out in bf16 so the end-of-kernel DMA drain is
    # half as long (quantization adds ~0.2% of max, within budget)
    outpb_d = nc.dram_tensor("outpb", [4, 128, C], BF16, kind="ExternalOutput")

    SWAP_MASK = list(range(16, 32)) + list(range(16))

    with tile.TileContext(nc) as tc:
        import contextlib

        with contextlib.ExitStack() as outer:
            persist = outer.enter_context(tc.tile_pool(name="persist", bufs=1))
            qk_pool = outer.enter_context(tc.tile_pool(name="qkt", bufs=5))
            pt_pool = outer.enter_context(tc.tile_pool(name="pt", bufs=2))
            on_pool = outer.enter_context(tc.tile_pool(name="on", bufs=4))
            rc_pool = outer.enter_context(tc.tile_pool(name="rc", bufs=6))
            tpool = outer.enter_context(tc.tile_pool(name="tmp", bufs=4))
            ob_pool = outer.enter_context(tc.tile_pool(name="ob", bufs=4))
            oba_pool = outer.enter_context(tc.tile_pool(name="oba", bufs=9))
            ps_sc = outer.enter_context(tc.tile_pool(name="ps_sc", bufs=2, space="PSUM"))
            ps_ms = outer.enter_context(tc.tile_pool(name="ps_ms", bufs=4, space="PSUM"))

            # ---- persistent tensors ----
            V_t = persist.tile([128, NLT, 8, VW], BF16, tag="V")
            OT_t = persist.tile([128, NPAIR, L], BF16, tag="OT")
            xT_t = persist.tile([128, NKC, L], BF16, tag="xT")
            wvT_t = persist.tile([128, NKC, FV], BF16, tag="wv")
            wpT_t = persist.tile([128, NPAIR, C], BF16, tag="wp")
            cos4_t = persist.tile([128, L], BF16, tag="cos")
            sin4_t = persist.tile([128, L], BF16, tag="sin")
            ident_t = persist.tile([128, 128], BF16, tag="id")
            ones_t = persist.tile([128, NLT, 8], BF16, tag="ones")

            # ---- input DMAs (front section; wpT deferred to phase C) ----
            wqk = {}

            def load_wqk(ft, split=False):
                wqk[ft] = qkw_pool.tile([128, NKC, 128], BF16, tag="wqk", name=f"wqk{ft}")
                if split:
                    nc.sync.dma_start(wqk[ft][:, 0:2], wqkT_d[ft, :, 0:2])
                    nc.sync.dma_start(wqk[ft][:, 2:NKC], wqkT_d[ft, :, 2:NKC])
                else:
                    nc.sync.dma_start(wqk[ft][:], wqkT_d[ft])

            qkw_pool = outer.enter_context(tc.tile_pool(name="qkw", bufs=4))
            # DMA order matches first-consumption: wqk0's first chunks, the
            # lq0 x slices, the lq0 cos/sin slices (for the first ropes), the
            # rest of wqk0 and wqk4, then everything else.
            load_wqk(0)
            nc.sync.dma_start(xT_t[:, 0:4, 0:QT], xT_d[:, 0:4, 0:QT])
            nc.sync.dma_start(xT_t[:, 4:NKC, 0:QT], xT_d[:, 4:NKC, 0:QT])
            nc.sync.dma_start(cos4_t[:, 0:QT], cos4_d[:, 0:QT])
            nc.sync.dma_start(sin4_t[:, 0:QT], sin4_d[:, 0:QT])
            load_wqk(4)
            nc.sync.dma_start(cos4_t[:, QT:], cos4_d[:, QT:])
            nc.sync.dma_start(sin4_t[:, QT:], sin4_d[:, QT:])
            for lq in range(1, NQT):
                sl = slice(lq * QT, (lq + 1) * QT)
                nc.sync.dma_start(xT_t[:, :, sl], xT_d[:, :, sl])
            nc.sync.dma_start(wvT_t[:], wvT_d[:])
            nc.sync.dma_start(ident_t[:], ident_d[:])

            # warm the Exp activation table while the input DMAs stream in so
            # the first real exp doesn't pay the 1283ns table load
            warm = tpool.tile([1, 1], F32, tag="warm", name="warm")
            nc.vector.memset(warm[:], 0.0)
            nc.scalar.activation(warm[:], warm[:], EXP)

            nc.vector.memset(ones_t[:], 1.0)
            nc.vector.tensor_copy(V_t[:, :, :, 64:65], ones_t[:, :, :, None])

            # ---- A2 helper: qT/kT for one f-tile (one pair, q or k) ----
            qkT = {}

            qk_acc = {}

            def emit_qk_lq(ft, lq, half=None):
                """Project + rope one 512-l chunk of f-tile ft into qkT[ft].
                half=0 emits the first 4 contraction chunks (PSUM group stays
                open), half=1 finishes and ropes; None does both."""
                if ft not in qkT:
                    qkT[ft] = qk_pool.tile([128, L], BF16, tag="qkt", name=f"qkT{ft}")
                dst = qkT[ft]
                sl = slice(lq * QT, (lq + 1) * QT)
                if half in (0, None):
                    qps = ps_ms.tile([128, QT], F32, tag="ms", name="qps")
                    qk_acc[ft, lq] = qps
                    kcs = range(0, 4 if half == 0 else NKC)
                else:
                    qps = qk_acc.pop((ft, lq))
                    kcs = range(4, NKC)
                for kc in kcs:
                    nc.tensor.matmul(
                        qps[:],
                        wqk[ft][:, kc, :],
                        xT_t[:, kc, sl],
                        start=(kc == 0),
                        stop=(kc == NKC - 1),
                    )
                if half == 0:
                    return
                # RoPE in bf16: dst = qb*cos4 + swap(qb)*sin4s
                qb = tpool.tile([128, QT], BF16, tag="qb")
                shufb = tpool.tile([128, QT], BF16, tag="shufb")
                nc.vector.tensor_copy(qb[:], qps[:])
                nc.vector.stream_shuffle(shufb[:], qb[:], SWAP_MASK)
                nc.vector.tensor_tensor(dst[:, sl], qb[:], cos4_t[:, sl], op=MULT)
                nc.vector.tensor_tensor(shufb[:], shufb[:], sin4_t[:, sl], op=MULT)
                nc.vector.tensor_tensor(dst[:, sl], dst[:, sl], shufb[:], op=ADD)

            a1_acc = {}

            def emit_a1_lt(lt, half=None):
                if half in (0, None):
                    vps = ps_ms.tile([128, FV], F32, tag="ms", name="vps")
                    a1_acc[lt] = vps
                    kcs = range(0, 4 if half == 0 else NKC)
                else:
                    vps = a1_acc.pop(lt)
                    kcs = range(4, NKC)
                for kc in kcs:
                    nc.tensor.matmul(
                        vps[:],
                        xT_t[:, kc, lt * 128:(lt + 1) * 128],
                        wvT_t[:, kc, :],
                        start=(kc == 0),
                        stop=(kc == NKC - 1),
                    )
                if half == 0:
                    return
                nc.vector.tensor_copy(V_t[:, lt, :, 0:64], vps[:])

            nc.sync.dma_start(wpT_t[:], wpT_d[:])

            # ---- phase C helpers: out-proj split into a pairs-0..2 partial
            # (weavable as soon as pair 2 finishes) and a pair-3 finisher so
            # only one matmul per output chunk trails the last attention tile.
            oba = {}

            def emit_c(lt, co):
                """Full 4-pair out-proj chunk + DVE copy + DMA (windows 0-2)."""
                pps = ps_ms.tile([128, QT], F32, tag="ms", name="pps")
                for kd in range(NPAIR):
                    nc.tensor.matmul(
                        pps[:],
                        OT_t[:, kd, lt * 128:(lt + 1) * 128],
                        wpT_t[:, kd, co * QT:(co + 1) * QT],
                        start=(kd == 0),
                        stop=(kd == NPAIR - 1),
                    )
                ob = ob_pool.tile([128, QT], F32, tag="ob")
                nc.vector.tensor_copy(ob[:], pps[:])
                nc.sync.dma_start(outp_d[lt, :, co * QT:(co + 1) * QT], ob[:])

            def emit_ca(lt, co):
                """Pairs 0-2 partial for the last window, parked in bf16."""
                pps = ps_ms.tile([128, QT], F32, tag="ms", name="ppsa")
                for kd in range(NPAIR - 1):
                    nc.tensor.matmul(
                        pps[:],
                        OT_t[:, kd, lt * 128:(lt + 1) * 128],
                        wpT_t[:, kd, co * QT:(co + 1) * QT],
                        start=(kd == 0),
                        stop=(kd == NPAIR - 2),
                    )
                t = oba_pool.tile([128, QT], BF16, tag="oba", name="oba")
                nc.vector.tensor_copy(t[:], pps[:])
                oba[lt, co] = t

            def emit_cb(lt, co):
                """Tail finisher: pair-3 matmul + identity-matmul folding the
                parked partial into the same PSUM group.  Runs only in the
                drain tail, when the score banks are idle — borrow those so
                the av/transpose rotation keeps ps_ms to itself."""
                scb = ps_sc.tile([128, GRP, QT], F32, tag="sc", name="ppsb")
                pps = scb[:, 0, :]
                nc.tensor.matmul(
                    pps[:],
                    OT_t[:, NPAIR - 1, lt * 128:(lt + 1) * 128],
                    wpT_t[:, NPAIR - 1, co * QT:(co + 1) * QT],
                    start=True,
                    stop=False,
                )
                nc.tensor.matmul(
                    pps[:],
                    ident_t[:],
                    oba[lt, co][:],
                    start=False,
                    stop=True,
                )
                ob = ob_pool.tile([128, QT], BF16, tag="ob", name="obb")
                if co == 0:
                    nc.scalar.copy(ob[:], pps[:])
                else:
                    nc.vector.tensor_copy(ob[:], pps[:])
                nc.sync.dma_start(outpb_d[lt - 12, :, co * QT:(co + 1) * QT], ob[:])

            def emit_av(pr, qt, qc, pt, tq):
                """Flipped attn@V for one 128-q chunk (both heads) + norm."""
                av = {}
                for hd in range(2):
                    av[hd] = ps_ms.tile([128, QT], F32, tag="ms", name=f"av{hd}")
                    for kt in range(NLT):
                        nc.tensor.matmul(
                            av[hd][:, 0:VW],
                            pt[:, 2 * kt + hd, qc * 128:(qc + 1) * 128],
                            V_t[:, kt, pr * 2 + hd, :],
                            start=(kt == 0),
                            stop=(kt == NLT - 1),
                        )
                on = on_pool.tile([128, 2, 64], BF16, tag="on")
                for hd in range(2):
                    rc = rc_pool.tile([128, 1], F32, tag="rc")
                    nc.vector.reciprocal(rc[:], av[hd][:, 64:65])
                    nc.vector.tensor_scalar(
                        on[:, hd, :], av[hd][:, 0:64], rc[:], None, op0=MULT
                    )
                tq[qc] = on

            def emit_transpose(pr, qt, qc, on):
                tp = ps_ms.tile([128, QT], F32, tag="ms", name="tp")
                tpb = tp.bitcast(BF16)
                nc.tensor.transpose(
                    tpb[:, 0:128], on[:].rearrange("p a b -> p (a b)"), ident_t[:]
                )
                dst = OT_t[:, pr, qt * QT + qc * 128: qt * QT + (qc + 1) * 128]
                nc.vector.tensor_copy(dst, tpb[:, 0:128])

            # ---- interleaved emission: weave filler PE chunks between score
            # groups so the Activation engine (the per-qtile bottleneck) never
            # starves behind the in-order PE stream.
            import collections

            fillers = collections.deque()  # (cost_cycles, fn, epoch)
            debt = [0.0]
            need = [0.0]   # deadline-critical cycles to force-spread this epoch
            forced = [0.0]
            PUMP = 1400.0  # PE filler cycles per score group (Act group ~1.04us)
            NGROUPS = 2 * (L // 128) // GRP

            def epoch_start(ep):
                # hard drain: entries tagged <= ep-2 must precede this epoch's
                # pt allocation (emission-order WAR on the pt pool slot);
                # normally empty because spreading finished them in ep-1
                while fillers and fillers[0][2] <= ep - 2:
                    _, fn, _ = fillers.popleft()
                    fn()
                # spread target: entries tagged <= ep-1 finish within this
                # epoch, woven across its score groups
                n = 0.0
                for c, _, e in fillers:
                    if e > ep - 1:
                        break
                    n += c
                need[0] = n
                forced[0] = 0.0
                debt[0] = 0.0

            def pump(g):
                # spread deadline-critical work across the epoch's groups,
                # plus opportunistic pumping at the steady rate; cap per-group
                # emission so a filler burst never blocks the score stream
                # long enough to starve the Activation engine
                target = need[0] * (g + 1) / NGROUPS
                cap = max(3000.0, need[0] / NGROUPS + 1200.0)
                debt[0] += PUMP
                emitted = 0.0
                while fillers and emitted < cap and (
                    forced[0] < target or debt[0] >= fillers[0][0]
                ):
                    c, fn, _ = fillers.popleft()
                    fn()
                    forced[0] += c
                    debt[0] -= c
                    emitted += c

            def drain_all():
                while fillers:
                    _, fn, _ = fillers.popleft()
                    fn()

            # A1 V-proj chunks: FIFO-ahead of qt0's AV, deadline epoch 0
            for lt in range(NLT):
                fillers.append((2048, (lambda lt=lt: emit_a1_lt(lt, 0)), 0))
                fillers.append((2200, (lambda lt=lt: emit_a1_lt(lt, 1)), 0))

            # Explicit target schedule for A2 f-tile chunks and phase C:
            # extra[ep] = chunks queued at the END of epoch ep (tag ep, woven
            # during ep+1), chosen to fill otherwise-idle epochs while
            # respecting qkT/qkw pool-slot reuse (a pair's tiles are only
            # reused two pairs later) and OT availability for C.
            extra = collections.defaultdict(list)

            def qk_sched(ft, targets):
                for lq, tgt in enumerate(targets):
                    extra[tgt - 1].append((2048, lambda ft=ft, lq=lq: emit_qk_lq(ft, lq, 0)))
                    extra[tgt - 1].append((2300, lambda ft=ft, lq=lq: emit_qk_lq(ft, lq, 1)))

            # Deadline rule: a chunk targeted at epoch t completes by t's end,
            # so writers must target <= (first consuming epoch) - 1.  Pair p's
            # q-quarter lq is first read at epoch 4p+lq; its k-tile is read
            # from group 0 of epoch 4p (k targets <= 4p-1 strictly).  qkT slot
            # reuse (bufs=5) frees a tile two pairs later; first-chunk epochs
            # respect that.
            qk_sched(1, [2, 3, 4, 5])
            qk_sched(5, [2, 3, 3, 3])
            qk_sched(2, [5, 6, 6, 7])
            qk_sched(6, [6, 7, 7, 7])
            qk_sched(3, [8, 9, 10, 11])
            qk_sched(7, [9, 10, 11, 11])
            WQK_LOAD_EP = {1: 1, 5: 1, 2: 4, 6: 5, 3: 7, 7: 8}
            # C: last window's pairs-0..2 partial woven in ep 12 (pr0-2 OT is
            # complete after ep 11); full windows 0..2 one epoch after their
            # transposes; window-3 finishers drain in the tail.
            for lt in range(3 * NQT, 4 * NQT):
                for co in range(C // QT):
                    extra[11].append((1536, (lambda lt=lt, co=co: emit_ca(lt, co))))
            for w in range(3):
                for lt in range(w * NQT, (w + 1) * NQT):
                    for co in range(C // QT):
                        extra[12 + w].append((2048, (lambda lt=lt, co=co: emit_c(lt, co))))

            # ---- phase B driver ----
            for pr in range(NPAIR):
                for qt in range(NQT):
                    ep = pr * NQT + qt
                    epoch_start(ep)
                    if pr == 0 and qt == 0:
                        emit_qk_lq(0, 0)
                        emit_qk_lq(4, 0)
                    qT_t, kT_t = qkT[pr], qkT[4 + pr]
                    qsl = slice(qt * QT, (qt + 1) * QT)
                    pt = pt_pool.tile([128, NS, QT], BF16, tag="pt", name="pt")
                    for g0 in range(0, NS, GRP):
                        if pr == 0 and qt == 0 and g0 // 2 in (1, 3, 5):
                            # stream pair-0 q/k projection just ahead of the
                            # score chunks that consume it
                            lq = {1: 1, 3: 2, 5: 3}[g0 // 2]
                            emit_qk_lq(0, lq)
                            emit_qk_lq(4, lq)
                        sc = ps_sc.tile([128, GRP, QT], F32, tag="sc")
                        for j in range(GRP):
                            s = g0 + j
                            kt, hd = s // 2, s % 2
                            nc.tensor.matmul(
                                sc[:, j, :],
                                kT_t[hd * 64:(hd + 1) * 64, kt * 128:(kt + 1) * 128],
                                qT_t[hd * 64:(hd + 1) * 64, qsl],
                                start=True,
                                stop=True,
                                tile_position=(hd * 64, 0),
                            )
                        nc.scalar.activation(
                            pt[:, g0:g0 + GRP, :], sc[:], EXP, scale=float(D) ** -0.5
                        )
                        if not (pr == 0 and qt == 0):
                            pump(g0 // GRP)
                        elif g0 // GRP >= 6:
                            # pair-0 q/k emission is done by group 6; start
                            # draining the V-proj backlog under the remaining
                            # score groups
                            pump(g0 // GRP)
                    # queue this qtile's AV + norm + transpose (+ C for pr3):
                    # they weave through the next qtile's score stream and must
                    # be emitted before epoch ep+2 reuses the pt buffer.
                    last_ep = ep == NPAIR * NQT - 1
                    tq = {}
                    for qc in range(QT // 128):
                        fillers.append(
                            (2080, (lambda pr=pr, qt=qt, qc=qc, pt=pt, tq=tq:
                                    emit_av(pr, qt, qc, pt, tq)), ep)
                        )
                        if qc > 0:
                            fillers.append(
                                (150, (lambda pr=pr, qt=qt, qc=qc, tq=tq:
                                       emit_transpose(pr, qt, qc - 1, tq[qc - 1])), ep)
                            )
                            if last_ep:
                                lt = qt * NQT + qc - 1
                                for co in range(C // QT):
                                    fillers.append(
                                        (1024, (lambda lt=lt, co=co: emit_cb(lt, co)), ep)
                                    )
                    fillers.append(
                        (150, (lambda pr=pr, qt=qt, tq=tq:
                               emit_transpose(pr, qt, NQT - 1, tq[NQT - 1])), ep)
                    )
                    if last_ep:
                        lt = qt * NQT + NQT - 1
                        for co in range(C // QT):
                            fillers.append(
                                (1024, (lambda lt=lt, co=co: emit_cb(lt, co)), ep)
                            )
                    for ent in extra.get(ep, ()):
                        fillers.append((ent[0], ent[1], ep))
                    for ft, lep in WQK_LOAD_EP.items():
                        if lep == ep:
                            load_wqk(ft)
            drain_all()

    nc.compile()
    return nc


def _get_nc():
    if "nc" not in _built:
        _built["nc"] = _build(None)
    return _built["nc"]


def _rope_perm():
    """Within-head row permutation: quadrant-local [evens(16) | odds(16)]."""
    perm = np.empty(64, np.int64)
    for j in range(2):
        for i in range(32):
            perm[j * 32 + i] = 2 * (j * 16 + i) if i < 16 else 2 * (j * 16 + i - 16) + 1
    return perm


def _shard_inputs(x, cos, sin, w_qkv, w_proj):
    import ml_dtypes

    Bb = ml_dtypes.bfloat16
    perm = _rope_perm()
    p = np.arange(128)
    quad, i = p // 32, p % 32
    pairidx = (quad % 2) * 16 + (i % 16)
    sign = np.where(i < 16, -1.0, 1.0).astype(np.float32)
    cos4 = np.ascontiguousarray(cos[:, pairidx].T).astype(Bb)              # [128, L]
    sin4 = np.ascontiguousarray((sin[:, pairidx] * sign[None, :]).T).astype(Bb)
    ident = np.eye(128, dtype=np.float32).astype(Bb)

    in_maps = []
    for c in range(NCORES):
        b, hg = c // 2, c % 2
        xT = np.ascontiguousarray(
            x[b].T.reshape(C // 128, 128, L).transpose(1, 0, 2)
        ).astype(Bb)  # [p, kc, l]

        rows = np.empty((8, 128), np.int64)
        for ft in range(8):
            t = 0 if ft < 4 else 1
            pr = ft % 4
            for fi in range(128):
                head = hg * 8 + 2 * pr + (0 if fi < 64 else 1)
                rows[ft, fi] = t * C + head * D + perm[fi % 64]
        wq = w_qkv[rows.reshape(-1)].reshape(8, 128, C // 128, 128)  # [ft, f, kc, p]
        wqkT = np.ascontiguousarray(wq.transpose(0, 3, 2, 1)).astype(Bb)  # [ft,p,kc,f]

        wv = w_qkv[2 * C + hg * 512: 2 * C + hg * 512 + 512]         # [fv, c]
        wvT = np.ascontiguousarray(
            wv.T.reshape(C // 128, 128, 512).transpose(1, 0, 2)
        ).astype(Bb)  # [p, kc, fv]

        wp = w_proj[:, hg * 512: hg * 512 + 512]                     # [co, d']
        wpT = np.ascontiguousarray(
            wp.T.reshape(4, 128, C).transpose(1, 0, 2)
        ).astype(Bb)  # [p, kd, co]

        in_maps.append(
            {
                "xT": xT, "wqkT": wqkT, "wvT": wvT, "wpT": wpT,
                "cos4": cos4, "sin4": sin4, "ident": ident,
            }
        )
    return in_maps


def kernel(x, cos, sin, w_qkv, w_proj, b_proj, _trace=False):
    from concourse.bass_utils import run_bass_kernel_spmd

    x = np.asarray(x, dtype=np.float32)
    cos = np.asarray(cos, dtype=np.float32)
    sin = np.asarray(sin, dtype=np.float32)
    w_qkv = np.asarray(w_qkv, dtype=np.float32)
    w_proj = np.asarray(w_proj, dtype=np.float32)
    b_proj = np.asarray(b_proj, dtype=np.float32)

    nc = _get_nc()
    in_maps = _shard_inputs(x, cos, sin, w_qkv, w_proj)
    res = run_bass_kernel_spmd(
        nc, in_maps, core_ids=list(range(NCORES)), trace=_trace
    )
    if _trace:
        print("exec_time_ns:", res.exec_time_ns)

    out = np.empty((B, L, C), dtype=np.float32)
    for b in range(B):
        p0 = res.results[2 * b]["outp"].reshape(L, C)
        p1 = res.results[2 * b + 1]["outp"].reshape(L, C)
        out[b] = p0 + p1
        p0b = res.results[2 * b]["outpb"].astype(np.float32).reshape(512, C)
        p1b = res.results[2 * b + 1]["outpb"].astype(np.float32).reshape(512, C)
        out[b, 1536:2048] = p0b + p1b
    out += b_proj[None, None, :]
    return out


# revision 73
# speedup vs baseline: 1.3503x; 1.0023x over previous
"""Multi-head attention (B=4, L=2048, C=1024, H=16, D=64) on 8 TRN2 NeuronCores.

Sharding: core c handles batch b = c//2 and head-group hg = c%2 (8 heads).
Megatron-style: w_qkv column-sharded, w_proj row-sharded; the proj all-reduce
(2 cores per batch) happens on the host during unshard.

All-bf16 dataflow (matmul cost: 1.0 cyc/row at any free size, keyed on the
moving operand's dtype; measured end-to-end max rel err ~1e-2 vs 2e-2 gate):

  scores:  S^T[k128, q512] = kT.T @ qT per (kt, hd); one PSUM bank per mm
           (tile_position packs the 2 heads on the 128 partitions).
  exp:     ScalarE Exp (scale 1/8 fused, no max-subtraction; |scores|<~6.5)
           PSUM -> persistent SBUF tile pt[128, 32, 512] bf16 per qtile,
           double-buffered.  The Activation engine is the second wall
           (~267us busy: 218us roofline + 185ns/inst access overhead).
  attn@V:  FLIPPED: O[q128, 65] += pt[:, 2kt+hd, qc*128:+128].T @ V'[k,65]
           over 16 kt; 65 = 64 dims + ones column so the softmax denom
           lands per-partition.  Free size 65 (vs 512 in the O^T form)
           halves attn@V PE cost since cost = output free size.
  norm:    DVE reciprocal [128,1] + per-partition tensor_scalar -> bf16
           O_n[q, 2, 64]; PE transpose via identity matmul -> OT (SBUF,
           no DRAM roundtrip); out-proj consumes OT chunks.
  proj:    out[l128, co512] over 4 pair-chunks; the last l-window is split
           into a pairs-0..2 partial (computed early, parked bf16) plus a
           tail finisher (pair-3 mm + identity-mm folding the partial into
           the same PSUM group) so only ~2 mms/chunk trail the last exp;
           its outputs ship bf16 to halve the final DMA drain.

Scheduling: the PE executes in program order, so all non-score PE work
(V-proj, q/k projections+RoPE for later pairs, attn@V of the previous
qtile, transposes, out-proj) is chopped into ~0.9us chunks and woven
between score groups by a debt/deadline pump ("fillers"), keeping the
Activation engine fed continuously.  Correctness constraint: a qtile's
attn@V must be emitted before the exp two qtiles later reuses its pt pool
slot (emission-order WAR), enforced by epoch deadlines + hard drain.

PSUM budget (8 banks): ps_sc 2 tiles x 2 banks (scores/exp double buffer;
ZERO_REGION = whole bank forbids co-tenant accumulation groups, capping
exp at 1024 elem/inst) + ps_ms 4 x 1 bank shared by qkv-proj psum, the
two attn@V accumulators, transpose dests, and out-proj psum.
"""

import sys

sys.path.insert(0, "/opt/trn_rl_repo")

import numpy as np

B, L, C, H, D = 4, 2048, 1024, 16, 64
NCORES = 8
QT = 512          # q-tile; one score mm per PSUM bank
GRP = 2           # score banks per exp group (exp ap = GRP*QT)
_built = {}


def _build(nc_mod):
    """Build the per-core Bass program (identical on all cores)."""
    import concourse.mybir as mybir
    import concourse.tile as tile
    from concourse import bacc
    from concourse.alu_op_type import AluOpType

    F32 = mybir.dt.float32
    BF16 = mybir.dt.bfloat16
    EXP = mybir.ActivationFunctionType.Exp
    MULT = AluOpType.mult
    ADD = AluOpType.add

    NKC = C // 128          # 8 contraction chunks for qkv proj
    NLT = L // 128          # 16 l-tiles (V rows, proj rows, k-chunks)
    NQT = L // QT           # 4 q-tiles per pair
    NPAIR = 4               # head pairs per core
    FV = 512                # v features per core
    VW = 65                 # V columns incl. ones
    NS = 2 * NLT            # 32 (kt, hd) score slices per qtile

    nc = bacc.Bacc(None, target_bir_lowering=False)

    xT_d = nc.dram_tensor("xT", [128, NKC, L], BF16, kind="ExternalInput")
    wqkT_d = nc.dram_tensor("wqkT", [8, 128, NKC, 128], BF16, kind="ExternalInput")
    wvT_d = nc.dram_tensor("wvT", [128, NKC, FV], BF16, kind="ExternalInput")
    wpT_d = nc.dram_tensor("wpT", [128, NPAIR, C], BF16, kind="ExternalInput")
    cos4_d = nc.dram_tensor("cos4", [128, L], BF16, kind="ExternalInput")
    sin4_d = nc.dram_tensor("sin4", [128, L], BF16, kind="ExternalInput")
    ident_d = nc.dram_tensor("ident", [128, 128], BF16, kind="ExternalInput")
    outp_d = nc.dram_tensor("outp", [NLT, 128, C], F32, kind="ExternalOutput")
    # last-window outputs go out in bf16 so the end-of-kernel DMA drain is
    # half as long (quantization adds ~0.2% of max, within budget)
    outpb_d = nc.dram_tensor("outpb", [4, 128, C], BF16, kind="ExternalOutput")

    SWAP_MASK = list(range(16, 32)) + list(range(16))

    with tile.TileContext(nc) as tc:
        import contextlib

        with contextlib.ExitStack() as outer:
            persist = outer.enter_context(tc.tile_pool(name="persist", bufs=1))
            qk_pool = outer.enter_context(tc.tile_pool(name="qkt", bufs=5))
            pt_pool = outer.enter_context(tc.tile_pool(name="pt", bufs=2))
            on_pool = outer.enter_context(tc.tile_pool(name="on", bufs=4))
            rc_pool = outer.enter_context(tc.tile_pool(name="rc", bufs=6))
            tpool = outer.enter_context(tc.tile_pool(name="tmp", bufs=4))
            ob_pool = outer.enter_context(tc.tile_pool(name="ob", bufs=4))
            oba_pool = outer.enter_context(tc.tile_pool(name="oba", bufs=9))
            ps_sc = outer.enter_context(tc.tile_pool(name="ps_sc", bufs=2, space="PSUM"))
            ps_ms = outer.enter_context(tc.tile_pool(name="ps_ms", bufs=4, space="PSUM"))

            # ---- persistent tensors ----
            V_t = persist.tile([128, NLT, 8, VW], BF16, tag="V")
            OT_t = persist.tile([128, NPAIR, L], BF16, tag="OT")
            xT_t = persist.tile([128, NKC, L], BF16, tag="xT")
            wvT_t = persist.tile([128, NKC, FV], BF16, tag="wv")
            wpT_t = persist.tile([128, NPAIR, C], BF16, tag="wp")
            cos4_t = persist.tile([128, L], BF16, tag="cos")
            sin4_t = persist.tile([128, L], BF16, tag="sin")
            ident_t = persist.tile([128, 128], BF16, tag="id")
            ones_t = persist.tile([128, NLT, 8], BF16, tag="ones")

            # ---- input DMAs (front section; wpT deferred to phase C) ----
            wqk = {}

            def load_wqk(ft, split=False):
                wqk[ft] = qkw_pool.tile([128, NKC, 128], BF16, tag="wqk", name=f"wqk{ft}")
                if split:
                    nc.sync.dma_start(wqk[ft][:, 0:2], wqkT_d[ft, :, 0:2])
                    nc.sync.dma_start(wqk[ft][:, 2:NKC], wqkT_d[ft, :, 2:NKC])
                else:
                    nc.sync.dma_start(wqk[ft][:], wqkT_d[ft])

            qkw_pool = outer.enter_context(tc.tile_pool(name="qkw", bufs=4))
            # DMA order matches first-consumption: wqk0's first chunks, the
            # lq0 x slices, the lq0 cos/sin slices (for the first ropes), the
            # rest of wqk0 and wqk4, then everything else.
            load_wqk(0)
            nc.sync.dma_start(xT_t[:, 0:4, 0:QT], xT_d[:, 0:4, 0:QT])
            nc.sync.dma_start(xT_t[:, 4:NKC, 0:QT], xT_d[:, 4:NKC, 0:QT])
            nc.sync.dma_start(cos4_t[:, 0:QT], cos4_d[:, 0:QT])
            nc.sync.dma_start(sin4_t[:, 0:QT], sin4_d[:, 0:QT])
            load_wqk(4)
            nc.sync.dma_start(cos4_t[:, QT:], cos4_d[:, QT:])
            nc.sync.dma_start(sin4_t[:, QT:], sin4_d[:, QT:])
            for lq in range(1, NQT):
                sl = slice(lq * QT, (lq + 1) * QT)
                nc.sync.dma_start(xT_t[:, :, sl], xT_d[:, :, sl])
            nc.sync.dma_start(wvT_t[:], wvT_d[:])
            nc.sync.dma_start(ident_t[:], ident_d[:])

            # warm the Exp activation table while the input DMAs stream in so
            # the first real exp doesn't pay the 1283ns table load
            warm = tpool.tile([1, 1], F32, tag="warm", name="warm")
            nc.vector.memset(warm[:], 0.0)
            nc.scalar.activation(warm[:], warm[:], EXP)

            nc.vector.memset(ones_t[:], 1.0)
            nc.vector.tensor_copy(V_t[:, :, :, 64:65], ones_t[:, :, :, None])

            # ---- A2 helper: qT/kT for one f-tile (one pair, q or k) ----
            qkT = {}

            qk_acc = {}

            def emit_qk_lq(ft, lq, half=None):
                """Project + rope one 512-l chunk of f-tile ft into qkT[ft].
                half=0 emits the first 4 contraction chunks (PSUM group stays
                open), half=1 finishes and ropes; None does both."""
                if ft not in qkT:
                    qkT[ft] = qk_pool.tile([128, L], BF16, tag="qkt", name=f"qkT{ft}")
                dst = qkT[ft]
                sl = slice(lq * QT, (lq + 1) * QT)
                if half in (0, None):
                    qps = ps_ms.tile([128, QT], F32, tag="ms", name="qps")
                    qk_acc[ft, lq] = qps
                    kcs = range(0, 4 if half == 0 else NKC)
                else:
                    qps = qk_acc.pop((ft, lq))
                    kcs = range(4, NKC)
                for kc in kcs:
                    nc.tensor.matmul(
                        qps[:],
                        wqk[ft][:, kc, :],
                        xT_t[:, kc, sl],
                        start=(kc == 0),
                        stop=(kc == NKC - 1),
                    )
                if half == 0:
                    return
                # RoPE in bf16: dst = qb*cos4 + swap(qb)*sin4s
                qb = tpool.tile([128, QT], BF16, tag="qb")
                shufb = tpool.tile([128, QT], BF16, tag="shufb")
                nc.vector.tensor_copy(qb[:], qps[:])
                nc.vector.stream_shuffle(shufb[:], qb[:], SWAP_MASK)
                nc.vector.tensor_tensor(dst[:, sl], qb[:], cos4_t[:, sl], op=MULT)
                nc.vector.tensor_tensor(shufb[:], shufb[:], sin4_t[:, sl], op=MULT)
                nc.vector.tensor_tensor(dst[:, sl], dst[:, sl], shufb[:], op=ADD)

            a1_acc = {}

            def emit_a1_lt(lt, half=None):
                if half in (0, None):
                    vps = ps_ms.tile([128, FV], F32, tag="ms", name="vps")
                    a1_acc[lt] = vps
                    kcs = range(0, 4 if half == 0 else NKC)
                else:
                    vps = a1_acc.pop(lt)
                    kcs = range(4, NKC)
                for kc in kcs:
                    nc.tensor.matmul(
                        vps[:],
                        xT_t[:, kc, lt * 128:(lt + 1) * 128],
                        wvT_t[:, kc, :],
                        start=(kc == 0),
                        stop=(kc == NKC - 1),
                    )
                if half == 0:
                    return
                nc.vector.tensor_copy(V_t[:, lt, :, 0:64], vps[:])

            nc.sync.dma_start(wpT_t[:], wpT_d[:])

            # ---- phase C helpers: out-proj split into a pairs-0..2 partial
            # (weavable as soon as pair 2 finishes) and a pair-3 finisher so
            # only one matmul per output chunk trails the last attention tile.
            oba = {}

            def emit_c(lt, co):
                """Full 4-pair out-proj chunk + DVE copy + DMA (windows 0-2)."""
                pps = ps_ms.tile([128, QT], F32, tag="ms", name="pps")
                for kd in range(NPAIR):
                    nc.tensor.matmul(
                        pps[:],
                        OT_t[:, kd, lt * 128:(lt + 1) * 128],
                        wpT_t[:, kd, co * QT:(co + 1) * QT],
                        start=(kd == 0),
                        stop=(kd == NPAIR - 1),
                    )
                ob = ob_pool.tile([128, QT], F32, tag="ob")
                nc.vector.tensor_copy(ob[:], pps[:])
                nc.sync.dma_start(outp_d[lt, :, co * QT:(co + 1) * QT], ob[:])

            def emit_ca(lt, co):
                """Pairs 0-2 partial for the last window, parked in bf16."""
                pps = ps_ms.tile([128, QT], F32, tag="ms", name="ppsa")
                for kd in range(NPAIR - 1):
                    nc.tensor.matmul(
                        pps[:],
                        OT_t[:, kd, lt * 128:(lt + 1) * 128],
                        wpT_t[:, kd, co * QT:(co + 1) * QT],
                        start=(kd == 0),
                        stop=(kd == NPAIR - 2),
                    )
                t = oba_pool.tile([128, QT], BF16, tag="oba", name="oba")
                nc.vector.tensor_copy(t[:], pps[:])
                oba[lt, co] = t

            def emit_cb(lt, co):
                """Tail finisher: pair-3 matmul + identity-matmul folding the
                parked partial into the same PSUM group.  Runs only in the
                drain tail, when the score banks are idle — borrow those so
                the av/transpose rotation keeps ps_ms to itself."""
                scb = ps_sc.tile([128, GRP, QT], F32, tag="sc", name="ppsb")
                pps = scb[:, 0, :]
                nc.tensor.matmul(
                    pps[:],
                    OT_t[:, NPAIR - 1, lt * 128:(lt + 1) * 128],
                    wpT_t[:, NPAIR - 1, co * QT:(co + 1) * QT],
                    start=True,
                    stop=False,
                )
                nc.tensor.matmul(
                    pps[:],
                    ident_t[:],
                    oba[lt, co][:],
                    start=False,
                    stop=True,
                )
                ob = ob_pool.tile([128, QT], BF16, tag="ob", name="obb")
                if co == 0:
                    nc.scalar.copy(ob[:], pps[:])
                else:
                    nc.vector.tensor_copy(ob[:], pps[:])
                nc.sync.dma_start(outpb_d[lt - 12, :, co * QT:(co + 1) * QT], ob[:])

            def emit_av(pr, qt, qc, pt, tq):
                """Flipped attn@V for one 128-q chunk (both heads) + norm."""
                av = {}
                for hd in range(2):
                    av[hd] = ps_ms.tile([128, QT], F32, tag="ms", name=f"av{hd}")
                    for kt in range(NLT):
                        nc.tensor.matmul(
                            av[hd][:, 0:VW],
                            pt[:, 2 * kt + hd, qc * 128:(qc + 1) * 128],
                            V_t[:, kt, pr * 2 + hd, :],
                            start=(kt == 0),
                            stop=(kt == NLT - 1),
                        )
                on = on_pool.tile([128, 2, 64], BF16, tag="on")
                for hd in range(2):
                    rc = rc_pool.tile([128, 1], F32, tag="rc")
                    nc.vector.reciprocal(rc[:], av[hd][:, 64:65])
                    nc.vector.tensor_scalar(
                        on[:, hd, :], av[hd][:, 0:64], rc[:], None, op0=MULT
                    )
                tq[qc] = on

            def emit_transpose(pr, qt, qc, on):
                tp = ps_ms.tile([128, QT], F32, tag="ms", name="tp")
                tpb = tp.bitcast(BF16)
                nc.tensor.transpose(
                    tpb[:, 0:128], on[:].rearrange("p a b -> p (a b)"), ident_t[:]
                )
                dst = OT_t[:, pr, qt * QT + qc * 128: qt * QT + (qc + 1) * 128]
                nc.vector.tensor_copy(dst, tpb[:, 0:128])

            # ---- interleaved emission: weave filler PE chunks between score
            # groups so the Activation engine (the per-qtile bottleneck) never
            # starves behind the in-order PE stream.
            import collections

            fillers = collections.deque()  # (cost_cycles, fn, epoch)
            debt = [0.0]
            need = [0.0]   # deadline-critical cycles to force-spread this epoch
            forced = [0.0]
            PUMP = 1400.0  # PE filler cycles per score group (Act group ~1.04us)
            NGROUPS = 2 * (L // 128) // GRP

            def epoch_start(ep):
                # hard drain: entries tagged <= ep-2 must precede this epoch's
                # pt allocation (emission-order WAR on the pt pool slot);
                # normally empty because spreading finished them in ep-1
                while fillers and fillers[0][2] <= ep - 2:
                    _, fn, _ = fillers.popleft()
                    fn()
                # spread target: entries tagged <= ep-1 finish within this
                # epoch, woven across its score groups
                n = 0.0
                for c, _, e in fillers:
                    if e > ep - 1:
                        break
                    n += c
                need[0] = n
                forced[0] = 0.0
                debt[0] = 0.0

            def pump(g):
                # spread deadline-critical work across the epoch's groups,
                # plus opportunistic pumping at the steady rate; cap per-group
                # emission so a filler burst never blocks the score stream
                # long enough to starve the Activation engine
                target = need[0] * (g + 1) / NGROUPS
                cap = max(3000.0, need[0] / NGROUPS + 1200.0)
                debt[0] += PUMP
                emitted = 0.0
                while fillers and emitted < cap and (
                    forced[0] < target or debt[0] >= fillers[0][0]
                ):
                    c, fn, _ = fillers.popleft()
                    fn()
                    forced[0] += c
                    debt[0] -= c
                    emitted += c

            def drain_all():
                while fillers:
                    _, fn, _ = fillers.popleft()
                    fn()

            # A1 V-proj chunks: FIFO-ahead of qt0's AV, deadline epoch 0
            for lt in range(NLT):
                fillers.append((2048, (lambda lt=lt: emit_a1_lt(lt, 0)), 0))
                fillers.append((2200, (lambda lt=lt: emit_a1_lt(lt, 1)), 0))

            # Explicit target schedule for A2 f-tile chunks and phase C:
            # extra[ep] = chunks queued at the END of epoch ep (tag ep, woven
            # during ep+1), chosen to fill otherwise-idle epochs while
            # respecting qkT/qkw pool-slot reuse (a pair's tiles are only
            # reused two pairs later) and OT availability for C.
            extra = collections.defaultdict(list)

            def qk_sched(ft, targets):
                for lq, tgt in enumerate(targets):
                    extra[tgt - 1].append((2048, lambda ft=ft, lq=lq: emit_qk_lq(ft, lq, 0)))
                    extra[tgt - 1].append((2300, lambda ft=ft, lq=lq: emit_qk_lq(ft, lq, 1)))

            # Deadline rule: a chunk targeted at epoch t completes by t's end,
            # so writers must target <= (first consuming epoch) - 1.  Pair p's
            # q-quarter lq is first read at epoch 4p+lq; its k-tile is read
            # from group 0 of epoch 4p (k targets <= 4p-1 strictly).  qkT slot
            # reuse (bufs=5) frees a tile two pairs later; first-chunk epochs
            # respect that.
            qk_sched(1, [2, 3, 4, 5])
            qk_sched(5, [2, 2, 3, 3])
            qk_sched(2, [4, 5, 6, 7])
            qk_sched(6, [6, 6, 7, 7])
            qk_sched(3, [8, 9, 10, 11])
            qk_sched(7, [8, 9, 10, 11])
            WQK_LOAD_EP = {1: 1, 5: 1, 2: 3, 6: 5, 3: 7, 7: 7}
            # C: last window's pairs-0..2 partial woven in ep 12 (pr0-2 OT is
            # complete after ep 11); full windows 0..2 one epoch after their
            # transposes; window-3 finishers drain in the tail.
            for lt in range(3 * NQT, 4 * NQT):
                for co in range(C // QT):
                    extra[11].append((1536, (lambda lt=lt, co=co: emit_ca(lt, co))))
            for w in range(3):
                for lt in range(w * NQT, (w + 1) * NQT):
                    for co in range(C // QT):
                        extra[12 + w].append((2048, (lambda lt=lt, co=co: emit_c(lt, co))))

            # ---- phase B driver ----
            for pr in range(NPAIR):
                for qt in range(NQT):
                    ep = pr * NQT + qt
                    epoch_start(ep)
                    if pr == 0 and qt == 0:
                        emit_qk_lq(0, 0)
                        emit_qk_lq(4, 0)
                    qT_t, kT_t = qkT[pr], qkT[4 + pr]
                    qsl = slice(qt * QT, (qt + 1) * QT)
                    pt = pt_pool.tile([128, NS, QT], BF16, tag="pt", name="pt")
                    for g0 in range(0, NS, GRP):
                        if pr == 0 and qt == 0 and g0 // 2 in (1, 3, 5):
                            # stream pair-0 q/k projection just ahead of the
                            # score chunks that consume it
                            lq = {1: 1, 3: 2, 5: 3}[g0 // 2]
                            emit_qk_lq(0, lq)
                            emit_qk_lq(4, lq)
                        sc = ps_sc.tile([128, GRP, QT], F32, tag="sc")
                        for j in range(GRP):
                            s = g0 + j
                            kt, hd = s // 2, s % 2
                            nc.tensor.matmul(
                                sc[:, j, :],
                                kT_t[hd * 64:(hd + 1) * 64, kt * 128:(kt + 1) * 128],
                                qT_t[hd * 64:(hd + 1) * 64, qsl],
                                start=True,
                                stop=True,
                                tile_position=(hd * 64, 0),
                            )
                        nc.scalar.activation(
                            pt[:, g0:g0 + GRP, :], sc[:], EXP, scale=float(D) ** -0.5
                        )
                        if not (pr == 0 and qt == 0):
                            pump(g0 // GRP)
                        elif g0 // GRP >= 6:
                            # pair-0 q/k emission is done by group 6; start
                            # draining the V-proj backlog under the remaining
                            # score groups
                            pump(g0 // GRP)
                    # queue this qtile's AV + norm + transpose (+ C for pr3):
                    # they weave through the next qtile's score stream and must
                    # be emitted before epoch ep+2 reuses the pt buffer.
                    last_ep = ep == NPAIR * NQT - 1
                    tq = {}
                    for qc in range(QT // 128):
                        fillers.append(
                            (2080, (lambda pr=pr, qt=qt, qc=qc, pt=pt, tq=tq:
                                    emit_av(pr, qt, qc, pt, tq)), ep)
                        )
                        if qc > 0:
                            fillers.append(
                                (150, (lambda pr=pr, qt=qt, qc=qc, tq=tq:
                                       emit_transpose(pr, qt, qc - 1, tq[qc - 1])), ep)
                            )
                            if last_ep:
                                lt = qt * NQT + qc - 1
                                for co in range(C // QT):
                                    fillers.append(
                                        (1024, (lambda lt=lt, co=co: emit_cb(lt, co)), ep)
                                    )
                    fillers.append(
                        (150, (lambda pr=pr, qt=qt, tq=tq:
                               emit_transpose(pr, qt, NQT - 1, tq[NQT - 1])), ep)
                    )
                    if last_ep:
                        lt = qt * NQT + NQT - 1
                        for co in range(C // QT):
                            fillers.append(
                                (1024, (lambda lt=lt, co=co: emit_cb(lt, co)), ep)
                            )
                    for ent in extra.get(ep, ()):
                        fillers.append((ent[0], ent[1], ep))
                    for ft, lep in WQK_LOAD_EP.items():
                        if lep == ep:
                            load_wqk(ft)
            drain_all()

    nc.compile()
    return nc


def _get_nc():
    if "nc" not in _built:
        _built["nc"] = _build(None)
    return _built["nc"]


def _rope_perm():
    """Within-head row permutation: quadrant-local [evens(16) | odds(16)]."""
    perm = np.empty(64, np.int64)
    for j in range(2):
        for i in range(32):
            perm[j * 32 + i] = 2 * (j * 16 + i) if i < 16 else 2 * (j * 16 + i - 16) + 1
    return perm


def _shard_inputs(x, cos, sin, w_qkv, w_proj):
    import ml_dtypes

    Bb = ml_dtypes.bfloat16
    perm = _rope_perm()
    p = np.arange(128)
    quad, i = p // 32, p % 32
    pairidx = (quad % 2) * 16 + (i % 16)
    sign = np.where(i < 16, -1.0, 1.0).astype(np.float32)
    cos4 = np.ascontiguousarray(cos[:, pairidx].T).astype(Bb)              # [128, L]
    sin4 = np.ascontiguousarray((sin[:, pairidx] * sign[None, :]).T).astype(Bb)
    ident = np.eye(128, dtype=np.float32).astype(Bb)

    in_maps = []
    for c in range(NCORES):
        b, hg = c // 2, c % 2
        xT = np.ascontiguousarray(
            x[b].T.reshape(C // 128, 128, L).transpose(1, 0, 2)
        ).astype(Bb)  # [p, kc, l]

        rows = np.empty((8, 128), np.int64)
        for ft in range(8):
            t = 0 if ft < 4 else 1
            pr = ft % 4
            for fi in range(128):
                head = hg * 8 + 2 * pr + (0 if fi < 64 else 1)
                rows[ft, fi] = t * C + head * D + perm[fi % 64]
        wq = w_qkv[rows.reshape(-1)].reshape(8, 128, C // 128, 128)  # [ft, f, kc, p]
        wqkT = np.ascontiguousarray(wq.transpose(0, 3, 2, 1)).astype(Bb)  # [ft,p,kc,f]

        wv = w_qkv[2 * C + hg * 512: 2 * C + hg * 512 + 512]         # [fv, c]
        wvT = np.ascontiguousarray(
            wv.T.reshape(C // 128, 128, 512).transpose(1, 0, 2)
        ).astype(Bb)  # [p, kc, fv]

        wp = w_proj[:, hg * 512: hg * 512 + 512]                     # [co, d']
        wpT = np.ascontiguousarray(
            wp.T.reshape(4, 128, C).transpose(1, 0, 2)
        ).astype(Bb)  # [p, kd, co]

        in_maps.append(
            {
                "xT": xT, "wqkT": wqkT, "wvT": wvT, "wpT": wpT,
                "cos4": cos4, "sin4": sin4, "ident": ident,
            }
        )
    return in_maps


def kernel(x, cos, sin, w_qkv, w_proj, b_proj, _trace=False):
    from concourse.bass_utils import run_bass_kernel_spmd

    x = np.asarray(x, dtype=np.float32)
    cos = np.asarray(cos, dtype=np.float32)
    sin = np.asarray(sin, dtype=np.float32)
    w_qkv = np.asarray(w_qkv, dtype=np.float32)
    w_proj = np.asarray(w_proj, dtype=np.float32)
    b_proj = np.asarray(b_proj, dtype=np.float32)

    nc = _get_nc()
    in_maps = _shard_inputs(x, cos, sin, w_qkv, w_proj)
    res = run_bass_kernel_spmd(
        nc, in_maps, core_ids=list(range(NCORES)), trace=_trace
    )
    if _trace:
        print("exec_time_ns:", res.exec_time_ns)

    out = np.empty((B, L, C), dtype=np.float32)
    for b in range(B):
        p0 = res.results[2 * b]["outp"].reshape(L, C)
        p1 = res.results[2 * b + 1]["outp"].reshape(L, C)
        out[b] = p0 + p1
        p0b = res.results[2 * b]["outpb"].astype(np.float32).reshape(512, C)
        p1b = res.results[2 * b + 1]["outpb"].astype(np.float32).reshape(512, C)
        out[b, 1536:2048] = p0b + p1b
    out += b_proj[None, None, :]
    return out


# revision 75
# speedup vs baseline: 1.3558x; 1.0040x over previous
"""Multi-head attention (B=4, L=2048, C=1024, H=16, D=64) on 8 TRN2 NeuronCores.

Sharding: core c handles batch b = c//2 and head-group hg = c%2 (8 heads).
Megatron-style: w_qkv column-sharded, w_proj row-sharded; the proj all-reduce
(2 cores per batch) happens on the host during unshard.

All-bf16 dataflow (matmul cost: 1.0 cyc/row at any free size, keyed on the
moving operand's dtype; measured end-to-end max rel err ~1e-2 vs 2e-2 gate):

  scores:  S^T[k128, q512] = kT.T @ qT per (kt, hd); one PSUM bank per mm
           (tile_position packs the 2 heads on the 128 partitions).
  exp:     ScalarE Exp (scale 1/8 fused, no max-subtraction; |scores|<~6.5)
           PSUM -> persistent SBUF tile pt[128, 32, 512] bf16 per qtile,
           double-buffered.  The Activation engine is the second wall
           (~267us busy: 218us roofline + 185ns/inst access overhead).
  attn@V:  FLIPPED: O[q128, 65] += pt[:, 2kt+hd, qc*128:+128].T @ V'[k,65]
           over 16 kt; 65 = 64 dims + ones column so the softmax denom
           lands per-partition.  Free size 65 (vs 512 in the O^T form)
           halves attn@V PE cost since cost = output free size.
  norm:    DVE reciprocal [128,1] + per-partition tensor_scalar -> bf16
           O_n[q, 2, 64]; PE transpose via identity matmul -> OT (SBUF,
           no DRAM roundtrip); out-proj consumes OT chunks.
  proj:    out[l128, co512] over 4 pair-chunks; the last l-window is split
           into a pairs-0..2 partial (computed early, parked bf16) plus a
           tail finisher (pair-3 mm + identity-mm folding the partial into
           the same PSUM group) so only ~2 mms/chunk trail the last exp;
           its outputs ship bf16 to halve the final DMA drain.

Scheduling: the PE executes in program order, so all non-score PE work
(V-proj, q/k projections+RoPE for later pairs, attn@V of the previous
qtile, transposes, out-proj) is chopped into ~0.9us chunks and woven
between score groups by a debt/deadline pump ("fillers"), keeping the
Activation engine fed continuously.  Correctness constraint: a qtile's
attn@V must be emitted before the exp two qtiles later reuses its pt pool
slot (emission-order WAR), enforced by epoch deadlines + hard drain.

PSUM budget (8 banks): ps_sc 2 tiles x 2 banks (scores/exp double buffer;
ZERO_REGION = whole bank forbids co-tenant accumulation groups, capping
exp at 1024 elem/inst) + ps_ms 4 x 1 bank shared by qkv-proj psum, the
two attn@V accumulators, transpose dests, and out-proj psum.
"""

import sys

sys.path.insert(0, "/opt/trn_rl_repo")

import numpy as np

B, L, C, H, D = 4, 2048, 1024, 16, 64
NCORES = 8
QT = 512          # q-tile; one score mm per PSUM bank
GRP = 2           # score banks per exp group (exp ap = GRP*QT)
_built = {}


def _build(nc_mod):
    """Build the per-core Bass program (identical on all cores)."""
    import concourse.mybir as mybir
    import concourse.tile as tile
    from concourse import bacc
    from concourse.alu_op_type import AluOpType

    F32 = mybir.dt.float32
    BF16 = mybir.dt.bfloat16
    EXP = mybir.ActivationFunctionType.Exp
    MULT = AluOpType.mult
    ADD = AluOpType.add

    NKC = C // 128          # 8 contraction chunks for qkv proj
    NLT = L // 128          # 16 l-tiles (V rows, proj rows, k-chunks)
    NQT = L // QT           # 4 q-tiles per pair
    NPAIR = 4               # head pairs per core
    FV = 512                # v features per core
    VW = 65                 # V columns incl. ones
    NS = 2 * NLT            # 32 (kt, hd) score slices per qtile

    nc = bacc.Bacc(None, target_bir_lowering=False)

    xT_d = nc.dram_tensor("xT", [128, NKC, L], BF16, kind="ExternalInput")
    wqkT_d = nc.dram_tensor("wqkT", [8, 128, NKC, 128], BF16, kind="ExternalInput")
    wvT_d = nc.dram_tensor("wvT", [128, NKC, FV], BF16, kind="ExternalInput")
    wpT_d = nc.dram_tensor("wpT", [128, NPAIR, C], BF16, kind="ExternalInput")
    cos4_d = nc.dram_tensor("cos4", [128, L], BF16, kind="ExternalInput")
    sin4_d = nc.dram_tensor("sin4", [128, L], BF16, kind="ExternalInput")
    ident_d = nc.dram_tensor("ident", [128, 128], BF16, kind="ExternalInput")
    outp_d = nc.dram_tensor("outp", [NLT, 128, C], F32, kind="ExternalOutput")
    # last-window outputs go out in bf16 so the end-of-kernel DMA drain is
    # half as long (quantization adds ~0.2% of max, within budget)
    outpb_d = nc.dram_tensor("outpb", [4, 128, C], BF16, kind="ExternalOutput")

    SWAP_MASK = list(range(16, 32)) + list(range(16))

    with tile.TileContext(nc) as tc:
        import contextlib

        with contextlib.ExitStack() as outer:
            persist = outer.enter_context(tc.tile_pool(name="persist", bufs=1))
            qk_pool = outer.enter_context(tc.tile_pool(name="qkt", bufs=5))
            pt_pool = outer.enter_context(tc.tile_pool(name="pt", bufs=2))
            on_pool = outer.enter_context(tc.tile_pool(name="on", bufs=4))
            rc_pool = outer.enter_context(tc.tile_pool(name="rc", bufs=6))
            tpool = outer.enter_context(tc.tile_pool(name="tmp", bufs=4))
            ob_pool = outer.enter_context(tc.tile_pool(name="ob", bufs=4))
            oba_pool = outer.enter_context(tc.tile_pool(name="oba", bufs=9))
            ps_sc = outer.enter_context(tc.tile_pool(name="ps_sc", bufs=2, space="PSUM"))
            ps_ms = outer.enter_context(tc.tile_pool(name="ps_ms", bufs=4, space="PSUM"))

            # ---- persistent tensors ----
            V_t = persist.tile([128, NLT, 8, VW], BF16, tag="V")
            OT_t = persist.tile([128, NPAIR, L], BF16, tag="OT")
            xT_t = persist.tile([128, NKC, L], BF16, tag="xT")
            wvT_t = persist.tile([128, NKC, FV], BF16, tag="wv")
            wpT_t = persist.tile([128, NPAIR, C], BF16, tag="wp")
            cos4_t = persist.tile([128, L], BF16, tag="cos")
            sin4_t = persist.tile([128, L], BF16, tag="sin")
            ident_t = persist.tile([128, 128], BF16, tag="id")
            ones_t = persist.tile([128, NLT, 8], BF16, tag="ones")

            # ---- input DMAs (front section; wpT deferred to phase C) ----
            wqk = {}

            def load_wqk(ft, split=False):
                wqk[ft] = qkw_pool.tile([128, NKC, 128], BF16, tag="wqk", name=f"wqk{ft}")
                if split:
                    nc.sync.dma_start(wqk[ft][:, 0:2], wqkT_d[ft, :, 0:2])
                    nc.sync.dma_start(wqk[ft][:, 2:NKC], wqkT_d[ft, :, 2:NKC])
                else:
                    nc.sync.dma_start(wqk[ft][:], wqkT_d[ft])

            qkw_pool = outer.enter_context(tc.tile_pool(name="qkw", bufs=4))
            # DMA order matches first-consumption: wqk0's first chunks, the
            # lq0 x slices, the lq0 cos/sin slices (for the first ropes), the
            # rest of wqk0 and wqk4, then everything else.
            load_wqk(0)
            nc.sync.dma_start(xT_t[:, 0:4, 0:QT], xT_d[:, 0:4, 0:QT])
            nc.sync.dma_start(xT_t[:, 4:NKC, 0:QT], xT_d[:, 4:NKC, 0:QT])
            nc.sync.dma_start(cos4_t[:, 0:QT], cos4_d[:, 0:QT])
            nc.sync.dma_start(sin4_t[:, 0:QT], sin4_d[:, 0:QT])
            load_wqk(4)
            nc.sync.dma_start(cos4_t[:, QT:], cos4_d[:, QT:])
            nc.sync.dma_start(sin4_t[:, QT:], sin4_d[:, QT:])
            for lq in range(1, NQT):
                sl = slice(lq * QT, (lq + 1) * QT)
                nc.sync.dma_start(xT_t[:, :, sl], xT_d[:, :, sl])
            nc.sync.dma_start(wvT_t[:], wvT_d[:])
            nc.sync.dma_start(ident_t[:], ident_d[:])

            # warm the Exp activation table while the input DMAs stream in so
            # the first real exp doesn't pay the 1283ns table load
            warm = tpool.tile([1, 1], F32, tag="warm", name="warm")
            nc.vector.memset(warm[:], 0.0)
            nc.scalar.activation(warm[:], warm[:], EXP)
            # burn the PE p-state ramp (0.65->2.4GHz over ~3us of continuous
            # execution) on dummy matmuls while the first input DMAs land, so
            # the real warmup matmuls run at full clock
            wpe = on_pool.tile([128, 2, 64], BF16, tag="on", name="wpe")
            nc.vector.memset(wpe[:], 0.0)
            wpf = wpe[:].rearrange("p a b -> p (a b)")
            for _ in range(12):
                dps = ps_ms.tile([128, QT], F32, tag="ms", name="dps")
                nc.tensor.matmul(dps[:, 0:128], wpf, wpf, start=True, stop=True)

            nc.vector.memset(ones_t[:], 1.0)
            nc.vector.tensor_copy(V_t[:, :, :, 64:65], ones_t[:, :, :, None])

            # ---- A2 helper: qT/kT for one f-tile (one pair, q or k) ----
            qkT = {}

            qk_acc = {}

            def emit_qk_lq(ft, lq, half=None):
                """Project + rope one 512-l chunk of f-tile ft into qkT[ft].
                half=0 emits the first 4 contraction chunks (PSUM group stays
                open), half=1 finishes and ropes; None does both."""
                if ft not in qkT:
                    qkT[ft] = qk_pool.tile([128, L], BF16, tag="qkt", name=f"qkT{ft}")
                dst = qkT[ft]
                sl = slice(lq * QT, (lq + 1) * QT)
                if half in (0, None):
                    qps = ps_ms.tile([128, QT], F32, tag="ms", name="qps")
                    qk_acc[ft, lq] = qps
                    kcs = range(0, 4 if half == 0 else NKC)
                else:
                    qps = qk_acc.pop((ft, lq))
                    kcs = range(4, NKC)
                for kc in kcs:
                    nc.tensor.matmul(
                        qps[:],
                        wqk[ft][:, kc, :],
                        xT_t[:, kc, sl],
                        start=(kc == 0),
                        stop=(kc == NKC - 1),
                    )
                if half == 0:
                    return
                # RoPE in bf16: dst = qb*cos4 + swap(qb)*sin4s
                qb = tpool.tile([128, QT], BF16, tag="qb")
                shufb = tpool.tile([128, QT], BF16, tag="shufb")
                nc.vector.tensor_copy(qb[:], qps[:])
                nc.vector.stream_shuffle(shufb[:], qb[:], SWAP_MASK)
                nc.vector.tensor_tensor(dst[:, sl], qb[:], cos4_t[:, sl], op=MULT)
                nc.vector.tensor_tensor(shufb[:], shufb[:], sin4_t[:, sl], op=MULT)
                nc.vector.tensor_tensor(dst[:, sl], dst[:, sl], shufb[:], op=ADD)

            a1_acc = {}

            def emit_a1_lt(lt, half=None):
                if half in (0, None):
                    vps = ps_ms.tile([128, FV], F32, tag="ms", name="vps")
                    a1_acc[lt] = vps
                    kcs = range(0, 4 if half == 0 else NKC)
                else:
                    vps = a1_acc.pop(lt)
                    kcs = range(4, NKC)
                for kc in kcs:
                    nc.tensor.matmul(
                        vps[:],
                        xT_t[:, kc, lt * 128:(lt + 1) * 128],
                        wvT_t[:, kc, :],
                        start=(kc == 0),
                        stop=(kc == NKC - 1),
                    )
                if half == 0:
                    return
                nc.vector.tensor_copy(V_t[:, lt, :, 0:64], vps[:])

            nc.sync.dma_start(wpT_t[:], wpT_d[:])

            # ---- phase C helpers: out-proj split into a pairs-0..2 partial
            # (weavable as soon as pair 2 finishes) and a pair-3 finisher so
            # only one matmul per output chunk trails the last attention tile.
            oba = {}

            def emit_c(lt, co):
                """Full 4-pair out-proj chunk + DVE copy + DMA (windows 0-2)."""
                pps = ps_ms.tile([128, QT], F32, tag="ms", name="pps")
                for kd in range(NPAIR):
                    nc.tensor.matmul(
                        pps[:],
                        OT_t[:, kd, lt * 128:(lt + 1) * 128],
                        wpT_t[:, kd, co * QT:(co + 1) * QT],
                        start=(kd == 0),
                        stop=(kd == NPAIR - 1),
                    )
                ob = ob_pool.tile([128, QT], F32, tag="ob")
                nc.vector.tensor_copy(ob[:], pps[:])
                nc.sync.dma_start(outp_d[lt, :, co * QT:(co + 1) * QT], ob[:])

            def emit_ca(lt, co):
                """Pairs 0-2 partial for the last window, parked in bf16."""
                pps = ps_ms.tile([128, QT], F32, tag="ms", name="ppsa")
                for kd in range(NPAIR - 1):
                    nc.tensor.matmul(
                        pps[:],
                        OT_t[:, kd, lt * 128:(lt + 1) * 128],
                        wpT_t[:, kd, co * QT:(co + 1) * QT],
                        start=(kd == 0),
                        stop=(kd == NPAIR - 2),
                    )
                t = oba_pool.tile([128, QT], BF16, tag="oba", name="oba")
                nc.vector.tensor_copy(t[:], pps[:])
                oba[lt, co] = t

            def emit_cb(lt, co):
                """Tail finisher: pair-3 matmul + identity-matmul folding the
                parked partial into the same PSUM group.  Runs only in the
                drain tail, when the score banks are idle — borrow those so
                the av/transpose rotation keeps ps_ms to itself."""
                scb = ps_sc.tile([128, GRP, QT], F32, tag="sc", name="ppsb")
                pps = scb[:, 0, :]
                nc.tensor.matmul(
                    pps[:],
                    OT_t[:, NPAIR - 1, lt * 128:(lt + 1) * 128],
                    wpT_t[:, NPAIR - 1, co * QT:(co + 1) * QT],
                    start=True,
                    stop=False,
                )
                nc.tensor.matmul(
                    pps[:],
                    ident_t[:],
                    oba[lt, co][:],
                    start=False,
                    stop=True,
                )
                ob = ob_pool.tile([128, QT], BF16, tag="ob", name="obb")
                if co == 0:
                    nc.scalar.copy(ob[:], pps[:])
                else:
                    nc.vector.tensor_copy(ob[:], pps[:])
                nc.sync.dma_start(outpb_d[lt - 12, :, co * QT:(co + 1) * QT], ob[:])

            def emit_av(pr, qt, qc, pt, tq):
                """Flipped attn@V for one 128-q chunk (both heads) + norm."""
                av = {}
                for hd in range(2):
                    av[hd] = ps_ms.tile([128, QT], F32, tag="ms", name=f"av{hd}")
                    for kt in range(NLT):
                        nc.tensor.matmul(
                            av[hd][:, 0:VW],
                            pt[:, 2 * kt + hd, qc * 128:(qc + 1) * 128],
                            V_t[:, kt, pr * 2 + hd, :],
                            start=(kt == 0),
                            stop=(kt == NLT - 1),
                        )
                on = on_pool.tile([128, 2, 64], BF16, tag="on")
                for hd in range(2):
                    rc = rc_pool.tile([128, 1], F32, tag="rc")
                    nc.vector.reciprocal(rc[:], av[hd][:, 64:65])
                    nc.vector.tensor_scalar(
                        on[:, hd, :], av[hd][:, 0:64], rc[:], None, op0=MULT
                    )
                tq[qc] = on

            def emit_transpose(pr, qt, qc, on):
                tp = ps_ms.tile([128, QT], F32, tag="ms", name="tp")
                tpb = tp.bitcast(BF16)
                nc.tensor.transpose(
                    tpb[:, 0:128], on[:].rearrange("p a b -> p (a b)"), ident_t[:]
                )
                dst = OT_t[:, pr, qt * QT + qc * 128: qt * QT + (qc + 1) * 128]
                nc.vector.tensor_copy(dst, tpb[:, 0:128])

            # ---- interleaved emission: weave filler PE chunks between score
            # groups so the Activation engine (the per-qtile bottleneck) never
            # starves behind the in-order PE stream.
            import collections

            fillers = collections.deque()  # (cost_cycles, fn, epoch)
            debt = [0.0]
            need = [0.0]   # deadline-critical cycles to force-spread this epoch
            forced = [0.0]
            PUMP = 1400.0  # PE filler cycles per score group (Act group ~1.04us)
            NGROUPS = 2 * (L // 128) // GRP

            def epoch_start(ep):
                # hard drain: entries tagged <= ep-2 must precede this epoch's
                # pt allocation (emission-order WAR on the pt pool slot);
                # normally empty because spreading finished them in ep-1
                while fillers and fillers[0][2] <= ep - 2:
                    _, fn, _ = fillers.popleft()
                    fn()
                # spread target: entries tagged <= ep-1 finish within this
                # epoch, woven across its score groups
                n = 0.0
                for c, _, e in fillers:
                    if e > ep - 1:
                        break
                    n += c
                need[0] = n
                forced[0] = 0.0
                debt[0] = 0.0

            def pump(g):
                # spread deadline-critical work across the epoch's groups,
                # plus opportunistic pumping at the steady rate; cap per-group
                # emission so a filler burst never blocks the score stream
                # long enough to starve the Activation engine
                target = need[0] * (g + 1) / NGROUPS
                cap = max(3000.0, need[0] / NGROUPS + 1200.0)
                debt[0] += PUMP
                emitted = 0.0
                while fillers and emitted < cap and (
                    forced[0] < target or debt[0] >= fillers[0][0]
                ):
                    c, fn, _ = fillers.popleft()
                    fn()
                    forced[0] += c
                    debt[0] -= c
                    emitted += c

            def drain_all():
                while fillers:
                    _, fn, _ = fillers.popleft()
                    fn()

            # A1 V-proj chunks: FIFO-ahead of qt0's AV, deadline epoch 0
            for lt in range(NLT):
                fillers.append((2048, (lambda lt=lt: emit_a1_lt(lt, 0)), 0))
                fillers.append((2200, (lambda lt=lt: emit_a1_lt(lt, 1)), 0))

            # Explicit target schedule for A2 f-tile chunks and phase C:
            # extra[ep] = chunks queued at the END of epoch ep (tag ep, woven
            # during ep+1), chosen to fill otherwise-idle epochs while
            # respecting qkT/qkw pool-slot reuse (a pair's tiles are only
            # reused two pairs later) and OT availability for C.
            extra = collections.defaultdict(list)

            def qk_sched(ft, targets):
                for lq, tgt in enumerate(targets):
                    extra[tgt - 1].append((2048, lambda ft=ft, lq=lq: emit_qk_lq(ft, lq, 0)))
                    extra[tgt - 1].append((2300, lambda ft=ft, lq=lq: emit_qk_lq(ft, lq, 1)))

            # Deadline rule: a chunk targeted at epoch t completes by t's end,
            # so writers must target <= (first consuming epoch) - 1.  Pair p's
            # q-quarter lq is first read at epoch 4p+lq; its k-tile is read
            # from group 0 of epoch 4p (k targets <= 4p-1 strictly).  qkT slot
            # reuse (bufs=5) frees a tile two pairs later; first-chunk epochs
            # respect that.
            qk_sched(1, [2, 3, 4, 5])
            qk_sched(5, [2, 2, 3, 3])
            qk_sched(2, [4, 5, 6, 7])
            qk_sched(6, [6, 6, 7, 7])
            qk_sched(3, [8, 9, 10, 11])
            qk_sched(7, [8, 9, 10, 11])
            WQK_LOAD_EP = {1: 1, 5: 1, 2: 3, 6: 5, 3: 7, 7: 7}
            # C: last window's pairs-0..2 partial woven in ep 12 (pr0-2 OT is
            # complete after ep 11); full windows 0..2 one epoch after their
            # transposes; window-3 finishers drain in the tail.
            for lt in range(3 * NQT, 4 * NQT):
                for co in range(C // QT):
                    extra[11].append((1536, (lambda lt=lt, co=co: emit_ca(lt, co))))
            for w in range(3):
                for lt in range(w * NQT, (w + 1) * NQT):
                    for co in range(C // QT):
                        extra[12 + w].append((2048, (lambda lt=lt, co=co: emit_c(lt, co))))

            # ---- phase B driver ----
            for pr in range(NPAIR):
                for qt in range(NQT):
                    ep = pr * NQT + qt
                    epoch_start(ep)
                    if pr == 0 and qt == 0:
                        emit_qk_lq(0, 0)
                        emit_qk_lq(4, 0)
                    qT_t, kT_t = qkT[pr], qkT[4 + pr]
                    qsl = slice(qt * QT, (qt + 1) * QT)
                    pt = pt_pool.tile([128, NS, QT], BF16, tag="pt", name="pt")
                    for g0 in range(0, NS, GRP):
                        if pr == 0 and qt == 0 and g0 // 2 in (1, 3, 5):
                            # stream pair-0 q/k projection just ahead of the
                            # score chunks that consume it
                            lq = {1: 1, 3: 2, 5: 3}[g0 // 2]
                            emit_qk_lq(0, lq)
                            emit_qk_lq(4, lq)
                        sc = ps_sc.tile([128, GRP, QT], F32, tag="sc")
                        for j in range(GRP):
                            s = g0 + j
                            kt, hd = s // 2, s % 2
                            nc.tensor.matmul(
                                sc[:, j, :],
                                kT_t[hd * 64:(hd + 1) * 64, kt * 128:(kt + 1) * 128],
                                qT_t[hd * 64:(hd + 1) * 64, qsl],
                                start=True,
                                stop=True,
                                tile_position=(hd * 64, 0),
                            )
                        nc.scalar.activation(
                            pt[:, g0:g0 + GRP, :], sc[:], EXP, scale=float(D) ** -0.5
                        )
                        if not (pr == 0 and qt == 0):
                            pump(g0 // GRP)
                        elif g0 // GRP >= 6:
                            # pair-0 q/k emission is done by group 6; start
                            # draining the V-proj backlog under the remaining
                            # score groups
                            pump(g0 // GRP)
                    # queue this qtile's AV + norm + transpose (+ C for pr3):
                    # they weave through the next qtile's score stream and must
                    # be emitted before epoch ep+2 reuses the pt buffer.
                    last_ep = ep == NPAIR * NQT - 1
                    tq = {}
                    for qc in range(QT // 128):
                        fillers.append(
                            (2080, (lambda pr=pr, qt=qt, qc=qc, pt=pt, tq=tq:
                                    emit_av(pr, qt, qc, pt, tq)), ep)
                        )
                        if qc > 0:
                            fillers.append(
                                (150, (lambda pr=pr, qt=qt, qc=qc, tq=tq:
                                       emit_transpose(pr, qt, qc - 1, tq[qc - 1])), ep)
                            )
                            if last_ep:
                                lt = qt * NQT + qc - 1
                                for co in range(C // QT):
                                    fillers.append(
                                        (1024, (lambda lt=lt, co=co: emit_cb(lt, co)), ep)
                                    )
                    fillers.append(
                        (150, (lambda pr=pr, qt=qt, tq=tq:
                               emit_transpose(pr, qt, NQT - 1, tq[NQT - 1])), ep)
                    )
                    if last_ep:
                        lt = qt * NQT + NQT - 1
                        for co in range(C // QT):
                            fillers.append(
                                (1024, (lambda lt=lt, co=co: emit_cb(lt, co)), ep)
                            )
                    for ent in extra.get(ep, ()):
                        fillers.append((ent[0], ent[1], ep))
                    for ft, lep in WQK_LOAD_EP.items():
                        if lep == ep:
                            load_wqk(ft)
            drain_all()

    nc.compile()
    return nc


def _get_nc():
    if "nc" not in _built:
        _built["nc"] = _build(None)
    return _built["nc"]


def _rope_perm():
    """Within-head row permutation: quadrant-local [evens(16) | odds(16)]."""
    perm = np.empty(64, np.int64)
    for j in range(2):
        for i in range(32):
            perm[j * 32 + i] = 2 * (j * 16 + i) if i < 16 else 2 * (j * 16 + i - 16) + 1
    return perm


def _shard_inputs(x, cos, sin, w_qkv, w_proj):
    import ml_dtypes

    Bb = ml_dtypes.bfloat16
    perm = _rope_perm()
    p = np.arange(128)
    quad, i = p // 32, p % 32
    pairidx = (quad % 2) * 16 + (i % 16)
    sign = np.where(i < 16, -1.0, 1.0).astype(np.float32)
    cos4 = np.ascontiguousarray(cos[:, pairidx].T).astype(Bb)              # [128, L]
    sin4 = np.ascontiguousarray((sin[:, pairidx] * sign[None, :]).T).astype(Bb)
    ident = np.eye(128, dtype=np.float32).astype(Bb)

    in_maps = []
    for c in range(NCORES):
        b, hg = c // 2, c % 2
        xT = np.ascontiguousarray(
            x[b].T.reshape(C // 128, 128, L).transpose(1, 0, 2)
        ).astype(Bb)  # [p, kc, l]

        rows = np.empty((8, 128), np.int64)
        for ft in range(8):
            t = 0 if ft < 4 else 1
            pr = ft % 4
            for fi in range(128):
                head = hg * 8 + 2 * pr + (0 if fi < 64 else 1)
                rows[ft, fi] = t * C + head * D + perm[fi % 64]
        wq = w_qkv[rows.reshape(-1)].reshape(8, 128, C // 128, 128)  # [ft, f, kc, p]
        wqkT = np.ascontiguousarray(wq.transpose(0, 3, 2, 1)).astype(Bb)  # [ft,p,kc,f]

        wv = w_qkv[2 * C + hg * 512: 2 * C + hg * 512 + 512]         # [fv, c]
        wvT = np.ascontiguousarray(
            wv.T.reshape(C // 128, 128, 512).transpose(1, 0, 2)
        ).astype(Bb)  # [p, kc, fv]

        wp = w_proj[:, hg * 512: hg * 512 + 512]                     # [co, d']
        wpT = np.ascontiguousarray(
            wp.T.reshape(4, 128, C).transpose(1, 0, 2)
        ).astype(Bb)  # [p, kd, co]

        in_maps.append(
            {
                "xT": xT, "wqkT": wqkT, "wvT": wvT, "wpT": wpT,
                "cos4": cos4, "sin4": sin4, "ident": ident,
            }
        )
    return in_maps


def kernel(x, cos, sin, w_qkv, w_proj, b_proj, _trace=False):
    from concourse.bass_utils import run_bass_kernel_spmd

    x = np.asarray(x, dtype=np.float32)
    cos = np.asarray(cos, dtype=np.float32)
    sin = np.asarray(sin, dtype=np.float32)
    w_qkv = np.asarray(w_qkv, dtype=np.float32)
    w_proj = np.asarray(w_proj, dtype=np.float32)
    b_proj = np.asarray(b_proj, dtype=np.float32)

    nc = _get_nc()
    in_maps = _shard_inputs(x, cos, sin, w_qkv, w_proj)
    res = run_bass_kernel_spmd(
        nc, in_maps, core_ids=list(range(NCORES)), trace=_trace
    )
    if _trace:
        print("exec_time_ns:", res.exec_time_ns)

    out = np.empty((B, L, C), dtype=np.float32)
    for b in range(B):
        p0 = res.results[2 * b]["outp"].reshape(L, C)
        p1 = res.results[2 * b + 1]["outp"].reshape(L, C)
        out[b] = p0 + p1
        p0b = res.results[2 * b]["outpb"].astype(np.float32).reshape(512, C)
        p1b = res.results[2 * b + 1]["outpb"].astype(np.float32).reshape(512, C)
        out[b, 1536:2048] = p0b + p1b
    out += b_proj[None, None, :]
    return out
